# revision 1
# baseline (speedup 1.0000x reference)
"""Trainium2 Bass kernel for nn_DiffusionInteractionBlock (GNN message passing).

Strategy: shard EDGES by receiver node range across 8 cores (receiver-sharded
edge parallelism).  Each core owns nodes [c*1250, (c+1)*1250) and processes
exactly the edges whose receiver lands in its range, so the segment-sum is
fully local and NO collective is needed.  Each core redundantly computes the
node-level linear tables (cheap), gathers per-edge sender/receiver rows with
dma_gather, runs the edge MLP + tensor product on-chip, and scatter-sums
messages into PSUM via one-hot matmuls.  Final e3nn linear also on-chip.

Host-side prep (inside kernel()): sorting edges by (core, node-tile),
padding each (core, tile) edge list to a uniform block count so all 8 cores
run the same program (SPMD), packing per-edge side arrays, and folding /
pre-scaling weight matrices.
"""

import os
import sys

import numpy as np

sys.path.insert(0, "/opt/trn_rl_repo")

import ml_dtypes

from concourse import bacc, bass, mybir, tile
from concourse.bass_utils import run_bass_kernel_spmd

BF16 = ml_dtypes.bfloat16

N = 10000
E = 160000
MUL = 128
NCORES = 8
NPC = N // NCORES  # 1250 nodes per core
NT = 10            # node tiles of 128 per core (1280 >= 1250)
NPAD = 79 * 128    # padded node count for the tables (10112)
SQ3 = float(np.sqrt(3.0))
INV = 1.0 / np.sqrt(MUL)
OUT_SCALE = 1.0 / (np.sqrt(2 * MUL) * 16.0)

dt = mybir.dt

_compiled = {}


# --------------------------------------------------------------------------
# Host-side preprocessing
# --------------------------------------------------------------------------

def _host_prep(inputs):
    node_feats = np.asarray(inputs["node_feats"], np.float32)
    edge_attrs = np.asarray(inputs["edge_attrs"], np.float32)
    edge_feats = np.asarray(inputs["edge_feats"], np.float32)
    lengths = np.asarray(inputs["lengths"], np.float32)
    edge_index = np.asarray(inputs["edge_index"], np.int64)
    W_scalar = np.asarray(inputs["W_scalar"], np.float32)
    W_up0 = np.asarray(inputs["W_up0"], np.float32)
    W_up1 = np.asarray(inputs["W_up1"], np.float32)
    W1 = np.asarray(inputs["W1"], np.float32)
    b1 = np.asarray(inputs["b1"], np.float32)
    W2 = np.asarray(inputs["W2"], np.float32)
    b2 = np.asarray(inputs["b2"], np.float32)
    W3 = np.asarray(inputs["W3"], np.float32)
    Wout0 = np.asarray(inputs["Wout0"], np.float32)
    Wout1 = np.asarray(inputs["Wout1"], np.float32)

    sender, receiver = edge_index[0], edge_index[1]

    # global tile id: core * NT + local tile
    loc = receiver % NPC
    gtile = (receiver // NPC) * NT + loc // 128
    counts = np.bincount(gtile, minlength=NCORES * NT)
    B_pad = int(np.ceil(counts.max() / 128))
    EPT = 128 * B_pad                 # edges per node-tile (padded)
    EPC = NT * EPT                    # edges per core (padded)

    # edge ids grouped by gtile; -1 marks padding
    order = np.argsort(gtile, kind="stable")
    epos = np.full((NCORES * NT, EPT), -1, np.int64)
    off = 0
    for g in range(NCORES * NT):
        c = counts[g]
        epos[g, :c] = order[off:off + c]
        off += c

    # per-edge padded values (pad: sender/receiver -> node 0, y/tail -> 0)
    valid = epos >= 0
    ep = np.where(valid, epos, 0)
    es = np.where(valid, sender[ep], 0).astype(np.int32)        # [G, EPT]
    er = np.where(valid, receiver[ep], 0).astype(np.int32)
    rl = np.where(valid, loc[ep] % 128, 0).astype(np.float32)   # local id in tile
    y = np.where(valid[..., None], edge_attrs[ep], 0.0)         # [G, EPT, 4]
    ef = np.where(valid[..., None], edge_feats[ep], 0.0)        # [G, EPT, 8]
    ln = np.where(valid[..., None], lengths[ep], 0.0)           # [G, EPT, 1]
    one = valid.astype(np.float32)[..., None]                   # bias row (0 on pads)

    # --- per-core device arrays ---
    G = NCORES * NT

    def wrap_idx(a):  # [G, EPT] -> [NCORES, 128, NT*B_pad] int32 (col = t*B_pad+b)
        a = a.astype(np.int32).reshape(NCORES, NT, B_pad, 128)
        a = np.transpose(a, (0, 3, 1, 2)).reshape(NCORES, 128, NT * B_pad)
        return np.ascontiguousarray(a)

    idx_s = wrap_idx(es)
    idx_r = wrap_idx(er)

    # tail10 [NCORES, NT, 10, EPT]: rows = [ef(8), len, indicator] per edge
    tail = np.concatenate([ef, ln, one], axis=-1)               # [G, EPT, 10]
    tail10 = np.transpose(tail.reshape(NCORES, NT, EPT, 10), (0, 1, 3, 2))
    tail10 = np.ascontiguousarray(tail10.astype(BF16))

    # yrl [NCORES, NT, 128, 5, B_pad]: f=0:y0, f=1..3:y1_i, f=4:recvloc
    yv = np.concatenate([y, rl[..., None]], axis=-1)            # [G, EPT, 5]
    yv = yv.reshape(NCORES, NT, B_pad, 128, 5)                  # j = b*128 + e
    yrl = np.transpose(yv, (0, 1, 3, 4, 2))                     # [NC, NT, 128, 5, B]
    yrl = np.ascontiguousarray(yrl.astype(np.float32))

    # --- shared (replicated) arrays ---
    x0 = node_feats[:, :MUL]                                    # [N,128]
    x1 = node_feats[:, MUL:].reshape(N, MUL, 3)
    nfT = np.zeros((512, NPAD), np.float32)
    nfT[0:128, :N] = x0.T
    for i in range(3):
        nfT[128 * (i + 1):128 * (i + 2), :N] = x1[:, :, i].T
    nfT = nfT.astype(BF16)

    Ws_inv = W_scalar * INV
    weights = {
        "Wup0": (W_up0 * INV).astype(BF16),
        "Wup1": (W_up1 * INV).astype(BF16),
        "WPs": (Ws_inv @ W1[:MUL]).astype(BF16),
        "WPr": (Ws_inv @ W1[MUL:2 * MUL]).astype(BF16),
        "W1c10": np.concatenate([W1[2 * MUL:], b1[None, :]], 0).astype(BF16),  # [10,128]
        "W2b": W2.astype(BF16),
        "b2row": b2[None, :].astype(BF16),
        "W3b": np.concatenate(
            [W3[:, :MUL], W3[:, MUL:2 * MUL] / SQ3,
             W3[:, 2 * MUL:3 * MUL], W3[:, 3 * MUL:]], axis=1).astype(BF16),   # [128,512]
        "Wout0t": (Wout0[:MUL] * OUT_SCALE).astype(BF16),
        "Wout0b": (Wout0[MUL:] * OUT_SCALE).astype(BF16),
        "Wout1t": (Wout1[:MUL] * OUT_SCALE).astype(BF16),
        "Wout1b": (Wout1[MUL:] * OUT_SCALE).astype(BF16),
        "identf": np.eye(128, dtype=np.float32),
        "ones1": np.ones((1, 128), BF16),
        "iota": np.tile(np.arange(128, dtype=np.float32), (128, 1)),
        "nfT": nfT,
    }

    # transposed one-hot S_T[n, e] = (recvloc[e] == n), per (core, tile)
    rl_i = np.where(valid, loc[ep] % 128, 0).astype(np.int64)   # [G, EPT]
    sT = (rl_i[:, None, :] == np.arange(128)[None, :, None]).astype(BF16)
    sT = np.ascontiguousarray(sT.reshape(NCORES, NT, 128, EPT))

    tid = np.zeros((NCORES, 128, NT), np.int32)
    for c in range(NCORES):
        for t in range(NT):
            tid[c, :, t] = c * NPC + t * 128 + np.arange(128)
    tid = np.minimum(tid, NPAD - 1)

    in_maps = []
    for c in range(NCORES):
        m = dict(weights)
        m["idx_s"] = idx_s[c]
        m["idx_t"] = tid[c]
        m["sT"] = sT[c].reshape(NT * 128, EPT)
        m["tail10"] = tail10[c].reshape(NT * 10, EPT)
        m["yrl"] = yrl[c].reshape(NT * 128, 5 * B_pad)
        in_maps.append(m)
    return B_pad, in_maps


# --------------------------------------------------------------------------
# Device program
# --------------------------------------------------------------------------

DEBUG_TAPS = False
USE_SILU = os.environ.get("BASS_NO_SILU", "") != "1"


def _build(B_pad):
    EPT = 128 * B_pad
    nc = bacc.Bacc("TRN2", target_bir_lowering=False, debug=False,
                   num_devices=NCORES)

    f32, bf16, i16, i32 = dt.float32, dt.bfloat16, dt.int16, dt.int32

    # inputs
    def din(name, shape, dtype):
        return nc.dram_tensor(name, list(shape), dtype, kind="ExternalInput")

    nfT = din("nfT", [512, NPAD], bf16)
    Wup0 = din("Wup0", [128, 128], bf16)
    Wup1 = din("Wup1", [128, 128], bf16)
    WPs = din("WPs", [128, 128], bf16)
    WPr = din("WPr", [128, 128], bf16)
    W1c10 = din("W1c10", [10, 128], bf16)
    W2b = din("W2b", [128, 128], bf16)
    b2row = din("b2row", [1, 128], bf16)
    W3b = din("W3b", [128, 512], bf16)
    Wout0t = din("Wout0t", [128, 128], bf16)
    Wout0b = din("Wout0b", [128, 128], bf16)
    Wout1t = din("Wout1t", [128, 128], bf16)
    Wout1b = din("Wout1b", [128, 128], bf16)
    identf = din("identf", [128, 128], f32)
    ones1 = din("ones1", [1, 128], bf16)
    iota = din("iota", [128, 128], f32)
    idx_s = din("idx_s", [128, NT * B_pad], i32)
    sT_d = din("sT", [NT * 128, EPT], bf16)
    idx_t = din("idx_t", [128, NT], i32)
    tail10 = din("tail10", [NT * 10, EPT], bf16)
    yrl = din("yrl", [NT * 128, 5 * B_pad], f32)

    out_d = nc.dram_tensor("out_dram", [NT * 128, 512], f32, kind="ExternalOutput")
    dbg = {}
    if DEBUG_TAPS:
        dbg["gs"] = nc.dram_tensor("dbg_gs", [128, EPT * 5], f32, kind="ExternalOutput")
        dbg["h1"] = nc.dram_tensor("dbg_h1", [128, 512], f32, kind="ExternalOutput")
        dbg["tpw"] = nc.dram_tensor("dbg_tpw", [128, 512], f32, kind="ExternalOutput")
        dbg["pa"] = nc.dram_tensor("dbg_pa", [128, 1024], f32, kind="ExternalOutput")
        dbg["agg"] = nc.dram_tensor("dbg_agg", [128, 1024], f32, kind="ExternalOutput")

    # internal DRAM node tables
    T_s = nc.dram_tensor("T_s", [NPAD, 640], bf16)
    T_r = nc.dram_tensor("T_r", [NPAD, 128], bf16)

    AL = mybir.AluOpType
    AF = mybir.ActivationFunctionType

    with tile.TileContext(nc) as tc:
        with (
            tc.tile_pool(name="const", bufs=1) as cp,
            tc.tile_pool(name="work", bufs=2) as wp,
            tc.tile_pool(name="gath", bufs=2) as gp,
            tc.tile_pool(name="psB", bufs=1, space="PSUM") as psB,
            tc.tile_pool(name="psC", bufs=2, space="PSUM") as psC,
            tc.tile_pool(name="psAgg", bufs=1, space="PSUM") as psAgg,
        ):
            # ---- load constants to SBUF ----
            def ld(src, p, fdim, dtype, pool=cp):
                t = pool.tile([p, fdim], dtype, tag=f"c_{src.name}")
                nc.sync.dma_start(out=t[:, :], in_=src[:, :])
                return t

            wup0_s = ld(Wup0, 128, 128, bf16)
            wup1_s = ld(Wup1, 128, 128, bf16)
            wps_s = ld(WPs, 128, 128, bf16)
            wpr_s = ld(WPr, 128, 128, bf16)
            w1c_s = ld(W1c10, 10, 128, bf16)
            w2_s = ld(W2b, 128, 128, bf16)
            b2_s = ld(b2row, 1, 128, bf16)
            w3_s = ld(W3b, 128, 512, bf16)
            wo0t_s = ld(Wout0t, 128, 128, bf16)
            wo0b_s = ld(Wout0b, 128, 128, bf16)
            wo1t_s = ld(Wout1t, 128, 128, bf16)
            wo1b_s = ld(Wout1b, 128, 128, bf16)
            idf_s = ld(identf, 128, 128, f32)
            ones_s = ld(ones1, 1, 128, bf16)
            iota_s = ld(iota, 128, 128, f32)
            idb_s = cp.tile([128, 128], bf16, tag="c_idb")
            nc.vector.tensor_copy(out=idb_s[:, :], in_=idf_s[:, :])
            zr_s = cp.tile([128, 128], bf16, tag="c_zr")
            nc.vector.memset(zr_s[:, :], 0.0)
            ixs_s = ld(idx_s, 128, NT * B_pad, i32)
            ixt_s = ld(idx_t, 128, NT, i32)

            # ---- node-table phase ----
            with tc.tile_pool(name="nodes", bufs=2) as npool:
              SC = 2048
              for c0 in range(0, NPAD, SC):
                csz = min(SC, NPAD - c0)
                x0t = npool.tile([128, SC], bf16, tag="nf0")
                x1t0 = npool.tile([128, SC], bf16, tag="nf1")
                x1t1 = npool.tile([128, SC], bf16, tag="nf2")
                x1t2 = npool.tile([128, SC], bf16, tag="nf3")
                for k, t in enumerate([x0t, x1t0, x1t1, x1t2]):
                    nc.sync.dma_start(
                        out=t[:, :csz],
                        in_=nfT[128 * k:128 * (k + 1), c0:c0 + csz])
                for s in range(csz // 128):
                    sl = slice(128 * s, 128 * (s + 1))
                    gsl = slice(c0 + 128 * s, c0 + 128 * (s + 1))
                    pn = psAgg.tile([128, 1024], f32, tag="agg")
                    mms = [
                        (x0t, wup0_s, 0),      # h0
                        (x1t0, wup1_s, 128),   # h1_0
                        (x1t1, wup1_s, 256),
                        (x1t2, wup1_s, 384),
                        (x0t, wps_s, 512),     # P_s
                        (x0t, wpr_s, 640),     # P_r
                    ]
                    for lhs, rhs, o in mms:
                        nc.tensor.matmul(out=pn[:, o:o + 128], lhsT=lhs[:, sl],
                                         rhs=rhs[:, :], start=True, stop=True)
                    tsb = wp.tile([128, 640], bf16, tag="tsb")
                    trb = wp.tile([128, 128], bf16, tag="trb")
                    nc.vector.tensor_copy(out=tsb[:, :], in_=pn[:, 0:640])
                    nc.vector.tensor_copy(out=trb[:, :], in_=pn[:, 640:768])
                    nc.sync.dma_start(out=T_s[gsl, :], in_=tsb[:, :])
                    nc.sync.dma_start(out=T_r[gsl, :], in_=trb[:, :])

            # ---- edge phase ----
            BB = 4  # blocks per batch-group
            for t in range(NT):
                gs_t = gp.tile([128, B_pad * 640], bf16, tag="gs")
                for b in range(B_pad):
                    col = t * B_pad + b
                    nc.gpsimd.indirect_dma_start(
                        out=gs_t[:, 640 * b:640 * (b + 1)], out_offset=None,
                        in_=T_s[:, :],
                        in_offset=bass.IndirectOffsetOnAxis(
                            ap=ixs_s[:, col:col + 1], axis=0))
                st_t = gp.tile([128, EPT], bf16, tag="stT")
                nc.sync.dma_start(out=st_t[:, :],
                                  in_=sT_d[t * 128:(t + 1) * 128, :])
                prt = gp.tile([128, 128], bf16, tag="prt")
                nc.gpsimd.indirect_dma_start(
                    out=prt[:, :], out_offset=None, in_=T_r[:, :],
                    in_offset=bass.IndirectOffsetOnAxis(
                        ap=ixt_s[:, t:t + 1], axis=0))
                if DEBUG_TAPS and t == 0:
                    nc.gpsimd.dma_start(out=dbg["gs"][:, :], in_=gs_t[:, :])
                tl_t = wp.tile([10, EPT], bf16, tag="tail")
                nc.sync.dma_start(out=tl_t[:, :], in_=tail10[t * 10:(t + 1) * 10, :])
                y_t = wp.tile([128, 5 * B_pad], f32, tag="yrl")
                nc.sync.dma_start(out=y_t[:, :], in_=yrl[t * 128:(t + 1) * 128, :])

                # selection matrices
                sp_t = wp.tile([128, B_pad * 128], bf16, tag="spl")
                rl3 = y_t[:, 4 * B_pad:5 * B_pad].unsqueeze(2)
                nc.vector.tensor_tensor(
                    out=sp_t[:, :].rearrange("p (b n) -> p b n", n=128),
                    in0=rl3.to_broadcast([128, B_pad, 128]),
                    in1=iota_s[:, :].unsqueeze(1).to_broadcast(
                        [128, B_pad, 128]),
                    op=AL.is_equal)
                sy_t = wp.tile([128, B_pad * 384], bf16, tag="syl")
                y13 = (y_t[:, B_pad:4 * B_pad]
                       .rearrange("p (f b) -> p f b", f=3)
                       .transpose([0, 2, 1])
                       .unsqueeze(3))
                nc.gpsimd.tensor_tensor(
                    out=sy_t[:, :].rearrange("p (b f n) -> p b f n", f=3, n=128),
                    in0=sp_t[:, :].rearrange("p (b n) -> p b n", n=128)
                        .unsqueeze(2).to_broadcast([128, B_pad, 3, 128]),
                    in1=y13.to_broadcast([128, B_pad, 3, 128]),
                    op=AL.mult)

                agg = psAgg.tile([128, 1024], f32, tag="agg")
                nc.tensor.matmul(out=agg[:, 0:512], lhsT=zr_s[:, :],
                                 rhs=w3_s[:, :], start=True, stop=False,
                                 skip_group_check=True)
                nc.tensor.matmul(out=agg[:, 512:1024], lhsT=zr_s[:, :],
                                 rhs=w3_s[:, :], start=True, stop=False,
                                 skip_group_check=True)

                nb_groups = (B_pad + BB - 1) // BB
                for g in range(nb_groups):
                    b0 = g * BB
                    gsz = min(BB, B_pad - b0)
                    p1 = psB.tile([128, 128 * BB], f32, tag="p1")
                    for bi in range(gsz):
                        b = b0 + bi
                        o = 128 * bi
                        nc.tensor.matmul(out=p1[:, o:o + 128],
                                         lhsT=tl_t[:, 128 * b:128 * (b + 1)],
                                         rhs=w1c_s[:, :], start=True, stop=False)
                        nc.tensor.matmul(out=p1[:, o:o + 128], lhsT=idb_s[:, :],
                                         rhs=gs_t[:, 640 * b + 512:640 * b + 640],
                                         start=False, stop=False)
                        nc.tensor.matmul(out=p1[:, o:o + 128],
                                         lhsT=st_t[:, 128 * b:128 * (b + 1)],
                                         rhs=prt[:, :],
                                         start=False, stop=True)
                    h1 = wp.tile([128, 128 * BB], f32, tag="h1")
                    _dbg_h1 = (DEBUG_TAPS and t == 0 and g == 0)
                    if USE_SILU:
                        nc.scalar.activation(out=h1[:, :128 * gsz],
                                             in_=p1[:, :128 * gsz], func=AF.Silu)
                    else:
                        sg1 = wp.tile([128, 128 * BB], f32, tag="sg1")
                        nc.scalar.activation(out=sg1[:, :128 * gsz],
                                             in_=p1[:, :128 * gsz], func=AF.Sigmoid)
                        nc.vector.tensor_tensor(out=h1[:, :128 * gsz],
                                                in0=p1[:, :128 * gsz],
                                                in1=sg1[:, :128 * gsz], op=AL.mult)
                    if _dbg_h1:
                        nc.sync.dma_start(out=dbg["h1"][:, :],
                                          in_=h1[:, :512])
                    pt1 = psB.tile([128, 128 * BB], f32, tag="pt1")
                    for bi in range(gsz):
                        o = 128 * bi
                        nc.tensor.transpose(out=pt1[:, o:o + 128],
                                            in_=h1[:, o:o + 128], identity=idf_s[:, :])
                    h1t = wp.tile([128, 128 * BB], bf16, tag="h1t")
                    nc.scalar.activation(out=h1t[:, :128 * gsz],
                                         in_=pt1[:, :128 * gsz], func=AF.Copy)

                    p2 = psB.tile([128, 128 * BB], f32, tag="p2")
                    for bi in range(gsz):
                        o = 128 * bi
                        nc.tensor.matmul(out=p2[:, o:o + 128], lhsT=h1t[:, o:o + 128],
                                         rhs=w2_s[:, :], start=True, stop=False)
                        nc.tensor.matmul(out=p2[:, o:o + 128], lhsT=ones_s[:, :],
                                         rhs=b2_s[:, :], start=False, stop=True)
                    h2 = wp.tile([128, 128 * BB], f32, tag="h2")
                    if USE_SILU:
                        nc.scalar.activation(out=h2[:, :128 * gsz],
                                             in_=p2[:, :128 * gsz], func=AF.Silu)
                    else:
                        sg2 = wp.tile([128, 128 * BB], f32, tag="sg2")
                        nc.scalar.activation(out=sg2[:, :128 * gsz],
                                             in_=p2[:, :128 * gsz], func=AF.Sigmoid)
                        nc.vector.tensor_tensor(out=h2[:, :128 * gsz],
                                                in0=p2[:, :128 * gsz],
                                                in1=sg2[:, :128 * gsz], op=AL.mult)
                    pt2 = psB.tile([128, 128 * BB], f32, tag="pt2")
                    for bi in range(gsz):
                        o = 128 * bi
                        nc.tensor.transpose(out=pt2[:, o:o + 128],
                                            in_=h2[:, o:o + 128], identity=idf_s[:, :])
                    h2t = wp.tile([128, 128 * BB], bf16, tag="h2t")
                    nc.scalar.activation(out=h2t[:, :128 * gsz],
                                         in_=pt2[:, :128 * gsz], func=AF.Copy)

                    for bi in range(gsz):
                        b = b0 + bi
                        o = 128 * bi
                        ptw = psC.tile([128, 512], f32, tag="ptw")
                        nc.tensor.matmul(out=ptw[:, :], lhsT=h2t[:, o:o + 128],
                                         rhs=w3_s[:, :], start=True, stop=True)
                        tpw = wp.tile([128, 512], bf16, tag="tpw")
                        nc.scalar.activation(out=tpw[:, :], in_=ptw[:, :],
                                             func=AF.Copy)

                        if DEBUG_TAPS and t == 0 and b == 0:
                            nc.gpsimd.dma_start(out=dbg["tpw"][:, :],
                                                in_=tpw[:, :])
                        xs0 = gs_t[:, 640 * b:640 * b + 128]
                        xs1 = gs_t[:, 640 * b + 128:640 * b + 512]
                        y0 = y_t[:, b:b + 1]
                        pa = wp.tile([128, 128], bf16, tag="pa")
                        pd = wp.tile([128, 384], bf16, tag="pd")
                        pb = wp.tile([128, 128], bf16, tag="pb")
                        pc = wp.tile([128, 384], bf16, tag="pc")
                        # A = xs0*wA*y0
                        nc.vector.tensor_tensor(out=pa[:, :], in0=xs0,
                                                in1=tpw[:, 0:128], op=AL.mult)
                        nc.scalar.activation(out=pa[:, :], in_=pa[:, :],
                                             func=AF.Copy, scale=y0)
                        # D_i = xs1_i*wD*y1_i
                        wd3 = tpw[:, 128:256].unsqueeze(1).to_broadcast(
                            [128, 3, 128])
                        y13b = (y_t[:, B_pad + b:4 * B_pad:B_pad]
                                .unsqueeze(2)
                                .to_broadcast([128, 3, 128]))
                        nc.vector.tensor_tensor(
                            out=pd[:, :].rearrange("p (f n) -> p f n", f=3),
                            in0=xs1.rearrange("p (f n) -> p f n", f=3),
                            in1=wd3, op=AL.mult)
                        nc.vector.tensor_tensor(
                            out=pd[:, :].rearrange("p (f n) -> p f n", f=3),
                            in0=pd[:, :].rearrange("p (f n) -> p f n", f=3),
                            in1=y13b, op=AL.mult)
                        # B = xs0*wB (y1 folded into S)
                        nc.vector.tensor_tensor(out=pb[:, :], in0=xs0,
                                                in1=tpw[:, 256:384], op=AL.mult)
                        # C_i = xs1_i*wC*y0  (on gpsimd to offload DVE)
                        wc3 = tpw[:, 384:512].unsqueeze(1).to_broadcast(
                            [128, 3, 128])
                        nc.vector.tensor_tensor(
                            out=pc[:, :].rearrange("p (f n) -> p f n", f=3),
                            in0=xs1.rearrange("p (f n) -> p f n", f=3),
                            in1=wc3, op=AL.mult)
                        nc.scalar.activation(out=pc[:, :], in_=pc[:, :],
                                             func=AF.Copy, scale=y0)

                        if DEBUG_TAPS and t == 0 and b == 0:
                            nc.gpsimd.dma_start(out=dbg["pa"][:, 0:128], in_=pa[:, :])
                            nc.gpsimd.dma_start(out=dbg["pa"][:, 128:512], in_=pd[:, :])
                            nc.gpsimd.dma_start(out=dbg["pa"][:, 512:640], in_=pb[:, :])
                            nc.gpsimd.dma_start(out=dbg["pa"][:, 640:1024], in_=pc[:, :])
                        lastb = (b == B_pad - 1)
                        sp_b = sp_t[:, 128 * b:128 * (b + 1)]
                        # bank0: A [0:128], B [128:512] (single N=384 mm)
                        nc.tensor.matmul(out=agg[:, 0:128], lhsT=pa[:, :], rhs=sp_b,
                                         start=False, stop=False,
                                         skip_group_check=True)
                        nc.tensor.matmul(out=agg[:, 128:512], lhsT=pb[:, :],
                                         rhs=sy_t[:, 384 * b:384 * (b + 1)],
                                         start=False, stop=lastb,
                                         skip_group_check=True)
                        # bank1: D [512:640], C [640:1024]
                        for i in range(3):
                            nc.tensor.matmul(out=agg[:, 512:640],
                                             lhsT=pd[:, 128 * i:128 * (i + 1)],
                                             rhs=sp_b, start=False, stop=False,
                                             skip_group_check=True)
                        for i in range(3):
                            last = lastb and (i == 2)
                            nc.tensor.matmul(out=agg[:, 640 + 128 * i:768 + 128 * i],
                                             lhsT=pc[:, 128 * i:128 * (i + 1)],
                                             rhs=sp_b, start=False, stop=last,
                                             skip_group_check=True)

                # ---- final linear for this node tile ----
                aggs = wp.tile([128, 1024], bf16, tag="aggs")
                nc.scalar.activation(out=aggs[:, :], in_=agg[:, :], func=AF.Copy)
                if DEBUG_TAPS and t == 0:
                    nc.gpsimd.dma_start(out=dbg["agg"][:, :], in_=aggs[:, :])
                pf = psC.tile([128, 512], f32, tag="ptw")
                nc.tensor.matmul(out=pf[:, 0:512], lhsT=zr_s[:, :],
                                 rhs=w3_s[:, :], start=True, stop=False,
                                 skip_group_check=True)
                nc.tensor.matmul(out=pf[:, 0:128], lhsT=aggs[:, 0:128],
                                 rhs=wo0t_s[:, :], start=False, stop=False,
                                 skip_group_check=True)
                nc.tensor.matmul(out=pf[:, 0:128], lhsT=aggs[:, 512:640],
                                 rhs=wo0b_s[:, :], start=False, stop=False,
                                 skip_group_check=True)
                for i in range(3):
                    o = 128 * (i + 1)
                    nc.tensor.matmul(out=pf[:, o:o + 128],
                                     lhsT=aggs[:, 128 + 128 * i:256 + 128 * i],
                                     rhs=wo1t_s[:, :], start=False, stop=False,
                                     skip_group_check=True)
                    nc.tensor.matmul(out=pf[:, o:o + 128],
                                     lhsT=aggs[:, 640 + 128 * i:768 + 128 * i],
                                     rhs=wo1b_s[:, :], start=False,
                                     stop=(i == 2), skip_group_check=True)
                outs = wp.tile([128, 512], f32, tag="outs")
                ov = outs[:, :].rearrange("p (m c) -> p m c", c=4)
                for c4 in range(4):
                    nc.vector.tensor_copy(out=ov[:, :, c4],
                                          in_=pf[:, 128 * c4:128 * (c4 + 1)])
                nc.sync.dma_start(out=out_d[128 * t:128 * (t + 1), :],
                                  in_=outs[:, :])

    nc.compile()
    return nc


# --------------------------------------------------------------------------
# Entry point
# --------------------------------------------------------------------------

def kernel(**inputs):
    B_pad, in_maps = _host_prep(inputs)
    if B_pad not in _compiled:
        _compiled[B_pad] = _build(B_pad)
    nc = _compiled[B_pad]

    if os.environ.get("BASS_KERNEL_SIM"):
        from concourse.bass_interp import CoreSim
        outs = []
        ncores = int(os.environ.get("BASS_KERNEL_SIM_CORES", "1"))
        for c in range(ncores):
            sim = CoreSim(nc, trace=False)
            for k, v in in_maps[c].items():
                sim.tensor(k)[:] = v
            sim.simulate(check_with_hw=False)
            outs.append(np.array(sim.tensor("out_dram")))
        # fill remaining cores with zeros (partial sim for quick checks)
        while len(outs) < NCORES:
            outs.append(np.zeros((NT * 128, 512), np.float32))
    else:
        res = run_bass_kernel_spmd(nc, in_maps, list(range(NCORES)))
        outs = [res.results[c]["out_dram"] for c in range(NCORES)]

    full = np.zeros((N, MUL, 4), np.float32)
    for c in range(NCORES):
        full[c * NPC:(c + 1) * NPC] = outs[c][:NPC].reshape(NPC, MUL, 4)
    return full



# revision 4
# speedup vs baseline: 2.8295x; 2.8295x over previous
"""Trainium2 Bass kernel for nn_DiffusionInteractionBlock (GNN message passing).

Strategy: shard EDGES by receiver node range across 8 cores (receiver-sharded
edge parallelism).  Each core owns nodes [c*1250, (c+1)*1250) and processes
exactly the edges whose receiver lands in its range, so the segment-sum is
fully local and no cross-core reduction is needed.  Node-level linear tables
are computed SHARDED (1/8 of nodes per core) and exchanged with an on-device
AllGather, so each core ships only its 1/8 slice of node features over the
host link.  Per-edge sender/receiver rows are fetched with indirect DMA, the
edge MLP + tensor product run on-chip, and messages scatter-sum into PSUM via
one-hot matmuls whose one-hot matrices are built on-device from the receiver
slot ids (nothing quadratic crosses the host link).

Host-side prep (inside kernel()): sorting edges by (core, node-tile),
padding each (core, tile) edge list to a uniform block count so all 8 cores
run the same program (SPMD), packing per-edge side arrays in bf16, and
folding / pre-scaling weight matrices into two packed tensors.
"""

import os
import sys

import numpy as np

sys.path.insert(0, "/opt/trn_rl_repo")

import ml_dtypes

from concourse import bacc, bass, mybir, tile
from concourse.bass_utils import run_bass_kernel_spmd

BF16 = ml_dtypes.bfloat16

N = 10000
E = 160000
MUL = 128
NCORES = 8
NPC = N // NCORES  # 1250 nodes per core (edge/receiver sharding)
NT = 10            # node tiles of 128 per core (1280 >= 1250)
NPAD = 80 * 128    # padded node count for the tables (10240)
NSH = NPAD // NCORES  # 1280 nodes per core-shard (node-table sharding)
SQ3 = float(np.sqrt(3.0))
INV = 1.0 / np.sqrt(MUL)
OUT_SCALE = 1.0 / (np.sqrt(2 * MUL) * 16.0)

dt = mybir.dt

_compiled = {}


# --------------------------------------------------------------------------
# Host-side preprocessing
# --------------------------------------------------------------------------

def _host_prep(inputs):
    node_feats = np.asarray(inputs["node_feats"], np.float32)
    edge_attrs = np.asarray(inputs["edge_attrs"], np.float32)
    edge_feats = np.asarray(inputs["edge_feats"], np.float32)
    lengths = np.asarray(inputs["lengths"], np.float32)
    edge_index = np.asarray(inputs["edge_index"], np.int64)
    W_scalar = np.asarray(inputs["W_scalar"], np.float32)
    W_up0 = np.asarray(inputs["W_up0"], np.float32)
    W_up1 = np.asarray(inputs["W_up1"], np.float32)
    W1 = np.asarray(inputs["W1"], np.float32)
    b1 = np.asarray(inputs["b1"], np.float32)
    W2 = np.asarray(inputs["W2"], np.float32)
    b2 = np.asarray(inputs["b2"], np.float32)
    W3 = np.asarray(inputs["W3"], np.float32)
    Wout0 = np.asarray(inputs["Wout0"], np.float32)
    Wout1 = np.asarray(inputs["Wout1"], np.float32)

    sender, receiver = edge_index[0], edge_index[1]

    # global tile id: core * NT + local tile
    loc = receiver % NPC
    gtile = (receiver // NPC) * NT + loc // 128
    counts = np.bincount(gtile, minlength=NCORES * NT)
    B_pad = int(np.ceil(counts.max() / 128))
    EPT = 128 * B_pad                 # edges per node-tile (padded)

    # edge ids grouped by gtile; -1 marks padding
    order = np.argsort(gtile, kind="stable")
    epos = np.full((NCORES * NT, EPT), -1, np.int64)
    off = 0
    for g in range(NCORES * NT):
        c = counts[g]
        epos[g, :c] = order[off:off + c]
        off += c

    # per-edge padded values (pad: sender -> node 0, y/tail -> 0)
    valid = epos >= 0
    ep = np.where(valid, epos, 0)
    es = np.where(valid, sender[ep], 0).astype(np.int32)        # [G, EPT]
    rl = np.where(valid, loc[ep] % 128, 0).astype(np.float32)   # local id in tile
    y = np.where(valid[..., None], edge_attrs[ep], 0.0)         # [G, EPT, 4]
    ef = np.where(valid[..., None], edge_feats[ep], 0.0)        # [G, EPT, 8]
    ln = np.where(valid[..., None], lengths[ep], 0.0)           # [G, EPT, 1]
    one = valid.astype(np.float32)[..., None]                   # bias row (0 on pads)

    def wrap_idx(a):  # [G, EPT] -> [NCORES, 128, NT*B_pad] int32 (col = t*B_pad+b)
        a = a.astype(np.int32).reshape(NCORES, NT, B_pad, 128)
        a = np.transpose(a, (0, 3, 1, 2)).reshape(NCORES, 128, NT * B_pad)
        return np.ascontiguousarray(a)

    idx_s = wrap_idx(es)

    # tail10 [NCORES, NT, 10, EPT]: rows = [ef(8), len, indicator] per edge
    tail = np.concatenate([ef, ln, one], axis=-1)               # [G, EPT, 10]
    tail10 = np.transpose(tail.reshape(NCORES, NT, EPT, 10), (0, 1, 3, 2))
    tail10 = np.ascontiguousarray(tail10.astype(BF16))

    # yrl [NCORES, NT, 128, 5, B_pad]: f=0:y0, f=1..3:y1_i, f=4:recvloc
    yv = np.concatenate([y, rl[..., None]], axis=-1)            # [G, EPT, 5]
    yv = yv.reshape(NCORES, NT, B_pad, 128, 5)                  # j = b*128 + e
    yrl = np.transpose(yv, (0, 1, 3, 4, 2))                     # [NC, NT, 128, 5, B]
    yrl = np.ascontiguousarray(yrl.astype(BF16))

    # --- node feature shards (transposed for matmul lhsT) ---
    x0 = node_feats[:, :MUL]                                    # [N,128]
    x1 = node_feats[:, MUL:].reshape(N, MUL, 3)
    nfT = np.zeros((512, NPAD), np.float32)
    nfT[0:128, :N] = x0.T
    for i in range(3):
        nfT[128 * (i + 1):128 * (i + 2), :N] = x1[:, :, i].T
    nfT = nfT.astype(BF16)

    Ws_inv = W_scalar * INV
    # packed 128-row weights [128, 1664]
    wpack = np.concatenate([
        W_up0 * INV,                       # 0:128    h0
        W_up1 * INV,                       # 128:256  h1
        Ws_inv @ W1[:MUL],                 # 256:384  P_s
        Ws_inv @ W1[MUL:2 * MUL],          # 384:512  P_r
        W2,                                # 512:640
        np.concatenate(                    # 640:1152 W3 (with /sqrt3 fold)
            [W3[:, :MUL], W3[:, MUL:2 * MUL] / SQ3,
             W3[:, 2 * MUL:3 * MUL], W3[:, 3 * MUL:]], axis=1),
        Wout0[:MUL] * OUT_SCALE,           # 1152:1280
        Wout0[MUL:] * OUT_SCALE,           # 1280:1408
        Wout1[:MUL] * OUT_SCALE,           # 1408:1536
        Wout1[MUL:] * OUT_SCALE,           # 1536:1664
    ], axis=1).astype(BF16)
    # small-row pack [11, 128]: rows 0-9 = [W1 tail rows; b1], row 10 = b2
    w1c11 = np.concatenate(
        [W1[2 * MUL:], b1[None, :], b2[None, :]], axis=0).astype(BF16)

    tid = np.zeros((NCORES, 128, NT), np.int32)
    for c in range(NCORES):
        for t in range(NT):
            tid[c, :, t] = c * NPC + t * 128 + np.arange(128)
    tid = np.minimum(tid, NPAD - 1)

    in_maps = []
    for c in range(NCORES):
        m = {
            "wpack": wpack,
            "w1c": w1c11,
            "nfs": np.ascontiguousarray(nfT[:, c * NSH:(c + 1) * NSH]),
            "idx_s": idx_s[c],
            "idx_t": tid[c],
            "tail10": tail10[c].reshape(NT * 10, EPT),
            "yrl": yrl[c].reshape(NT * 128, 5 * B_pad),
        }
        in_maps.append(m)
    return B_pad, in_maps


# --------------------------------------------------------------------------
# Device program
# --------------------------------------------------------------------------

def _build(B_pad):
    EPT = 128 * B_pad
    nc = bacc.Bacc("TRN2", target_bir_lowering=False, debug=False,
                   num_devices=NCORES)

    f32, bf16, i32 = dt.float32, dt.bfloat16, dt.int32

    # inputs
    def din(name, shape, dtype):
        return nc.dram_tensor(name, list(shape), dtype, kind="ExternalInput")

    nfs = din("nfs", [512, NSH], bf16)
    wpack = din("wpack", [128, 1664], bf16)
    w1c = din("w1c", [11, 128], bf16)
    idx_s = din("idx_s", [128, NT * B_pad], i32)
    idx_t = din("idx_t", [128, NT], i32)
    tail10 = din("tail10", [NT * 10, EPT], bf16)
    yrl = din("yrl", [NT * 128, 5 * B_pad], bf16)

    out_d = nc.dram_tensor("out_dram", [NT * 128, 512], bf16,
                           kind="ExternalOutput")

    # NEFF-embedded constants (no per-call transfer)
    identf = nc.inline_tensor(np.eye(128, dtype=np.float32), name="identf")
    iota = nc.inline_tensor(
        np.tile(np.arange(128, dtype=np.float32), (128, 1)), name="iota")

    # internal DRAM node tables (part = this core's shard, full = gathered)
    T_s_part = nc.dram_tensor("T_s_part", [NSH, 640], bf16)
    T_r_part = nc.dram_tensor("T_r_part", [NSH, 128], bf16)
    T_s = nc.dram_tensor("T_s", [NPAD, 640], bf16)
    T_r = nc.dram_tensor("T_r", [NPAD, 128], bf16)

    AL = mybir.AluOpType
    AF = mybir.ActivationFunctionType

    with tile.TileContext(nc) as tc:
        with (
            tc.tile_pool(name="const", bufs=1) as cp,
            tc.tile_pool(name="work", bufs=2) as wp,
            tc.tile_pool(name="gath", bufs=2) as gp,
            tc.tile_pool(name="psB", bufs=1, space="PSUM") as psB,
            tc.tile_pool(name="psC", bufs=2, space="PSUM") as psC,
            tc.tile_pool(name="psAgg", bufs=1, space="PSUM") as psAgg,
        ):
            # ---- load constants to SBUF ----
            def ld(src, p, fdim, dtype, pool=cp):
                t = pool.tile([p, fdim], dtype, tag=f"c_{src.name}")
                nc.sync.dma_start(out=t[:, :], in_=src[:, :])
                return t

            wpk_s = ld(wpack, 128, 1664, bf16)
            wup0_s = wpk_s[:, 0:128]
            wup1_s = wpk_s[:, 128:256]
            wps_s = wpk_s[:, 256:384]
            wpr_s = wpk_s[:, 384:512]
            w2_s = wpk_s[:, 512:640]
            w3_s = wpk_s[:, 640:1152]
            wo0t_s = wpk_s[:, 1152:1280]
            wo0b_s = wpk_s[:, 1280:1408]
            wo1t_s = wpk_s[:, 1408:1536]
            wo1b_s = wpk_s[:, 1536:1664]
            w1c_s = ld(w1c, 11, 128, bf16)
            w1c10 = w1c_s[0:10, :]
            b2_s = cp.tile([1, 128], bf16, tag="c_b2")
            nc.sync.dma_start(out=b2_s[:, :], in_=w1c[10:11, :])
            idf_s = ld(identf, 128, 128, f32)
            iota_s = ld(iota, 128, 128, f32)
            idb_s = cp.tile([128, 128], bf16, tag="c_idb")
            nc.vector.tensor_copy(out=idb_s[:, :], in_=idf_s[:, :])
            zr_s = cp.tile([128, 128], bf16, tag="c_zr")
            nc.vector.memset(zr_s[:, :], 0.0)
            ones_s = cp.tile([1, 128], bf16, tag="c_ones")
            nc.vector.memset(ones_s[:, :], 1.0)
            ixs_s = ld(idx_s, 128, NT * B_pad, i32)
            ixt_s = ld(idx_t, 128, NT, i32)

            # ---- node-table phase (this core's 1/8 shard) ----
            with tc.tile_pool(name="nodes", bufs=1) as npool:
                x0t = npool.tile([128, NSH], bf16, tag="nf0")
                x1t0 = npool.tile([128, NSH], bf16, tag="nf1")
                x1t1 = npool.tile([128, NSH], bf16, tag="nf2")
                x1t2 = npool.tile([128, NSH], bf16, tag="nf3")
                for k, t in enumerate([x0t, x1t0, x1t1, x1t2]):
                    nc.sync.dma_start(
                        out=t[:, :], in_=nfs[128 * k:128 * (k + 1), :])
                for s in range(NSH // 128):
                    sl = slice(128 * s, 128 * (s + 1))
                    pn = psAgg.tile([128, 1024], f32, tag="agg")
                    mms = [
                        (x0t, wup0_s, 0),      # h0
                        (x1t0, wup1_s, 128),   # h1_0
                        (x1t1, wup1_s, 256),
                        (x1t2, wup1_s, 384),
                        (x0t, wps_s, 512),     # P_s
                        (x0t, wpr_s, 640),     # P_r
                    ]
                    for lhs, rhs, o in mms:
                        nc.tensor.matmul(out=pn[:, o:o + 128], lhsT=lhs[:, sl],
                                         rhs=rhs, start=True, stop=True)
                    tsb = wp.tile([128, 640], bf16, tag="tsb")
                    trb = wp.tile([128, 128], bf16, tag="trb")
                    nc.vector.tensor_copy(out=tsb[:, :], in_=pn[:, 0:640])
                    nc.vector.tensor_copy(out=trb[:, :], in_=pn[:, 640:768])
                    nc.sync.dma_start(out=T_s_part[sl, :], in_=tsb[:, :])
                    nc.sync.dma_start(out=T_r_part[sl, :], in_=trb[:, :])

            # ---- all-gather the node tables across the 8 cores ----
            nc.gpsimd.collective_compute(
                "AllGather", AL.bypass,
                replica_groups=[list(range(NCORES))],
                ins=[T_s_part[:, :].opt()], outs=[T_s[:, :].opt()])
            nc.gpsimd.collective_compute(
                "AllGather", AL.bypass,
                replica_groups=[list(range(NCORES))],
                ins=[T_r_part[:, :].opt()], outs=[T_r[:, :].opt()])

            # ---- edge phase ----
            BB = 4  # blocks per batch-group
            for t in range(NT):
                gs_t = gp.tile([128, B_pad * 640], bf16, tag="gs")
                for b in range(B_pad):
                    col = t * B_pad + b
                    nc.gpsimd.indirect_dma_start(
                        out=gs_t[:, 640 * b:640 * (b + 1)], out_offset=None,
                        in_=T_s[:, :],
                        in_offset=bass.IndirectOffsetOnAxis(
                            ap=ixs_s[:, col:col + 1], axis=0))
                prt = gp.tile([128, 128], bf16, tag="prt")
                nc.gpsimd.indirect_dma_start(
                    out=prt[:, :], out_offset=None, in_=T_r[:, :],
                    in_offset=bass.IndirectOffsetOnAxis(
                        ap=ixt_s[:, t:t + 1], axis=0))
                tl_t = wp.tile([10, EPT], bf16, tag="tail")
                nc.sync.dma_start(out=tl_t[:, :], in_=tail10[t * 10:(t + 1) * 10, :])
                yb_t = wp.tile([128, 5 * B_pad], bf16, tag="yrlb")
                nc.sync.dma_start(out=yb_t[:, :], in_=yrl[t * 128:(t + 1) * 128, :])
                y_t = wp.tile([128, 5 * B_pad], f32, tag="yrl")
                nc.vector.tensor_copy(out=y_t[:, :], in_=yb_t[:, :])

                # selection matrices (f32 master for the PE transpose)
                sp_f = wp.tile([128, B_pad * 128], f32, tag="spf")
                rl3 = y_t[:, 4 * B_pad:5 * B_pad].unsqueeze(2)
                nc.vector.tensor_tensor(
                    out=sp_f[:, :].rearrange("p (b n) -> p b n", n=128),
                    in0=rl3.to_broadcast([128, B_pad, 128]),
                    in1=iota_s[:, :].unsqueeze(1).to_broadcast(
                        [128, B_pad, 128]),
                    op=AL.is_equal)
                sp_t = wp.tile([128, B_pad * 128], bf16, tag="spl")
                nc.vector.tensor_copy(out=sp_t[:, :], in_=sp_f[:, :])
                # transposed one-hot (node-partition) built on-device
                st_t = gp.tile([128, EPT], bf16, tag="stT")
                for q0 in range(0, B_pad, 4):
                    qn = min(4, B_pad - q0)
                    pq = psC.tile([128, 512], f32, tag="ptw")
                    for j in range(qn):
                        nc.tensor.transpose(
                            out=pq[:, 128 * j:128 * (j + 1)],
                            in_=sp_f[:, 128 * (q0 + j):128 * (q0 + j + 1)],
                            identity=idf_s[:, :])
                    nc.scalar.activation(
                        out=st_t[:, 128 * q0:128 * (q0 + qn)],
                        in_=pq[:, :128 * qn], func=AF.Copy)
                sy_t = wp.tile([128, B_pad * 384], bf16, tag="syl")
                y13 = (y_t[:, B_pad:4 * B_pad]
                       .rearrange("p (f b) -> p f b", f=3)
                       .transpose([0, 2, 1])
                       .unsqueeze(3))
                nc.gpsimd.tensor_tensor(
                    out=sy_t[:, :].rearrange("p (b f n) -> p b f n", f=3, n=128),
                    in0=sp_t[:, :].rearrange("p (b n) -> p b n", n=128)
                        .unsqueeze(2).to_broadcast([128, B_pad, 3, 128]),
                    in1=y13.to_broadcast([128, B_pad, 3, 128]),
                    op=AL.mult)

                agg = psAgg.tile([128, 1024], f32, tag="agg")
                nc.tensor.matmul(out=agg[:, 0:512], lhsT=zr_s[:, :],
                                 rhs=w3_s, start=True, stop=False,
                                 skip_group_check=True)
                nc.tensor.matmul(out=agg[:, 512:1024], lhsT=zr_s[:, :],
                                 rhs=w3_s, start=True, stop=False,
                                 skip_group_check=True)

                nb_groups = (B_pad + BB - 1) // BB
                for g in range(nb_groups):
                    b0 = g * BB
                    gsz = min(BB, B_pad - b0)
                    p1 = psB.tile([128, 128 * BB], f32, tag="p1")
                    for bi in range(gsz):
                        b = b0 + bi
                        o = 128 * bi
                        nc.tensor.matmul(out=p1[:, o:o + 128],
                                         lhsT=tl_t[:, 128 * b:128 * (b + 1)],
                                         rhs=w1c10, start=True, stop=False)
                        nc.tensor.matmul(out=p1[:, o:o + 128], lhsT=idb_s[:, :],
                                         rhs=gs_t[:, 640 * b + 512:640 * b + 640],
                                         start=False, stop=False)
                        nc.tensor.matmul(out=p1[:, o:o + 128],
                                         lhsT=st_t[:, 128 * b:128 * (b + 1)],
                                         rhs=prt[:, :],
                                         start=False, stop=True)
                    h1 = wp.tile([128, 128 * BB], f32, tag="h1")
                    nc.scalar.activation(out=h1[:, :128 * gsz],
                                         in_=p1[:, :128 * gsz], func=AF.Silu)
                    pt1 = psB.tile([128, 128 * BB], f32, tag="pt1")
                    for bi in range(gsz):
                        o = 128 * bi
                        nc.tensor.transpose(out=pt1[:, o:o + 128],
                                            in_=h1[:, o:o + 128], identity=idf_s[:, :])
                    h1t = wp.tile([128, 128 * BB], bf16, tag="h1t")
                    nc.scalar.activation(out=h1t[:, :128 * gsz],
                                         in_=pt1[:, :128 * gsz], func=AF.Copy)

                    p2 = psB.tile([128, 128 * BB], f32, tag="p2")
                    for bi in range(gsz):
                        o = 128 * bi
                        nc.tensor.matmul(out=p2[:, o:o + 128], lhsT=h1t[:, o:o + 128],
                                         rhs=w2_s, start=True, stop=False)
                        nc.tensor.matmul(out=p2[:, o:o + 128], lhsT=ones_s[:, :],
                                         rhs=b2_s, start=False, stop=True)
                    h2 = wp.tile([128, 128 * BB], f32, tag="h2")
                    nc.scalar.activation(out=h2[:, :128 * gsz],
                                         in_=p2[:, :128 * gsz], func=AF.Silu)
                    pt2 = psB.tile([128, 128 * BB], f32, tag="pt2")
                    for bi in range(gsz):
                        o = 128 * bi
                        nc.tensor.transpose(out=pt2[:, o:o + 128],
                                            in_=h2[:, o:o + 128], identity=idf_s[:, :])
                    h2t = wp.tile([128, 128 * BB], bf16, tag="h2t")
                    nc.scalar.activation(out=h2t[:, :128 * gsz],
                                         in_=pt2[:, :128 * gsz], func=AF.Copy)

                    for bi in range(gsz):
                        b = b0 + bi
                        o = 128 * bi
                        ptw = psC.tile([128, 512], f32, tag="ptw")
                        nc.tensor.matmul(out=ptw[:, :], lhsT=h2t[:, o:o + 128],
                                         rhs=w3_s, start=True, stop=True)
                        tpw = wp.tile([128, 512], bf16, tag="tpw")
                        nc.scalar.activation(out=tpw[:, :], in_=ptw[:, :],
                                             func=AF.Copy)

                        xs0 = gs_t[:, 640 * b:640 * b + 128]
                        xs1 = gs_t[:, 640 * b + 128:640 * b + 512]
                        y0 = y_t[:, b:b + 1]
                        pa = wp.tile([128, 128], bf16, tag="pa")
                        pd = wp.tile([128, 384], bf16, tag="pd")
                        pb = wp.tile([128, 128], bf16, tag="pb")
                        pc = wp.tile([128, 384], bf16, tag="pc")
                        # A = xs0*wA*y0
                        nc.vector.tensor_tensor(out=pa[:, :], in0=xs0,
                                                in1=tpw[:, 0:128], op=AL.mult)
                        nc.scalar.activation(out=pa[:, :], in_=pa[:, :],
                                             func=AF.Copy, scale=y0)
                        # D_i = xs1_i*wD*y1_i
                        wd3 = tpw[:, 128:256].unsqueeze(1).to_broadcast(
                            [128, 3, 128])
                        y13b = (y_t[:, B_pad + b:4 * B_pad:B_pad]
                                .unsqueeze(2)
                                .to_broadcast([128, 3, 128]))
                        nc.vector.tensor_tensor(
                            out=pd[:, :].rearrange("p (f n) -> p f n", f=3),
                            in0=xs1.rearrange("p (f n) -> p f n", f=3),
                            in1=wd3, op=AL.mult)
                        nc.vector.tensor_tensor(
                            out=pd[:, :].rearrange("p (f n) -> p f n", f=3),
                            in0=pd[:, :].rearrange("p (f n) -> p f n", f=3),
                            in1=y13b, op=AL.mult)
                        # B = xs0*wB (y1 folded into S)
                        nc.vector.tensor_tensor(out=pb[:, :], in0=xs0,
                                                in1=tpw[:, 256:384], op=AL.mult)
                        # C_i = xs1_i*wC*y0
                        wc3 = tpw[:, 384:512].unsqueeze(1).to_broadcast(
                            [128, 3, 128])
                        nc.vector.tensor_tensor(
                            out=pc[:, :].rearrange("p (f n) -> p f n", f=3),
                            in0=xs1.rearrange("p (f n) -> p f n", f=3),
                            in1=wc3, op=AL.mult)
                        nc.scalar.activation(out=pc[:, :], in_=pc[:, :],
                                             func=AF.Copy, scale=y0)

                        lastb = (b == B_pad - 1)
                        sp_b = sp_t[:, 128 * b:128 * (b + 1)]
                        # bank0: A [0:128], B [128:512]
                        nc.tensor.matmul(out=agg[:, 0:128], lhsT=pa[:, :], rhs=sp_b,
                                         start=False, stop=False,
                                         skip_group_check=True)
                        nc.tensor.matmul(out=agg[:, 128:512], lhsT=pb[:, :],
                                         rhs=sy_t[:, 384 * b:384 * (b + 1)],
                                         start=False, stop=lastb,
                                         skip_group_check=True)
                        # bank1: D [512:640], C [640:1024]
                        for i in range(3):
                            nc.tensor.matmul(out=agg[:, 512:640],
                                             lhsT=pd[:, 128 * i:128 * (i + 1)],
                                             rhs=sp_b, start=False, stop=False,
                                             skip_group_check=True)
                        for i in range(3):
                            last = lastb and (i == 2)
                            nc.tensor.matmul(out=agg[:, 640 + 128 * i:768 + 128 * i],
                                             lhsT=pc[:, 128 * i:128 * (i + 1)],
                                             rhs=sp_b, start=False, stop=last,
                                             skip_group_check=True)

                # ---- final linear for this node tile ----
                aggs = wp.tile([128, 1024], bf16, tag="aggs")
                nc.scalar.activation(out=aggs[:, :], in_=agg[:, :], func=AF.Copy)
                pf = psC.tile([128, 512], f32, tag="ptw")
                nc.tensor.matmul(out=pf[:, 0:512], lhsT=zr_s[:, :],
                                 rhs=w3_s, start=True, stop=False,
                                 skip_group_check=True)
                nc.tensor.matmul(out=pf[:, 0:128], lhsT=aggs[:, 0:128],
                                 rhs=wo0t_s, start=False, stop=False,
                                 skip_group_check=True)
                nc.tensor.matmul(out=pf[:, 0:128], lhsT=aggs[:, 512:640],
                                 rhs=wo0b_s, start=False, stop=False,
                                 skip_group_check=True)
                for i in range(3):
                    o = 128 * (i + 1)
                    nc.tensor.matmul(out=pf[:, o:o + 128],
                                     lhsT=aggs[:, 128 + 128 * i:256 + 128 * i],
                                     rhs=wo1t_s, start=False, stop=False,
                                     skip_group_check=True)
                    nc.tensor.matmul(out=pf[:, o:o + 128],
                                     lhsT=aggs[:, 640 + 128 * i:768 + 128 * i],
                                     rhs=wo1b_s, start=False,
                                     stop=(i == 2), skip_group_check=True)
                outs = wp.tile([128, 512], bf16, tag="outs")
                ov = outs[:, :].rearrange("p (m c) -> p m c", c=4)
                for c4 in range(4):
                    nc.vector.tensor_copy(out=ov[:, :, c4],
                                          in_=pf[:, 128 * c4:128 * (c4 + 1)])
                nc.sync.dma_start(out=out_d[128 * t:128 * (t + 1), :],
                                  in_=outs[:, :])

    nc.compile()
    return nc


# --------------------------------------------------------------------------
# Entry point
# --------------------------------------------------------------------------

def kernel(**inputs):
    B_pad, in_maps = _host_prep(inputs)
    if B_pad not in _compiled:
        _compiled[B_pad] = _build(B_pad)
    nc = _compiled[B_pad]

    if os.environ.get("BASS_KERNEL_SIM"):
        from concourse.bass_interp import MultiCoreSim
        sim = MultiCoreSim(nc, NCORES)
        for c in range(NCORES):
            for k, v in in_maps[c].items():
                sim.cores[c].tensor(k)[:] = v
        sim.simulate(check_with_hw=False)
        outs = [np.array(sim.cores[c].tensor("out_dram")) for c in range(NCORES)]
    else:
        res = run_bass_kernel_spmd(nc, in_maps, list(range(NCORES)))
        outs = [res.results[c]["out_dram"] for c in range(NCORES)]

    full = np.zeros((N, MUL, 4), np.float32)
    for c in range(NCORES):
        full[c * NPC:(c + 1) * NPC] = (
            outs[c][:NPC].astype(np.float32).reshape(NPC, MUL, 4))
    return full


# revision 10
# speedup vs baseline: 4.1024x; 1.4499x over previous
"""Trainium2 Bass kernel for nn_DiffusionInteractionBlock (GNN message passing).

Strategy: shard EDGES by receiver node range across 8 cores (receiver-sharded
edge parallelism).  Each core owns nodes [c*1250, (c+1)*1250) and processes
exactly the edges whose receiver lands in its range, so the segment-sum is
fully local.  Node-level linear tables are computed SHARDED (1/8 of nodes per
core) and exchanged with an on-device AllGather, so each core ships only its
1/8 slice of node features over the host link (in fp8).  Per-edge
sender rows are fetched with indirect DMA, the edge MLP + tensor product run
on-chip, and messages scatter-sum into PSUM via a single one-hot matmul pair
per 128-edge block (node-partitioned aggregate).  The per-node-tile edge
pipeline runs under a hardware For_i loop so the program stays small — the
host-side per-call cost of shipping/loading the program scales with
instruction count.

Host-side prep (inside kernel()): sorting edges by (core, node-tile),
padding each (core, tile) edge list to a uniform block count so all 8 cores
run the same program (SPMD), packing per-edge side arrays in fp8/bf16/int16,
and folding / pre-scaling weight matrices into one packed tensor that is
itself sharded across cores and AllGathered on device.
"""

import os
import sys

import numpy as np

sys.path.insert(0, "/opt/trn_rl_repo")

import ml_dtypes

from concourse import bacc, bass, mybir, tile
from concourse.bass import ds
from concourse.bass_utils import run_bass_kernel_spmd

BF16 = ml_dtypes.bfloat16
FP8 = ml_dtypes.float8_e4m3fn

N = 10000
E = 160000
MUL = 128
NCORES = 8
NPC = N // NCORES  # 1250 nodes per core (edge/receiver sharding)
NT = 10            # node tiles of 128 per core (1280 >= 1250)
NPAD = 80 * 128    # padded node count for the tables (10240)
NSH = NPAD // NCORES  # 1280 nodes per core-shard (node-table sharding)
SQ3 = float(np.sqrt(3.0))
INV = 1.0 / np.sqrt(MUL)
OUT_SCALE = 1.0 / (np.sqrt(2 * MUL) * 16.0)

dt = mybir.dt

USE_SILU = os.environ.get("BASS_NO_SILU", "") != "1"

_compiled = {}


# --------------------------------------------------------------------------
# Host-side preprocessing
# --------------------------------------------------------------------------

def _host_prep(inputs):
    node_feats = np.asarray(inputs["node_feats"], np.float32)
    edge_attrs = np.asarray(inputs["edge_attrs"], np.float32)
    edge_feats = np.asarray(inputs["edge_feats"], np.float32)
    lengths = np.asarray(inputs["lengths"], np.float32)
    edge_index = np.asarray(inputs["edge_index"], np.int64)
    W_scalar = np.asarray(inputs["W_scalar"], np.float32)
    W_up0 = np.asarray(inputs["W_up0"], np.float32)
    W_up1 = np.asarray(inputs["W_up1"], np.float32)
    W1 = np.asarray(inputs["W1"], np.float32)
    b1 = np.asarray(inputs["b1"], np.float32)
    W2 = np.asarray(inputs["W2"], np.float32)
    b2 = np.asarray(inputs["b2"], np.float32)
    W3 = np.asarray(inputs["W3"], np.float32)
    Wout0 = np.asarray(inputs["Wout0"], np.float32)
    Wout1 = np.asarray(inputs["Wout1"], np.float32)

    sender, receiver = edge_index[0], edge_index[1]

    # global tile id: core * NT + local tile
    loc = receiver % NPC
    gtile = (receiver // NPC) * NT + loc // 128
    counts = np.bincount(gtile, minlength=NCORES * NT)
    B_pad = int(np.ceil(counts.max() / 128))
    EPT = 128 * B_pad                 # edges per node-tile (padded)

    # edge ids grouped by gtile; -1 marks padding
    order = np.argsort(gtile, kind="stable")
    epos = np.full((NCORES * NT, EPT), -1, np.int64)
    off = 0
    for g in range(NCORES * NT):
        c = counts[g]
        epos[g, :c] = order[off:off + c]
        off += c

    # per-edge padded values (pad: sender -> node 0, y/tail -> 0)
    valid = epos >= 0
    ep = np.where(valid, epos, 0)
    es = np.where(valid, sender[ep], 0).astype(np.int16)        # [G, EPT]
    rl = np.where(valid, loc[ep] % 128, 0).astype(np.float32)   # local id in tile
    y = np.where(valid[..., None], edge_attrs[ep], 0.0)         # [G, EPT, 4]
    ef = np.where(valid[..., None], edge_feats[ep], 0.0)        # [G, EPT, 8]
    ln = np.where(valid[..., None], lengths[ep], 0.0)           # [G, EPT, 1]
    one = valid.astype(np.float32)[..., None]                   # bias row (0 on pads)

    def wrap_idx(a):  # [G, EPT] -> [NCORES, 128, NT*B_pad] (col = t*B_pad+b)
        a = a.reshape(NCORES, NT, B_pad, 128)
        a = np.transpose(a, (0, 3, 1, 2)).reshape(NCORES, 128, NT * B_pad)
        return np.ascontiguousarray(a)

    idx_s = wrap_idx(es)

    # tail10 [NCORES, NT, 10, EPT]: rows = [ef(8), len, indicator] per edge
    tail = np.concatenate([ef, ln, one], axis=-1)               # [G, EPT, 10]
    tail10 = np.transpose(tail.reshape(NCORES, NT, EPT, 10), (0, 1, 3, 2))
    tail10 = np.ascontiguousarray(tail10.astype(FP8))

    # yrl [NCORES, NT, 128, 5, B_pad]: f=0:y0, f=1..3:y1_i, f=4:recvloc
    yv = np.concatenate([y, rl[..., None]], axis=-1)            # [G, EPT, 5]
    yv = yv.reshape(NCORES, NT, B_pad, 128, 5)                  # j = b*128 + e
    yrl = np.transpose(yv, (0, 1, 3, 4, 2))                     # [NC, NT, 128, 5, B]
    yrl = np.ascontiguousarray(yrl.astype(BF16))

    # --- node feature shards (transposed for matmul lhsT) ---
    x0 = node_feats[:, :MUL]                                    # [N,128]
    x1 = node_feats[:, MUL:].reshape(N, MUL, 3)
    nfT = np.zeros((512, NPAD), np.float32)
    nfT[0:128, :N] = x0.T
    for i in range(3):
        nfT[128 * (i + 1):128 * (i + 2), :N] = x1[:, :, i].T
    nfT = nfT.astype(BF16)

    Ws_inv = W_scalar * INV
    # packed 128-row weights [128, 1664]
    wpack = np.concatenate([
        W_up0 * INV,                       # 0:128    h0
        W_up1 * INV,                       # 128:256  h1
        Ws_inv @ W1[:MUL],                 # 256:384  P_s
        Ws_inv @ W1[MUL:2 * MUL],          # 384:512  P_r
        W2,                                # 512:640
        np.concatenate(                    # 640:1152 W3 (with /sqrt3 fold)
            [W3[:, :MUL], W3[:, MUL:2 * MUL] / SQ3,
             W3[:, 2 * MUL:3 * MUL], W3[:, 3 * MUL:]], axis=1),
        Wout0[:MUL] * OUT_SCALE,           # 1152:1280
        Wout0[MUL:] * OUT_SCALE,           # 1280:1408
        Wout1[:MUL] * OUT_SCALE,           # 1408:1536
        Wout1[MUL:] * OUT_SCALE,           # 1536:1664
    ], axis=1).astype(BF16)
    # small-row pack [11, 128]: rows 0-9 = [W1 tail rows; b1], row 10 = b2
    w1c11 = np.concatenate(
        [W1[2 * MUL:], b1[None, :], b2[None, :]], axis=0).astype(BF16)

    tid = np.zeros((NCORES, 128, NT), np.int32)
    for c in range(NCORES):
        for t in range(NT):
            tid[c, :, t] = c * NPC + t * 128 + np.arange(128)
    tid = np.minimum(tid, NPAD - 1)

    WSH = 128 // NCORES  # 16 wpack rows per core
    in_maps = []
    for c in range(NCORES):
        m = {
            "wps": np.ascontiguousarray(wpack[c * WSH:(c + 1) * WSH]),
            "w1c": w1c11,
            "nfs": np.ascontiguousarray(nfT[:, c * NSH:(c + 1) * NSH]),
            "idx_s": idx_s[c],
            "idx_t": tid[c],
            "tail10": tail10[c].reshape(NT * 10, EPT),
            "yrl": yrl[c].reshape(NT * 128, 5 * B_pad),
        }
        in_maps.append(m)
    return B_pad, in_maps


# --------------------------------------------------------------------------
# Device program
# --------------------------------------------------------------------------

def _build(B_pad):
    EPT = 128 * B_pad
    nc = bacc.Bacc("TRN2", target_bir_lowering=False, debug=False,
                   num_devices=NCORES)

    f32, bf16, fp8, i16, i32 = (dt.float32, dt.bfloat16, dt.float8e4,
                                dt.int16, dt.int32)

    # inputs
    def din(name, shape, dtype):
        return nc.dram_tensor(name, list(shape), dtype, kind="ExternalInput")

    WSH = 128 // NCORES
    nfs = din("nfs", [512, NSH], bf16)
    wps = din("wps", [WSH, 1664], bf16)
    w1c = din("w1c", [11, 128], bf16)
    idx_s = din("idx_s", [128, NT * B_pad], i16)
    idx_t = din("idx_t", [128, NT], i32)
    tail10 = din("tail10", [NT * 10, EPT], fp8)
    yrl = din("yrl", [NT * 128, 5 * B_pad], bf16)

    out_d = nc.dram_tensor("out_dram", [NT * 128, 512], bf16,
                           kind="ExternalOutput")

    # NEFF-embedded constants (no per-call transfer)
    identf = nc.inline_tensor(np.eye(128, dtype=np.float32), name="identf")
    iota = nc.inline_tensor(
        np.tile(np.arange(128, dtype=np.float32), (128, 1)), name="iota")

    # internal DRAM: weight-pack bounce + node tables (part/full)
    wps_bin = nc.dram_tensor("wps_bin", [WSH, 1664], bf16)
    wpk_full = nc.dram_tensor("wpk_full", [128, 1664], bf16)
    T_s_part = nc.dram_tensor("T_s_part", [NSH, 640], bf16)
    T_r_part = nc.dram_tensor("T_r_part", [NSH, 128], bf16)
    T_s = nc.dram_tensor("T_s", [NPAD, 640], bf16)
    T_r = nc.dram_tensor("T_r", [NPAD, 128], bf16)

    AL = mybir.AluOpType
    AF = mybir.ActivationFunctionType
    RG = [list(range(NCORES))]

    with tile.TileContext(nc) as tc:
        with (
            tc.tile_pool(name="const", bufs=1) as cp,
            tc.tile_pool(name="work", bufs=2) as wp,
            tc.tile_pool(name="gath", bufs=2) as gp,
            tc.tile_pool(name="psB", bufs=1, space="PSUM") as psB,
            tc.tile_pool(name="psC", bufs=2, space="PSUM") as psC,
            tc.tile_pool(name="psAgg", bufs=1, space="PSUM") as psAgg,
        ):
            # ---- gather the packed weights across cores ----
            nc.gpsimd.dma_start(out=wps_bin[:, :], in_=wps[:, :])
            nc.gpsimd.collective_compute(
                "AllGather", AL.bypass, replica_groups=RG,
                ins=[wps_bin[:, :].opt()], outs=[wpk_full[:, :].opt()])

            # ---- load constants to SBUF ----
            def ld(src, p, fdim, dtype, pool=cp):
                t = pool.tile([p, fdim], dtype, tag=f"c_{src.name}")
                nc.sync.dma_start(out=t[:, :], in_=src[:, :])
                return t

            wpk_s = ld(wpk_full, 128, 1664, bf16)
            wup0_s = wpk_s[:, 0:128]
            wup1_s = wpk_s[:, 128:256]
            wps_s = wpk_s[:, 256:384]
            wpr_s = wpk_s[:, 384:512]
            w2_s = wpk_s[:, 512:640]
            w3_s = wpk_s[:, 640:1152]
            wo0t_s = wpk_s[:, 1152:1280]
            wo0b_s = wpk_s[:, 1280:1408]
            wo1t_s = wpk_s[:, 1408:1536]
            wo1b_s = wpk_s[:, 1536:1664]
            w1c_s = ld(w1c, 11, 128, bf16)
            w1c10 = w1c_s[0:10, :]
            b2_s = cp.tile([1, 128], bf16, tag="c_b2")
            nc.sync.dma_start(out=b2_s[:, :], in_=w1c[10:11, :])
            idf_s = ld(identf, 128, 128, f32)
            iota_s = ld(iota, 128, 128, f32)
            idb_s = cp.tile([128, 128], bf16, tag="c_idb")
            nc.vector.tensor_copy(out=idb_s[:, :], in_=idf_s[:, :])
            ones_s = cp.tile([1, 128], bf16, tag="c_ones")
            nc.vector.memset(ones_s[:, :], 1.0)
            ix16_s = ld(idx_s, 128, NT * B_pad, i16)
            ixs_s = cp.tile([128, NT * B_pad], i32, tag="c_ixs")
            nc.vector.tensor_copy(out=ixs_s[:, :], in_=ix16_s[:, :])
            ixt_s = ld(idx_t, 128, NT, i32)

            # ---- node-table phase (this core's 1/8 shard) ----
            with tc.tile_pool(name="nodes", bufs=1) as npool:
                xts = []
                for k in range(4):
                    xb = npool.tile([128, NSH], bf16, tag=f"nb_{k}")
                    nc.sync.dma_start(
                        out=xb[:, :], in_=nfs[128 * k:128 * (k + 1), :])
                    xts.append(xb)
                x0t, x1t0, x1t1, x1t2 = xts
                for s in range(NSH // 128):
                    sl = slice(128 * s, 128 * (s + 1))
                    pn = psAgg.tile([128, 1024], f32, tag="agg")
                    mms = [
                        (x0t, wup0_s, 0),      # h0
                        (x1t0, wup1_s, 128),   # h1_0
                        (x1t1, wup1_s, 256),
                        (x1t2, wup1_s, 384),
                        (x0t, wps_s, 512),     # P_s
                        (x0t, wpr_s, 640),     # P_r
                    ]
                    for lhs, rhs, o in mms:
                        nc.tensor.matmul(out=pn[:, o:o + 128], lhsT=lhs[:, sl],
                                         rhs=rhs, start=True, stop=True)
                    tsb = wp.tile([128, 640], bf16, tag="tsb")
                    trb = wp.tile([128, 128], bf16, tag="trb")
                    nc.vector.tensor_copy(out=tsb[:, :], in_=pn[:, 0:640])
                    nc.vector.tensor_copy(out=trb[:, :], in_=pn[:, 640:768])
                    nc.sync.dma_start(out=T_s_part[sl, :], in_=tsb[:, :])
                    nc.sync.dma_start(out=T_r_part[sl, :], in_=trb[:, :])

            # ---- all-gather the node tables across the 8 cores ----
            nc.gpsimd.collective_compute(
                "AllGather", AL.bypass, replica_groups=RG,
                ins=[T_s_part[:, :].opt()], outs=[T_s[:, :].opt()])
            nc.gpsimd.collective_compute(
                "AllGather", AL.bypass, replica_groups=RG,
                ins=[T_r_part[:, :].opt()], outs=[T_r[:, :].opt()])

            # ---- edge phase (hardware loop over node tiles) ----
            BB = 4  # blocks per batch-group
            with tc.For_i(0, NT, 1) as t:
                # stage this tile's gather offsets into fixed tiles so the
                # indirect DMAs see static access patterns
                ixcur = wp.tile([128, B_pad + 1], i32, tag="ixcur")
                nc.vector.tensor_copy(out=ixcur[:, 0:B_pad],
                                      in_=ixs_s[:, ds(t * B_pad, B_pad)])
                nc.vector.tensor_copy(out=ixcur[:, B_pad:B_pad + 1],
                                      in_=ixt_s[:, ds(t, 1)])
                gs_t = gp.tile([128, B_pad * 640], bf16, tag="gs")
                for b in range(B_pad):
                    nc.gpsimd.indirect_dma_start(
                        out=gs_t[:, 640 * b:640 * (b + 1)], out_offset=None,
                        in_=T_s[:, :],
                        in_offset=bass.IndirectOffsetOnAxis(
                            ap=ixcur[:, b:b + 1], axis=0))
                prt = gp.tile([128, 128], bf16, tag="prt")
                nc.gpsimd.indirect_dma_start(
                    out=prt[:, :], out_offset=None, in_=T_r[:, :],
                    in_offset=bass.IndirectOffsetOnAxis(
                        ap=ixcur[:, B_pad:B_pad + 1], axis=0))
                tl8_t = wp.tile([10, EPT], fp8, tag="tail8")
                nc.sync.dma_start(out=tl8_t[:, :], in_=tail10[ds(t * 10, 10), :])
                tl_t = wp.tile([10, EPT], bf16, tag="tail")
                nc.vector.tensor_copy(out=tl_t[:, :], in_=tl8_t[:, :])
                yb_t = wp.tile([128, 5 * B_pad], bf16, tag="yrlb")
                nc.sync.dma_start(out=yb_t[:, :], in_=yrl[ds(t * 128, 128), :])
                y_t = wp.tile([128, 5 * B_pad], f32, tag="yrl")
                nc.vector.tensor_copy(out=y_t[:, :], in_=yb_t[:, :])

                # selection matrices (f32 master for the PE transpose)
                sp_f = wp.tile([128, B_pad * 128], f32, tag="spf")
                rl3 = y_t[:, 4 * B_pad:5 * B_pad].unsqueeze(2)
                nc.vector.tensor_tensor(
                    out=sp_f[:, :].rearrange("p (b n) -> p b n", n=128),
                    in0=rl3.to_broadcast([128, B_pad, 128]),
                    in1=iota_s[:, :].unsqueeze(1).to_broadcast(
                        [128, B_pad, 128]),
                    op=AL.is_equal)
                sp_t = wp.tile([128, B_pad * 128], bf16, tag="spl")
                nc.vector.tensor_copy(out=sp_t[:, :], in_=sp_f[:, :])
                # transposed one-hot (node-partition) built on-device
                st_t = gp.tile([128, EPT], bf16, tag="stT")
                for q0 in range(0, B_pad, 4):
                    qn = min(4, B_pad - q0)
                    pq = psC.tile([128, 512], f32, tag="ptw")
                    for j in range(qn):
                        nc.tensor.transpose(
                            out=pq[:, 128 * j:128 * (j + 1)],
                            in_=sp_f[:, 128 * (q0 + j):128 * (q0 + j + 1)],
                            identity=idf_s[:, :])
                    nc.scalar.activation(
                        out=st_t[:, 128 * q0:128 * (q0 + qn)],
                        in_=pq[:, :128 * qn], func=AF.Copy)

                agg = psAgg.tile([128, 1024], f32, tag="agg")

                nb_groups = (B_pad + BB - 1) // BB
                for g in range(nb_groups):
                    b0 = g * BB
                    gsz = min(BB, B_pad - b0)
                    p1 = psB.tile([128, 128 * BB], f32, tag="p1")
                    for bi in range(gsz):
                        b = b0 + bi
                        o = 128 * bi
                        nc.tensor.matmul(out=p1[:, o:o + 128],
                                         lhsT=tl_t[:, 128 * b:128 * (b + 1)],
                                         rhs=w1c10, start=True, stop=False)
                        nc.tensor.matmul(out=p1[:, o:o + 128], lhsT=idb_s[:, :],
                                         rhs=gs_t[:, 640 * b + 512:640 * b + 640],
                                         start=False, stop=False)
                        nc.tensor.matmul(out=p1[:, o:o + 128],
                                         lhsT=st_t[:, 128 * b:128 * (b + 1)],
                                         rhs=prt[:, :],
                                         start=False, stop=True)
                    h1 = wp.tile([128, 128 * BB], f32, tag="h1")
                    if USE_SILU:
                        nc.scalar.activation(out=h1[:, :128 * gsz],
                                             in_=p1[:, :128 * gsz], func=AF.Silu)
                    else:
                        sg1 = wp.tile([128, 128 * BB], f32, tag="sg1")
                        nc.scalar.activation(out=sg1[:, :128 * gsz],
                                             in_=p1[:, :128 * gsz], func=AF.Sigmoid)
                        nc.vector.tensor_tensor(out=h1[:, :128 * gsz],
                                                in0=p1[:, :128 * gsz],
                                                in1=sg1[:, :128 * gsz], op=AL.mult)
                    pt1 = psB.tile([128, 128 * BB], f32, tag="pt1")
                    for bi in range(gsz):
                        o = 128 * bi
                        nc.tensor.transpose(out=pt1[:, o:o + 128],
                                            in_=h1[:, o:o + 128], identity=idf_s[:, :])
                    h1t = wp.tile([128, 128 * BB], bf16, tag="h1t")
                    nc.scalar.activation(out=h1t[:, :128 * gsz],
                                         in_=pt1[:, :128 * gsz], func=AF.Copy)

                    p2 = psB.tile([128, 128 * BB], f32, tag="p2")
                    for bi in range(gsz):
                        o = 128 * bi
                        nc.tensor.matmul(out=p2[:, o:o + 128], lhsT=h1t[:, o:o + 128],
                                         rhs=w2_s, start=True, stop=False)
                        nc.tensor.matmul(out=p2[:, o:o + 128], lhsT=ones_s[:, :],
                                         rhs=b2_s[:, :], start=False, stop=True)
                    h2 = wp.tile([128, 128 * BB], f32, tag="h2")
                    if USE_SILU:
                        nc.scalar.activation(out=h2[:, :128 * gsz],
                                             in_=p2[:, :128 * gsz], func=AF.Silu)
                    else:
                        sg2 = wp.tile([128, 128 * BB], f32, tag="sg2")
                        nc.scalar.activation(out=sg2[:, :128 * gsz],
                                             in_=p2[:, :128 * gsz], func=AF.Sigmoid)
                        nc.vector.tensor_tensor(out=h2[:, :128 * gsz],
                                                in0=p2[:, :128 * gsz],
                                                in1=sg2[:, :128 * gsz], op=AL.mult)
                    pt2 = psB.tile([128, 128 * BB], f32, tag="pt2")
                    for bi in range(gsz):
                        o = 128 * bi
                        nc.tensor.transpose(out=pt2[:, o:o + 128],
                                            in_=h2[:, o:o + 128], identity=idf_s[:, :])
                    h2t = wp.tile([128, 128 * BB], bf16, tag="h2t")
                    nc.scalar.activation(out=h2t[:, :128 * gsz],
                                         in_=pt2[:, :128 * gsz], func=AF.Copy)

                    for bi in range(gsz):
                        b = b0 + bi
                        o = 128 * bi
                        ptw = psC.tile([128, 512], f32, tag="ptw")
                        nc.tensor.matmul(out=ptw[:, :], lhsT=h2t[:, o:o + 128],
                                         rhs=w3_s, start=True, stop=True)
                        tpw = wp.tile([128, 512], bf16, tag="tpw")
                        nc.scalar.activation(out=tpw[:, :], in_=ptw[:, :],
                                             func=AF.Copy)

                        xs0 = gs_t[:, 640 * b:640 * b + 128]
                        xs1 = gs_t[:, 640 * b + 128:640 * b + 512]
                        y0 = y_t[:, b:b + 1]
                        y13b = (y_t[:, B_pad + b:4 * B_pad:B_pad]
                                .unsqueeze(2)
                                .to_broadcast([128, 3, 128]))
                        # msgs layout: [A(128) | D(128) | B(384) | C(384)]
                        msgs = wp.tile([128, 1024], bf16, tag="msgs")
                        # A = xs0*wA*y0
                        nc.vector.tensor_tensor(out=msgs[:, 0:128], in0=xs0,
                                                in1=tpw[:, 0:128], op=AL.mult)
                        nc.scalar.activation(out=msgs[:, 0:128],
                                             in_=msgs[:, 0:128],
                                             func=AF.Copy, scale=y0)
                        # D = (sum_i xs1_i*y1_i) * wD
                        pd = wp.tile([128, 384], bf16, tag="pd")
                        nc.vector.tensor_tensor(
                            out=pd[:, :].rearrange("p (f n) -> p f n", f=3),
                            in0=xs1.rearrange("p (f n) -> p f n", f=3),
                            in1=y13b, op=AL.mult)
                        dd = wp.tile([128, 128], f32, tag="dd")
                        nc.vector.tensor_reduce(
                            out=dd[:, :],
                            in_=pd[:, :].rearrange("p (f n) -> p n f", f=3),
                            axis=mybir.AxisListType.X, op=AL.add)
                        nc.vector.tensor_tensor(out=msgs[:, 128:256],
                                                in0=dd[:, :],
                                                in1=tpw[:, 128:256], op=AL.mult)
                        # B_i = xs0*wB*y1_i
                        pb = wp.tile([128, 128], bf16, tag="pb")
                        nc.vector.tensor_tensor(out=pb[:, :], in0=xs0,
                                                in1=tpw[:, 256:384], op=AL.mult)
                        nc.vector.tensor_tensor(
                            out=msgs[:, 256:640].rearrange(
                                "p (f n) -> p f n", f=3),
                            in0=pb[:, :].unsqueeze(1).to_broadcast(
                                [128, 3, 128]),
                            in1=y13b, op=AL.mult)
                        # C_i = xs1_i*wC*y0
                        wc3 = tpw[:, 384:512].unsqueeze(1).to_broadcast(
                            [128, 3, 128])
                        nc.vector.tensor_tensor(
                            out=msgs[:, 640:1024].rearrange(
                                "p (f n) -> p f n", f=3),
                            in0=xs1.rearrange("p (f n) -> p f n", f=3),
                            in1=wc3, op=AL.mult)
                        nc.scalar.activation(out=msgs[:, 640:1024],
                                             in_=msgs[:, 640:1024],
                                             func=AF.Copy, scale=y0)

                        # node-partitioned scatter: agg[n, X] += sum_e sp[e,n]*msgs[e,X]
                        lastb = (b == B_pad - 1)
                        sp_b = sp_t[:, 128 * b:128 * (b + 1)]
                        nc.tensor.matmul(out=agg[:, 0:512], lhsT=sp_b,
                                         rhs=msgs[:, 0:512],
                                         start=(b == 0), stop=lastb,
                                         skip_group_check=True)
                        nc.tensor.matmul(out=agg[:, 512:1024], lhsT=sp_b,
                                         rhs=msgs[:, 512:1024],
                                         start=(b == 0), stop=lastb,
                                         skip_group_check=True)

                # ---- transpose aggregate back to channel-partition ----
                aggc = wp.tile([128, 1024], f32, tag="aggc")
                nc.scalar.activation(out=aggc[:, :], in_=agg[:, :], func=AF.Copy)
                aggT = wp.tile([128, 1024], bf16, tag="aggT")
                for h in range(2):
                    pq = psC.tile([128, 512], f32, tag="ptw")
                    for j in range(4):
                        nc.tensor.transpose(
                            out=pq[:, 128 * j:128 * (j + 1)],
                            in_=aggc[:, 512 * h + 128 * j:512 * h + 128 * (j + 1)],
                            identity=idf_s[:, :])
                    nc.scalar.activation(out=aggT[:, 512 * h:512 * (h + 1)],
                                         in_=pq[:, :], func=AF.Copy)

                # ---- final linear for this node tile ----
                # aggT chunks: [A | D | B0 B1 B2 | C0 C1 C2], each [m=128, n=128]
                pf = psC.tile([128, 512], f32, tag="ptw")
                nc.tensor.matmul(out=pf[:, 0:128], lhsT=aggT[:, 0:128],
                                 rhs=wo0t_s, start=True, stop=False,
                                 skip_group_check=True)
                nc.tensor.matmul(out=pf[:, 0:128], lhsT=aggT[:, 128:256],
                                 rhs=wo0b_s, start=False, stop=True,
                                 skip_group_check=True)
                for i in range(3):
                    o = 128 * (i + 1)
                    nc.tensor.matmul(out=pf[:, o:o + 128],
                                     lhsT=aggT[:, 256 + 128 * i:384 + 128 * i],
                                     rhs=wo1t_s, start=True, stop=False,
                                     skip_group_check=True)
                    nc.tensor.matmul(out=pf[:, o:o + 128],
                                     lhsT=aggT[:, 640 + 128 * i:768 + 128 * i],
                                     rhs=wo1b_s, start=False, stop=True,
                                     skip_group_check=True)
                outs = wp.tile([128, 512], bf16, tag="outs")
                ov = outs[:, :].rearrange("p (m c) -> p m c", c=4)
                for c4 in range(4):
                    nc.vector.tensor_copy(out=ov[:, :, c4],
                                          in_=pf[:, 128 * c4:128 * (c4 + 1)])
                nc.sync.dma_start(out=out_d[ds(t * 128, 128), :],
                                  in_=outs[:, :])

    nc.compile()
    return nc


# --------------------------------------------------------------------------
# Entry point
# --------------------------------------------------------------------------

def kernel(**inputs):
    B_pad, in_maps = _host_prep(inputs)
    if B_pad not in _compiled:
        _compiled[B_pad] = _build(B_pad)
    nc = _compiled[B_pad]

    if os.environ.get("BASS_KERNEL_SIM"):
        from concourse.bass_interp import MultiCoreSim
        sim = MultiCoreSim(nc, NCORES)
        for c in range(NCORES):
            for k, v in in_maps[c].items():
                sim.cores[c].tensor(k)[:] = v
        sim.simulate(check_with_hw=False)
        outs = [np.array(sim.cores[c].tensor("out_dram")) for c in range(NCORES)]
    else:
        res = run_bass_kernel_spmd(nc, in_maps, list(range(NCORES)))
        outs = [res.results[c]["out_dram"] for c in range(NCORES)]

    full = np.zeros((N, MUL, 4), np.float32)
    for c in range(NCORES):
        full[c * NPC:(c + 1) * NPC] = (
            outs[c][:NPC].astype(np.float32).reshape(NPC, MUL, 4))
    return full


# revision 11
# speedup vs baseline: 4.4238x; 1.0784x over previous
"""Trainium2 Bass kernel for nn_DiffusionInteractionBlock (GNN message passing).

Strategy: shard EDGES by receiver node range across 8 cores (receiver-sharded
edge parallelism).  Each core owns nodes [c*1250, (c+1)*1250) and processes
exactly the edges whose receiver lands in its range, so the segment-sum is
fully local.  Node-level linear tables are computed SHARDED (1/8 of nodes per
core) and exchanged with an on-device AllGather, so each core ships only its
1/8 slice of node features over the host link (in fp8).  Per-edge
sender rows are fetched with indirect DMA, the edge MLP + tensor product run
on-chip, and messages scatter-sum into PSUM via a single one-hot matmul pair
per 128-edge block (node-partitioned aggregate).  The per-node-tile edge
pipeline runs under a hardware For_i loop so the program stays small — the
host-side per-call cost of shipping/loading the program scales with
instruction count.

Host-side prep (inside kernel()): sorting edges by (core, node-tile),
padding each (core, tile) edge list to a uniform block count so all 8 cores
run the same program (SPMD), packing per-edge side arrays in fp8/bf16/int16,
and folding / pre-scaling weight matrices into one packed tensor that is
itself sharded across cores and AllGathered on device.
"""

import os
import sys

import numpy as np

sys.path.insert(0, "/opt/trn_rl_repo")

import ml_dtypes

from concourse import bacc, bass, mybir, tile
from concourse.bass import ds
from concourse.bass_utils import run_bass_kernel_spmd

BF16 = ml_dtypes.bfloat16
FP8 = ml_dtypes.float8_e4m3fn

N = 10000
E = 160000
MUL = 128
NCORES = 8
NPC = N // NCORES  # 1250 nodes per core (edge/receiver sharding)
NT = 10            # node tiles of 128 per core (1280 >= 1250)
NPAD = 80 * 128    # padded node count for the tables (10240)
NSH = NPAD // NCORES  # 1280 nodes per core-shard (node-table sharding)
SQ3 = float(np.sqrt(3.0))
INV = 1.0 / np.sqrt(MUL)
OUT_SCALE = 1.0 / (np.sqrt(2 * MUL) * 16.0)

dt = mybir.dt

USE_SILU = os.environ.get("BASS_NO_SILU", "") != "1"

_compiled = {}


# --------------------------------------------------------------------------
# Host-side preprocessing
# --------------------------------------------------------------------------

def _host_prep(inputs):
    node_feats = np.asarray(inputs["node_feats"], np.float32)
    edge_attrs = np.asarray(inputs["edge_attrs"], np.float32)
    edge_feats = np.asarray(inputs["edge_feats"], np.float32)
    lengths = np.asarray(inputs["lengths"], np.float32)
    edge_index = np.asarray(inputs["edge_index"], np.int64)
    W_scalar = np.asarray(inputs["W_scalar"], np.float32)
    W_up0 = np.asarray(inputs["W_up0"], np.float32)
    W_up1 = np.asarray(inputs["W_up1"], np.float32)
    W1 = np.asarray(inputs["W1"], np.float32)
    b1 = np.asarray(inputs["b1"], np.float32)
    W2 = np.asarray(inputs["W2"], np.float32)
    b2 = np.asarray(inputs["b2"], np.float32)
    W3 = np.asarray(inputs["W3"], np.float32)
    Wout0 = np.asarray(inputs["Wout0"], np.float32)
    Wout1 = np.asarray(inputs["Wout1"], np.float32)

    sender, receiver = edge_index[0], edge_index[1]

    # global tile id: core * NT + local tile
    loc = receiver % NPC
    gtile = (receiver // NPC) * NT + loc // 128
    counts = np.bincount(gtile, minlength=NCORES * NT)
    B_pad = int(np.ceil(counts.max() / 128))
    EPT = 128 * B_pad                 # edges per node-tile (padded)

    # edge ids grouped by gtile; -1 marks padding
    order = np.argsort(gtile, kind="stable")
    epos = np.full((NCORES * NT, EPT), -1, np.int64)
    off = 0
    for g in range(NCORES * NT):
        c = counts[g]
        epos[g, :c] = order[off:off + c]
        off += c

    # per-edge padded values (pad: sender -> node 0, y/tail -> 0)
    valid = epos >= 0
    ep = np.where(valid, epos, 0)
    es = np.where(valid, sender[ep], 0).astype(np.int16)        # [G, EPT]
    rl = np.where(valid, loc[ep] % 128, 0).astype(np.float32)   # local id in tile
    y = np.where(valid[..., None], edge_attrs[ep], 0.0)         # [G, EPT, 4]
    ef = np.where(valid[..., None], edge_feats[ep], 0.0)        # [G, EPT, 8]
    ln = np.where(valid[..., None], lengths[ep], 0.0)           # [G, EPT, 1]
    one = valid.astype(np.float32)[..., None]                   # bias row (0 on pads)

    def wrap_idx(a):  # [G, EPT] -> [NCORES, 128, NT*B_pad] (col = t*B_pad+b)
        a = a.reshape(NCORES, NT, B_pad, 128)
        a = np.transpose(a, (0, 3, 1, 2)).reshape(NCORES, 128, NT * B_pad)
        return np.ascontiguousarray(a)

    idx_s = wrap_idx(es)

    # tail10 [NCORES, NT, 10, EPT]: rows = [ef(8), len, indicator] per edge
    tail = np.concatenate([ef, ln, one], axis=-1)               # [G, EPT, 10]
    tail10 = np.transpose(tail.reshape(NCORES, NT, EPT, 10), (0, 1, 3, 2))
    tail10 = np.ascontiguousarray(tail10.astype(FP8))

    # yrl [NCORES, NT, 128, 5, B_pad]: f=0:y0, f=1..3:y1_i, f=4:recvloc
    yv = np.concatenate([y, rl[..., None]], axis=-1)            # [G, EPT, 5]
    yv = yv.reshape(NCORES, NT, B_pad, 128, 5)                  # j = b*128 + e
    yrl = np.transpose(yv, (0, 1, 3, 4, 2))                     # [NC, NT, 128, 5, B]
    yrl = np.ascontiguousarray(yrl.astype(BF16))

    # --- node feature shards (transposed for matmul lhsT) ---
    x0 = node_feats[:, :MUL]                                    # [N,128]
    x1 = node_feats[:, MUL:].reshape(N, MUL, 3)
    nfT = np.zeros((512, NPAD), np.float32)
    nfT[0:128, :N] = x0.T
    for i in range(3):
        nfT[128 * (i + 1):128 * (i + 2), :N] = x1[:, :, i].T
    # int8 quantization, one scale per node column
    nfsc = np.maximum(np.abs(nfT).max(axis=0), 1e-20) / 127.0   # [NPAD]
    nfq = np.rint(nfT / nfsc[None, :]).astype(np.int8)
    nfsc = nfsc.astype(np.float32)

    Ws_inv = W_scalar * INV
    # packed 128-row weights [128, 1664]
    wpack = np.concatenate([
        W_up0 * INV,                       # 0:128    h0
        W_up1 * INV,                       # 128:256  h1
        Ws_inv @ W1[:MUL],                 # 256:384  P_s
        Ws_inv @ W1[MUL:2 * MUL],          # 384:512  P_r
        W2,                                # 512:640
        np.concatenate(                    # 640:1152 W3 (with /sqrt3 fold)
            [W3[:, :MUL], W3[:, MUL:2 * MUL] / SQ3,
             W3[:, 2 * MUL:3 * MUL], W3[:, 3 * MUL:]], axis=1),
        Wout0[:MUL] * OUT_SCALE,           # 1152:1280
        Wout0[MUL:] * OUT_SCALE,           # 1280:1408
        Wout1[:MUL] * OUT_SCALE,           # 1408:1536
        Wout1[MUL:] * OUT_SCALE,           # 1536:1664
    ], axis=1).astype(BF16)
    # small-row pack [11, 128]: rows 0-9 = [W1 tail rows; b1], row 10 = b2
    w1c11 = np.concatenate(
        [W1[2 * MUL:], b1[None, :], b2[None, :]], axis=0).astype(BF16)

    tid = np.zeros((NCORES, 128, NT), np.int32)
    for c in range(NCORES):
        for t in range(NT):
            tid[c, :, t] = c * NPC + t * 128 + np.arange(128)
    tid = np.minimum(tid, NPAD - 1)

    WSH = 128 // NCORES  # 16 wpack rows per core
    in_maps = []
    for c in range(NCORES):
        m = {
            "wps": np.ascontiguousarray(wpack[c * WSH:(c + 1) * WSH]),
            "w1c": w1c11,
            "nfs": np.ascontiguousarray(nfq[:, c * NSH:(c + 1) * NSH]),
            "nsc": np.ascontiguousarray(
                nfsc[c * NSH:(c + 1) * NSH].reshape(NSH // 128, 128).T),
            "idx_s": idx_s[c],
            "idx_t": tid[c],
            "tail10": tail10[c].reshape(NT * 10, EPT),
            "yrl": yrl[c].reshape(NT * 128, 5 * B_pad),
        }
        in_maps.append(m)
    return B_pad, in_maps


# --------------------------------------------------------------------------
# Device program
# --------------------------------------------------------------------------

def _build(B_pad):
    EPT = 128 * B_pad
    nc = bacc.Bacc("TRN2", target_bir_lowering=False, debug=False,
                   num_devices=NCORES)

    f32, bf16, fp8, i16, i32 = (dt.float32, dt.bfloat16, dt.float8e4,
                                dt.int16, dt.int32)

    # inputs
    def din(name, shape, dtype):
        return nc.dram_tensor(name, list(shape), dtype, kind="ExternalInput")

    WSH = 128 // NCORES
    nfs = din("nfs", [512, NSH], dt.int8)
    nsc = din("nsc", [128, NSH // 128], f32)
    wps = din("wps", [WSH, 1664], bf16)
    w1c = din("w1c", [11, 128], bf16)
    idx_s = din("idx_s", [128, NT * B_pad], i16)
    idx_t = din("idx_t", [128, NT], i32)
    tail10 = din("tail10", [NT * 10, EPT], fp8)
    yrl = din("yrl", [NT * 128, 5 * B_pad], bf16)

    out_d = nc.dram_tensor("out_dram", [NT * 128, 512], bf16,
                           kind="ExternalOutput")

    # NEFF-embedded constants (no per-call transfer)
    identf = nc.inline_tensor(np.eye(128, dtype=np.float32), name="identf")
    iota = nc.inline_tensor(
        np.tile(np.arange(128, dtype=np.float32), (128, 1)), name="iota")

    # internal DRAM: weight-pack bounce + node tables (part/full)
    wps_bin = nc.dram_tensor("wps_bin", [WSH, 1664], bf16)
    wpk_full = nc.dram_tensor("wpk_full", [128, 1664], bf16)
    T_s_part = nc.dram_tensor("T_s_part", [NSH, 640], bf16)
    T_r_part = nc.dram_tensor("T_r_part", [NSH, 128], bf16)
    T_s = nc.dram_tensor("T_s", [NPAD, 640], bf16)
    T_r = nc.dram_tensor("T_r", [NPAD, 128], bf16)

    AL = mybir.AluOpType
    AF = mybir.ActivationFunctionType
    RG = [list(range(NCORES))]

    with tile.TileContext(nc) as tc:
        with (
            tc.tile_pool(name="const", bufs=1) as cp,
            tc.tile_pool(name="work", bufs=2) as wp,
            tc.tile_pool(name="gath", bufs=2) as gp,
            tc.tile_pool(name="psB", bufs=1, space="PSUM") as psB,
            tc.tile_pool(name="psC", bufs=2, space="PSUM") as psC,
            tc.tile_pool(name="psAgg", bufs=1, space="PSUM") as psAgg,
        ):
            # ---- gather the packed weights across cores ----
            nc.gpsimd.dma_start(out=wps_bin[:, :], in_=wps[:, :])
            nc.gpsimd.collective_compute(
                "AllGather", AL.bypass, replica_groups=RG,
                ins=[wps_bin[:, :].opt()], outs=[wpk_full[:, :].opt()])

            # ---- load constants to SBUF ----
            def ld(src, p, fdim, dtype, pool=cp):
                t = pool.tile([p, fdim], dtype, tag=f"c_{src.name}")
                nc.sync.dma_start(out=t[:, :], in_=src[:, :])
                return t

            wpk_s = ld(wpk_full, 128, 1664, bf16)
            wup0_s = wpk_s[:, 0:128]
            wup1_s = wpk_s[:, 128:256]
            wps_s = wpk_s[:, 256:384]
            wpr_s = wpk_s[:, 384:512]
            w2_s = wpk_s[:, 512:640]
            w3_s = wpk_s[:, 640:1152]
            wo0t_s = wpk_s[:, 1152:1280]
            wo0b_s = wpk_s[:, 1280:1408]
            wo1t_s = wpk_s[:, 1408:1536]
            wo1b_s = wpk_s[:, 1536:1664]
            w1c_s = ld(w1c, 11, 128, bf16)
            w1c10 = w1c_s[0:10, :]
            b2_s = cp.tile([1, 128], bf16, tag="c_b2")
            nc.sync.dma_start(out=b2_s[:, :], in_=w1c[10:11, :])
            idf_s = ld(identf, 128, 128, f32)
            iota_s = ld(iota, 128, 128, f32)
            idb_s = cp.tile([128, 128], bf16, tag="c_idb")
            nc.vector.tensor_copy(out=idb_s[:, :], in_=idf_s[:, :])
            ones_s = cp.tile([1, 128], bf16, tag="c_ones")
            nc.vector.memset(ones_s[:, :], 1.0)
            ix16_s = ld(idx_s, 128, NT * B_pad, i16)
            ixs_s = cp.tile([128, NT * B_pad], i32, tag="c_ixs")
            nc.vector.tensor_copy(out=ixs_s[:, :], in_=ix16_s[:, :])
            ixt_s = ld(idx_t, 128, NT, i32)
            nsc_s = ld(nsc, 128, NSH // 128, f32)

            # ---- node-table phase (this core's 1/8 shard) ----
            with tc.tile_pool(name="nodes", bufs=1) as npool:
                xts = []
                for k in range(4):
                    x8 = npool.tile([128, NSH], dt.int8, tag=f"n8_{k}")
                    nc.sync.dma_start(
                        out=x8[:, :], in_=nfs[128 * k:128 * (k + 1), :])
                    xb = npool.tile([128, NSH], bf16, tag=f"nb_{k}")
                    nc.vector.tensor_copy(out=xb[:, :], in_=x8[:, :])
                    xts.append(xb)
                x0t, x1t0, x1t1, x1t2 = xts
                for s in range(NSH // 128):
                    sl = slice(128 * s, 128 * (s + 1))
                    pn = psAgg.tile([128, 1024], f32, tag="agg")
                    mms = [
                        (x0t, wup0_s, 0),      # h0
                        (x1t0, wup1_s, 128),   # h1_0
                        (x1t1, wup1_s, 256),
                        (x1t2, wup1_s, 384),
                        (x0t, wps_s, 512),     # P_s
                        (x0t, wpr_s, 640),     # P_r
                    ]
                    for lhs, rhs, o in mms:
                        nc.tensor.matmul(out=pn[:, o:o + 128], lhsT=lhs[:, sl],
                                         rhs=rhs, start=True, stop=True)
                    tsb = wp.tile([128, 640], bf16, tag="tsb")
                    trb = wp.tile([128, 128], bf16, tag="trb")
                    nc.scalar.activation(out=tsb[:, :], in_=pn[:, 0:640],
                                         func=AF.Copy, scale=nsc_s[:, s:s + 1])
                    nc.scalar.activation(out=trb[:, :], in_=pn[:, 640:768],
                                         func=AF.Copy, scale=nsc_s[:, s:s + 1])
                    nc.sync.dma_start(out=T_s_part[sl, :], in_=tsb[:, :])
                    nc.sync.dma_start(out=T_r_part[sl, :], in_=trb[:, :])

            # ---- all-gather the node tables across the 8 cores ----
            nc.gpsimd.collective_compute(
                "AllGather", AL.bypass, replica_groups=RG,
                ins=[T_s_part[:, :].opt()], outs=[T_s[:, :].opt()])
            nc.gpsimd.collective_compute(
                "AllGather", AL.bypass, replica_groups=RG,
                ins=[T_r_part[:, :].opt()], outs=[T_r[:, :].opt()])

            # ---- edge phase (hardware loop over node tiles) ----
            BB = 4  # blocks per batch-group
            with tc.For_i(0, NT, 1) as t:
                # stage this tile's gather offsets into fixed tiles so the
                # indirect DMAs see static access patterns
                ixcur = wp.tile([128, B_pad + 1], i32, tag="ixcur")
                nc.vector.tensor_copy(out=ixcur[:, 0:B_pad],
                                      in_=ixs_s[:, ds(t * B_pad, B_pad)])
                nc.vector.tensor_copy(out=ixcur[:, B_pad:B_pad + 1],
                                      in_=ixt_s[:, ds(t, 1)])
                gs_t = gp.tile([128, B_pad * 640], bf16, tag="gs")
                for b in range(B_pad):
                    nc.gpsimd.indirect_dma_start(
                        out=gs_t[:, 640 * b:640 * (b + 1)], out_offset=None,
                        in_=T_s[:, :],
                        in_offset=bass.IndirectOffsetOnAxis(
                            ap=ixcur[:, b:b + 1], axis=0))
                prt = gp.tile([128, 128], bf16, tag="prt")
                nc.gpsimd.indirect_dma_start(
                    out=prt[:, :], out_offset=None, in_=T_r[:, :],
                    in_offset=bass.IndirectOffsetOnAxis(
                        ap=ixcur[:, B_pad:B_pad + 1], axis=0))
                tl8_t = wp.tile([10, EPT], fp8, tag="tail8")
                nc.sync.dma_start(out=tl8_t[:, :], in_=tail10[ds(t * 10, 10), :])
                tl_t = wp.tile([10, EPT], bf16, tag="tail")
                nc.vector.tensor_copy(out=tl_t[:, :], in_=tl8_t[:, :])
                yb_t = wp.tile([128, 5 * B_pad], bf16, tag="yrlb")
                nc.sync.dma_start(out=yb_t[:, :], in_=yrl[ds(t * 128, 128), :])
                y_t = wp.tile([128, 5 * B_pad], f32, tag="yrl")
                nc.vector.tensor_copy(out=y_t[:, :], in_=yb_t[:, :])

                # selection matrices (f32 master for the PE transpose)
                sp_f = wp.tile([128, B_pad * 128], f32, tag="spf")
                rl3 = y_t[:, 4 * B_pad:5 * B_pad].unsqueeze(2)
                nc.vector.tensor_tensor(
                    out=sp_f[:, :].rearrange("p (b n) -> p b n", n=128),
                    in0=rl3.to_broadcast([128, B_pad, 128]),
                    in1=iota_s[:, :].unsqueeze(1).to_broadcast(
                        [128, B_pad, 128]),
                    op=AL.is_equal)
                sp_t = wp.tile([128, B_pad * 128], bf16, tag="spl")
                nc.vector.tensor_copy(out=sp_t[:, :], in_=sp_f[:, :])
                # transposed one-hot (node-partition) built on-device
                st_t = gp.tile([128, EPT], bf16, tag="stT")
                for q0 in range(0, B_pad, 4):
                    qn = min(4, B_pad - q0)
                    pq = psC.tile([128, 512], f32, tag="ptw")
                    for j in range(qn):
                        nc.tensor.transpose(
                            out=pq[:, 128 * j:128 * (j + 1)],
                            in_=sp_f[:, 128 * (q0 + j):128 * (q0 + j + 1)],
                            identity=idf_s[:, :])
                    nc.scalar.activation(
                        out=st_t[:, 128 * q0:128 * (q0 + qn)],
                        in_=pq[:, :128 * qn], func=AF.Copy)

                agg = psAgg.tile([128, 1024], f32, tag="agg")

                nb_groups = (B_pad + BB - 1) // BB
                for g in range(nb_groups):
                    b0 = g * BB
                    gsz = min(BB, B_pad - b0)
                    p1 = psB.tile([128, 128 * BB], f32, tag="p1")
                    for bi in range(gsz):
                        b = b0 + bi
                        o = 128 * bi
                        nc.tensor.matmul(out=p1[:, o:o + 128],
                                         lhsT=tl_t[:, 128 * b:128 * (b + 1)],
                                         rhs=w1c10, start=True, stop=False)
                        nc.tensor.matmul(out=p1[:, o:o + 128], lhsT=idb_s[:, :],
                                         rhs=gs_t[:, 640 * b + 512:640 * b + 640],
                                         start=False, stop=False)
                        nc.tensor.matmul(out=p1[:, o:o + 128],
                                         lhsT=st_t[:, 128 * b:128 * (b + 1)],
                                         rhs=prt[:, :],
                                         start=False, stop=True)
                    h1 = wp.tile([128, 128 * BB], f32, tag="h1")
                    if USE_SILU:
                        nc.scalar.activation(out=h1[:, :128 * gsz],
                                             in_=p1[:, :128 * gsz], func=AF.Silu)
                    else:
                        sg1 = wp.tile([128, 128 * BB], f32, tag="sg1")
                        nc.scalar.activation(out=sg1[:, :128 * gsz],
                                             in_=p1[:, :128 * gsz], func=AF.Sigmoid)
                        nc.vector.tensor_tensor(out=h1[:, :128 * gsz],
                                                in0=p1[:, :128 * gsz],
                                                in1=sg1[:, :128 * gsz], op=AL.mult)
                    pt1 = psB.tile([128, 128 * BB], f32, tag="pt1")
                    for bi in range(gsz):
                        o = 128 * bi
                        nc.tensor.transpose(out=pt1[:, o:o + 128],
                                            in_=h1[:, o:o + 128], identity=idf_s[:, :])
                    h1t = wp.tile([128, 128 * BB], bf16, tag="h1t")
                    nc.scalar.activation(out=h1t[:, :128 * gsz],
                                         in_=pt1[:, :128 * gsz], func=AF.Copy)

                    p2 = psB.tile([128, 128 * BB], f32, tag="p2")
                    for bi in range(gsz):
                        o = 128 * bi
                        nc.tensor.matmul(out=p2[:, o:o + 128], lhsT=h1t[:, o:o + 128],
                                         rhs=w2_s, start=True, stop=False)
                        nc.tensor.matmul(out=p2[:, o:o + 128], lhsT=ones_s[:, :],
                                         rhs=b2_s[:, :], start=False, stop=True)
                    h2 = wp.tile([128, 128 * BB], f32, tag="h2")
                    if USE_SILU:
                        nc.scalar.activation(out=h2[:, :128 * gsz],
                                             in_=p2[:, :128 * gsz], func=AF.Silu)
                    else:
                        sg2 = wp.tile([128, 128 * BB], f32, tag="sg2")
                        nc.scalar.activation(out=sg2[:, :128 * gsz],
                                             in_=p2[:, :128 * gsz], func=AF.Sigmoid)
                        nc.vector.tensor_tensor(out=h2[:, :128 * gsz],
                                                in0=p2[:, :128 * gsz],
                                                in1=sg2[:, :128 * gsz], op=AL.mult)
                    pt2 = psB.tile([128, 128 * BB], f32, tag="pt2")
                    for bi in range(gsz):
                        o = 128 * bi
                        nc.tensor.transpose(out=pt2[:, o:o + 128],
                                            in_=h2[:, o:o + 128], identity=idf_s[:, :])
                    h2t = wp.tile([128, 128 * BB], bf16, tag="h2t")
                    nc.scalar.activation(out=h2t[:, :128 * gsz],
                                         in_=pt2[:, :128 * gsz], func=AF.Copy)

                    for bi in range(gsz):
                        b = b0 + bi
                        o = 128 * bi
                        ptw = psC.tile([128, 512], f32, tag="ptw")
                        nc.tensor.matmul(out=ptw[:, :], lhsT=h2t[:, o:o + 128],
                                         rhs=w3_s, start=True, stop=True)
                        tpw = wp.tile([128, 512], bf16, tag="tpw")
                        nc.scalar.activation(out=tpw[:, :], in_=ptw[:, :],
                                             func=AF.Copy)

                        xs0 = gs_t[:, 640 * b:640 * b + 128]
                        xs1 = gs_t[:, 640 * b + 128:640 * b + 512]
                        y0 = y_t[:, b:b + 1]
                        y13b = (y_t[:, B_pad + b:4 * B_pad:B_pad]
                                .unsqueeze(2)
                                .to_broadcast([128, 3, 128]))
                        # msgs layout: [A(128) | D(128) | B(384) | C(384)]
                        msgs = wp.tile([128, 1024], bf16, tag="msgs")
                        # A = xs0*wA*y0
                        nc.vector.tensor_tensor(out=msgs[:, 0:128], in0=xs0,
                                                in1=tpw[:, 0:128], op=AL.mult)
                        nc.scalar.activation(out=msgs[:, 0:128],
                                             in_=msgs[:, 0:128],
                                             func=AF.Copy, scale=y0)
                        # D = (sum_i xs1_i*y1_i) * wD
                        pd = wp.tile([128, 384], bf16, tag="pd")
                        nc.vector.tensor_tensor(
                            out=pd[:, :].rearrange("p (f n) -> p f n", f=3),
                            in0=xs1.rearrange("p (f n) -> p f n", f=3),
                            in1=y13b, op=AL.mult)
                        dd = wp.tile([128, 128], f32, tag="dd")
                        nc.vector.tensor_reduce(
                            out=dd[:, :],
                            in_=pd[:, :].rearrange("p (f n) -> p n f", f=3),
                            axis=mybir.AxisListType.X, op=AL.add)
                        nc.vector.tensor_tensor(out=msgs[:, 128:256],
                                                in0=dd[:, :],
                                                in1=tpw[:, 128:256], op=AL.mult)
                        # B_i = xs0*wB*y1_i
                        pb = wp.tile([128, 128], bf16, tag="pb")
                        nc.vector.tensor_tensor(out=pb[:, :], in0=xs0,
                                                in1=tpw[:, 256:384], op=AL.mult)
                        nc.vector.tensor_tensor(
                            out=msgs[:, 256:640].rearrange(
                                "p (f n) -> p f n", f=3),
                            in0=pb[:, :].unsqueeze(1).to_broadcast(
                                [128, 3, 128]),
                            in1=y13b, op=AL.mult)
                        # C_i = xs1_i*wC*y0
                        wc3 = tpw[:, 384:512].unsqueeze(1).to_broadcast(
                            [128, 3, 128])
                        nc.vector.tensor_tensor(
                            out=msgs[:, 640:1024].rearrange(
                                "p (f n) -> p f n", f=3),
                            in0=xs1.rearrange("p (f n) -> p f n", f=3),
                            in1=wc3, op=AL.mult)
                        nc.scalar.activation(out=msgs[:, 640:1024],
                                             in_=msgs[:, 640:1024],
                                             func=AF.Copy, scale=y0)

                        # node-partitioned scatter: agg[n, X] += sum_e sp[e,n]*msgs[e,X]
                        lastb = (b == B_pad - 1)
                        sp_b = sp_t[:, 128 * b:128 * (b + 1)]
                        nc.tensor.matmul(out=agg[:, 0:512], lhsT=sp_b,
                                         rhs=msgs[:, 0:512],
                                         start=(b == 0), stop=lastb,
                                         skip_group_check=True)
                        nc.tensor.matmul(out=agg[:, 512:1024], lhsT=sp_b,
                                         rhs=msgs[:, 512:1024],
                                         start=(b == 0), stop=lastb,
                                         skip_group_check=True)

                # ---- transpose aggregate back to channel-partition ----
                aggc = wp.tile([128, 1024], f32, tag="aggc")
                nc.scalar.activation(out=aggc[:, :], in_=agg[:, :], func=AF.Copy)
                aggT = wp.tile([128, 1024], bf16, tag="aggT")
                for h in range(2):
                    pq = psC.tile([128, 512], f32, tag="ptw")
                    for j in range(4):
                        nc.tensor.transpose(
                            out=pq[:, 128 * j:128 * (j + 1)],
                            in_=aggc[:, 512 * h + 128 * j:512 * h + 128 * (j + 1)],
                            identity=idf_s[:, :])
                    nc.scalar.activation(out=aggT[:, 512 * h:512 * (h + 1)],
                                         in_=pq[:, :], func=AF.Copy)

                # ---- final linear for this node tile ----
                # aggT chunks: [A | D | B0 B1 B2 | C0 C1 C2], each [m=128, n=128]
                pf = psC.tile([128, 512], f32, tag="ptw")
                nc.tensor.matmul(out=pf[:, 0:128], lhsT=aggT[:, 0:128],
                                 rhs=wo0t_s, start=True, stop=False,
                                 skip_group_check=True)
                nc.tensor.matmul(out=pf[:, 0:128], lhsT=aggT[:, 128:256],
                                 rhs=wo0b_s, start=False, stop=True,
                                 skip_group_check=True)
                for i in range(3):
                    o = 128 * (i + 1)
                    nc.tensor.matmul(out=pf[:, o:o + 128],
                                     lhsT=aggT[:, 256 + 128 * i:384 + 128 * i],
                                     rhs=wo1t_s, start=True, stop=False,
                                     skip_group_check=True)
                    nc.tensor.matmul(out=pf[:, o:o + 128],
                                     lhsT=aggT[:, 640 + 128 * i:768 + 128 * i],
                                     rhs=wo1b_s, start=False, stop=True,
                                     skip_group_check=True)
                outs = wp.tile([128, 512], bf16, tag="outs")
                ov = outs[:, :].rearrange("p (m c) -> p m c", c=4)
                for c4 in range(4):
                    nc.vector.tensor_copy(out=ov[:, :, c4],
                                          in_=pf[:, 128 * c4:128 * (c4 + 1)])
                nc.sync.dma_start(out=out_d[ds(t * 128, 128), :],
                                  in_=outs[:, :])

    nc.compile()
    return nc


# --------------------------------------------------------------------------
# Entry point
# --------------------------------------------------------------------------

def kernel(**inputs):
    B_pad, in_maps = _host_prep(inputs)
    if B_pad not in _compiled:
        _compiled[B_pad] = _build(B_pad)
    nc = _compiled[B_pad]

    if os.environ.get("BASS_KERNEL_SIM"):
        from concourse.bass_interp import MultiCoreSim
        sim = MultiCoreSim(nc, NCORES)
        for c in range(NCORES):
            for k, v in in_maps[c].items():
                sim.cores[c].tensor(k)[:] = v
        sim.simulate(check_with_hw=False)
        outs = [np.array(sim.cores[c].tensor("out_dram")) for c in range(NCORES)]
    else:
        res = run_bass_kernel_spmd(nc, in_maps, list(range(NCORES)))
        outs = [res.results[c]["out_dram"] for c in range(NCORES)]

    full = np.zeros((N, MUL, 4), np.float32)
    for c in range(NCORES):
        full[c * NPC:(c + 1) * NPC] = (
            outs[c][:NPC].astype(np.float32).reshape(NPC, MUL, 4))
    return full


# revision 14
# speedup vs baseline: 4.7277x; 1.0687x over previous
"""Trainium2 Bass kernel for nn_DiffusionInteractionBlock (GNN message passing).

Strategy: shard EDGES by receiver node range across 8 cores (receiver-sharded
edge parallelism).  Each core owns nodes [c*1250, (c+1)*1250) and processes
exactly the edges whose receiver lands in its range, so the segment-sum is
fully local.  Node-level linear tables are computed SHARDED (1/8 of nodes per
core) and exchanged with an on-device AllGather, so each core ships only its
1/8 slice of node features over the host link (in fp8).  Per-edge
sender rows are fetched with indirect DMA, the edge MLP + tensor product run
on-chip, and messages scatter-sum into PSUM via a single one-hot matmul pair
per 128-edge block (node-partitioned aggregate).  The per-node-tile edge
pipeline runs under a hardware For_i loop so the program stays small — the
host-side per-call cost of shipping/loading the program scales with
instruction count.

Host-side prep (inside kernel()): sorting edges by (core, node-tile),
padding each (core, tile) edge list to a uniform block count so all 8 cores
run the same program (SPMD), packing per-edge side arrays in fp8/bf16/int16,
and folding / pre-scaling weight matrices into one packed tensor that is
itself sharded across cores and AllGathered on device.
"""

import os
import sys

import numpy as np

sys.path.insert(0, "/opt/trn_rl_repo")

import ml_dtypes

from concourse import bacc, bass, mybir, tile
from concourse.bass import ds
from concourse.bass_utils import run_bass_kernel_spmd

BF16 = ml_dtypes.bfloat16
FP8 = ml_dtypes.float8_e4m3fn

N = 10000
E = 160000
MUL = 128
NCORES = 8
NPC = N // NCORES  # 1250 nodes per core (edge/receiver sharding)
NT = 10            # node tiles of 128 per core (1280 >= 1250)
NPAD = 80 * 128    # padded node count for the tables (10240)
NSH = NPAD // NCORES  # 1280 nodes per core-shard (node-table sharding)
SQ3 = float(np.sqrt(3.0))
INV = 1.0 / np.sqrt(MUL)
OUT_SCALE = 1.0 / (np.sqrt(2 * MUL) * 16.0)

dt = mybir.dt

USE_SILU = os.environ.get("BASS_NO_SILU", "") != "1"

_compiled = {}


# --------------------------------------------------------------------------
# Host-side preprocessing
# --------------------------------------------------------------------------

def _host_prep(inputs):
    node_feats = np.asarray(inputs["node_feats"], np.float32)
    edge_attrs = np.asarray(inputs["edge_attrs"], np.float32)
    edge_feats = np.asarray(inputs["edge_feats"], np.float32)
    lengths = np.asarray(inputs["lengths"], np.float32)
    edge_index = np.asarray(inputs["edge_index"], np.int64)
    W_scalar = np.asarray(inputs["W_scalar"], np.float32)
    W_up0 = np.asarray(inputs["W_up0"], np.float32)
    W_up1 = np.asarray(inputs["W_up1"], np.float32)
    W1 = np.asarray(inputs["W1"], np.float32)
    b1 = np.asarray(inputs["b1"], np.float32)
    W2 = np.asarray(inputs["W2"], np.float32)
    b2 = np.asarray(inputs["b2"], np.float32)
    W3 = np.asarray(inputs["W3"], np.float32)
    Wout0 = np.asarray(inputs["Wout0"], np.float32)
    Wout1 = np.asarray(inputs["Wout1"], np.float32)

    sender, receiver = edge_index[0], edge_index[1]

    # global tile id: core * NT + local tile
    loc = receiver % NPC
    gtile = (receiver // NPC) * NT + loc // 128
    counts = np.bincount(gtile, minlength=NCORES * NT)
    B_pad = int(np.ceil(counts.max() / 128))
    EPT = 128 * B_pad                 # edges per node-tile (padded)

    # edge ids grouped by gtile; -1 marks padding
    order = np.argsort(gtile, kind="stable")
    epos = np.full((NCORES * NT, EPT), -1, np.int64)
    off = 0
    for g in range(NCORES * NT):
        c = counts[g]
        epos[g, :c] = order[off:off + c]
        off += c

    # per-edge padded values (pad: sender -> node 0, y/tail -> 0)
    valid = epos >= 0
    ep = np.where(valid, epos, 0)
    es = np.where(valid, sender[ep], 0).astype(np.int16)        # [G, EPT]
    rl = np.where(valid, loc[ep] % 128, 0).astype(np.float32)   # local id in tile
    y = np.where(valid[..., None], edge_attrs[ep], 0.0)         # [G, EPT, 4]
    ef = np.where(valid[..., None], edge_feats[ep], 0.0)        # [G, EPT, 8]
    ln = np.where(valid[..., None], lengths[ep], 0.0)           # [G, EPT, 1]
    one = valid.astype(np.float32)[..., None]                   # bias row (0 on pads)

    def wrap_idx(a):  # [G, EPT] -> [NCORES, 128, NT*B_pad] (col = t*B_pad+b)
        a = a.reshape(NCORES, NT, B_pad, 128)
        a = np.transpose(a, (0, 3, 1, 2)).reshape(NCORES, 128, NT * B_pad)
        return np.ascontiguousarray(a)

    idx_s = wrap_idx(es)

    # tail10 [NCORES, NT, 10, EPT]: rows = [ef(8), len, indicator] per edge
    tail = np.concatenate([ef, ln, one], axis=-1)               # [G, EPT, 10]
    tail10 = np.transpose(tail.reshape(NCORES, NT, EPT, 10), (0, 1, 3, 2))
    tail10 = np.ascontiguousarray(tail10.astype(FP8))

    # yrl [NCORES, NT, 128, 5, B_pad]: f=0:y0, f=1..3:y1_i, f=4:recvloc
    yv = np.concatenate([y, rl[..., None]], axis=-1)            # [G, EPT, 5]
    yv = yv.reshape(NCORES, NT, B_pad, 128, 5)                  # j = b*128 + e
    yrl = np.transpose(yv, (0, 1, 3, 4, 2))                     # [NC, NT, 128, 5, B]
    yrl = np.ascontiguousarray(yrl.astype(BF16))

    # --- node feature shards (transposed for matmul lhsT) ---
    x0 = node_feats[:, :MUL]                                    # [N,128]
    x1 = node_feats[:, MUL:].reshape(N, MUL, 3)
    nfT = np.zeros((512, NPAD), np.float32)
    nfT[0:128, :N] = x0.T
    for i in range(3):
        nfT[128 * (i + 1):128 * (i + 2), :N] = x1[:, :, i].T
    # int8 quantization, one scale per node column
    nfsc = np.maximum(np.abs(nfT).max(axis=0), 1e-20) / 127.0   # [NPAD]
    nfq = np.rint(nfT / nfsc[None, :]).astype(np.int8)
    nfsc = nfsc.astype(np.float32)

    Ws_inv = W_scalar * INV
    # packed 128-row weights [128, 1664]
    wpack = np.concatenate([
        W_up0 * INV,                       # 0:128    h0
        W_up1 * INV,                       # 128:256  h1
        Ws_inv @ W1[:MUL],                 # 256:384  P_s
        Ws_inv @ W1[MUL:2 * MUL],          # 384:512  P_r
        W2,                                # 512:640
        np.concatenate(                    # 640:1152 W3 (with /sqrt3 fold)
            [W3[:, :MUL], W3[:, MUL:2 * MUL] / SQ3,
             W3[:, 2 * MUL:3 * MUL], W3[:, 3 * MUL:]], axis=1),
        Wout0[:MUL] * OUT_SCALE,           # 1152:1280
        Wout0[MUL:] * OUT_SCALE,           # 1280:1408
        Wout1[:MUL] * OUT_SCALE,           # 1408:1536
        Wout1[MUL:] * OUT_SCALE,           # 1536:1664
    ], axis=1).astype(BF16)
    # small-row pack [11, 128]: rows 0-9 = [W1 tail rows; b1], row 10 = b2
    w1c11 = np.concatenate(
        [W1[2 * MUL:], b1[None, :], b2[None, :]], axis=0).astype(BF16)

    tid = np.zeros((NCORES, 128, NT), np.int32)
    for c in range(NCORES):
        for t in range(NT):
            tid[c, :, t] = c * NPC + t * 128 + np.arange(128)
    tid = np.minimum(tid, NPAD - 1)

    WSH = 128 // NCORES  # 16 wpack rows per core
    in_maps = []
    for c in range(NCORES):
        m = {
            "wps": np.ascontiguousarray(wpack[c * WSH:(c + 1) * WSH]),
            "w1c": w1c11,
            "nfs": np.ascontiguousarray(nfq[:, c * NSH:(c + 1) * NSH]),
            "nsc": np.ascontiguousarray(
                nfsc[c * NSH:(c + 1) * NSH].reshape(NSH // 128, 128).T),
            "idx_s": idx_s[c],
            "idx_t": tid[c],
            "tail10": tail10[c].reshape(NT * 10, EPT),
            "yrl": yrl[c].reshape(NT * 128, 5 * B_pad),
        }
        in_maps.append(m)
    return B_pad, in_maps


# --------------------------------------------------------------------------
# Device program
# --------------------------------------------------------------------------

def _build(B_pad):
    EPT = 128 * B_pad
    nc = bacc.Bacc("TRN2", target_bir_lowering=False, debug=False,
                   num_devices=NCORES)

    f32, bf16, fp8, i16, i32 = (dt.float32, dt.bfloat16, dt.float8e4,
                                dt.int16, dt.int32)

    # inputs
    def din(name, shape, dtype):
        return nc.dram_tensor(name, list(shape), dtype, kind="ExternalInput")

    WSH = 128 // NCORES
    nfs = din("nfs", [512, NSH], dt.int8)
    nsc = din("nsc", [128, NSH // 128], f32)
    wps = din("wps", [WSH, 1664], bf16)
    w1c = din("w1c", [11, 128], bf16)
    idx_s = din("idx_s", [128, NT * B_pad], i16)
    idx_t = din("idx_t", [128, NT], i32)
    tail10 = din("tail10", [NT * 10, EPT], fp8)
    yrl = din("yrl", [NT * 128, 5 * B_pad], bf16)

    out_d = nc.dram_tensor("out_dram", [NT * 128, 512], bf16,
                           kind="ExternalOutput")

    # NEFF-embedded constants (no per-call transfer)
    identf = nc.inline_tensor(np.eye(128, dtype=np.float32), name="identf")
    iota = nc.inline_tensor(
        np.tile(np.arange(128, dtype=np.float32), (128, 1)), name="iota")

    # internal DRAM: weight-pack bounce + node tables (part/full)
    wps_bin = nc.dram_tensor("wps_bin", [WSH, 1664], bf16)
    wpk_full = nc.dram_tensor("wpk_full", [128, 1664], bf16)
    T_s_part = nc.dram_tensor("T_s_part", [NSH, 640], bf16)
    T_r_part = nc.dram_tensor("T_r_part", [NSH, 128], bf16)
    T_s = nc.dram_tensor("T_s", [NPAD, 640], bf16)
    T_r = nc.dram_tensor("T_r", [NPAD, 128], bf16)

    AL = mybir.AluOpType
    AF = mybir.ActivationFunctionType
    RG = [list(range(NCORES))]

    with tile.TileContext(nc) as tc:
        with (
            tc.tile_pool(name="const", bufs=1) as cp,
            tc.tile_pool(name="work", bufs=2) as wp,
            tc.tile_pool(name="gath", bufs=2) as gp,
            tc.tile_pool(name="psB", bufs=1, space="PSUM") as psB,
            tc.tile_pool(name="psC", bufs=2, space="PSUM") as psC,
            tc.tile_pool(name="psAgg", bufs=1, space="PSUM") as psAgg,
        ):
            # ---- gather the packed weights across cores ----
            nc.gpsimd.dma_start(out=wps_bin[:, :], in_=wps[:, :])
            nc.gpsimd.collective_compute(
                "AllGather", AL.bypass, replica_groups=RG,
                ins=[wps_bin[:, :].opt()], outs=[wpk_full[:, :].opt()])

            # ---- load constants to SBUF ----
            def ld(src, p, fdim, dtype, pool=cp):
                t = pool.tile([p, fdim], dtype, tag=f"c_{src.name}")
                nc.sync.dma_start(out=t[:, :], in_=src[:, :])
                return t

            wpk_s = ld(wpk_full, 128, 1664, bf16)
            wup0_s = wpk_s[:, 0:128]
            wup1_s = wpk_s[:, 128:256]
            wps_s = wpk_s[:, 256:384]
            wpr_s = wpk_s[:, 384:512]
            w2_s = wpk_s[:, 512:640]
            w3_s = wpk_s[:, 640:1152]
            wo0t_s = wpk_s[:, 1152:1280]
            wo0b_s = wpk_s[:, 1280:1408]
            wo1t_s = wpk_s[:, 1408:1536]
            wo1b_s = wpk_s[:, 1536:1664]
            w1c_s = ld(w1c, 11, 128, bf16)
            w1c10 = w1c_s[0:10, :]
            b2_s = cp.tile([1, 128], bf16, tag="c_b2")
            nc.sync.dma_start(out=b2_s[:, :], in_=w1c[10:11, :])
            idf_s = ld(identf, 128, 128, f32)
            iota_s = ld(iota, 128, 128, f32)
            idb_s = cp.tile([128, 128], bf16, tag="c_idb")
            nc.vector.tensor_copy(out=idb_s[:, :], in_=idf_s[:, :])
            ones_s = cp.tile([1, 128], bf16, tag="c_ones")
            nc.vector.memset(ones_s[:, :], 1.0)
            ix16_s = ld(idx_s, 128, NT * B_pad, i16)
            ixs_s = cp.tile([128, NT * B_pad], i32, tag="c_ixs")
            nc.vector.tensor_copy(out=ixs_s[:, :], in_=ix16_s[:, :])
            ixt_s = ld(idx_t, 128, NT, i32)
            nsc_s = ld(nsc, 128, NSH // 128, f32)

            # ---- node-table phase (this core's 1/8 shard) ----
            with tc.tile_pool(name="nodes", bufs=1) as npool:
                xts = []
                for k in range(4):
                    x8 = npool.tile([128, NSH], dt.int8, tag=f"n8_{k}")
                    nc.sync.dma_start(
                        out=x8[:, :], in_=nfs[128 * k:128 * (k + 1), :])
                    xb = npool.tile([128, NSH], bf16, tag=f"nb_{k}")
                    nc.vector.tensor_copy(out=xb[:, :], in_=x8[:, :])
                    xts.append(xb)
                x0t, x1t0, x1t1, x1t2 = xts
                for s in range(NSH // 128):
                    sl = slice(128 * s, 128 * (s + 1))
                    pn = psAgg.tile([128, 1024], f32, tag="agg")
                    mms = [
                        (x0t, wup0_s, 0),      # h0
                        (x1t0, wup1_s, 128),   # h1_0
                        (x1t1, wup1_s, 256),
                        (x1t2, wup1_s, 384),
                        (x0t, wps_s, 512),     # P_s
                        (x0t, wpr_s, 640),     # P_r
                    ]
                    for lhs, rhs, o in mms:
                        nc.tensor.matmul(out=pn[:, o:o + 128], lhsT=lhs[:, sl],
                                         rhs=rhs, start=True, stop=True)
                    tsb = wp.tile([128, 640], bf16, tag="tsb")
                    trb = wp.tile([128, 128], bf16, tag="trb")
                    nc.scalar.activation(out=tsb[:, :], in_=pn[:, 0:640],
                                         func=AF.Copy, scale=nsc_s[:, s:s + 1])
                    nc.scalar.activation(out=trb[:, :], in_=pn[:, 640:768],
                                         func=AF.Copy, scale=nsc_s[:, s:s + 1])
                    nc.sync.dma_start(out=T_s_part[sl, :], in_=tsb[:, :])
                    nc.sync.dma_start(out=T_r_part[sl, :], in_=trb[:, :])

            # ---- all-gather the node tables across the 8 cores ----
            nc.gpsimd.collective_compute(
                "AllGather", AL.bypass, replica_groups=RG,
                ins=[T_s_part[:, :].opt()], outs=[T_s[:, :].opt()])
            nc.gpsimd.collective_compute(
                "AllGather", AL.bypass, replica_groups=RG,
                ins=[T_r_part[:, :].opt()], outs=[T_r[:, :].opt()])

            # ---- edge phase (hardware loop over node tiles) ----
            BB = 4  # blocks per batch-group
            with tc.For_i(0, NT, 1) as t:
                # stage this tile's gather offsets into fixed tiles so the
                # indirect DMAs see static access patterns
                ixcur = wp.tile([128, B_pad + 1], i32, tag="ixcur")
                nc.vector.tensor_copy(out=ixcur[:, 0:B_pad],
                                      in_=ixs_s[:, ds(t * B_pad, B_pad)])
                nc.vector.tensor_copy(out=ixcur[:, B_pad:B_pad + 1],
                                      in_=ixt_s[:, ds(t, 1)])
                gs_t = gp.tile([128, B_pad * 640], bf16, tag="gs")
                for b in range(B_pad):
                    nc.gpsimd.indirect_dma_start(
                        out=gs_t[:, 640 * b:640 * (b + 1)], out_offset=None,
                        in_=T_s[:, :],
                        in_offset=bass.IndirectOffsetOnAxis(
                            ap=ixcur[:, b:b + 1], axis=0))
                prt = gp.tile([128, 128], bf16, tag="prt")
                nc.gpsimd.indirect_dma_start(
                    out=prt[:, :], out_offset=None, in_=T_r[:, :],
                    in_offset=bass.IndirectOffsetOnAxis(
                        ap=ixcur[:, B_pad:B_pad + 1], axis=0))
                tl8_t = wp.tile([10, EPT], fp8, tag="tail8")
                nc.sync.dma_start(out=tl8_t[:, :], in_=tail10[ds(t * 10, 10), :])
                tl_t = wp.tile([10, EPT], bf16, tag="tail")
                nc.vector.tensor_copy(out=tl_t[:, :], in_=tl8_t[:, :])
                yb_t = wp.tile([128, 5 * B_pad], bf16, tag="yrlb")
                nc.sync.dma_start(out=yb_t[:, :], in_=yrl[ds(t * 128, 128), :])
                y_t = wp.tile([128, 5 * B_pad], f32, tag="yrl")
                nc.vector.tensor_copy(out=y_t[:, :], in_=yb_t[:, :])

                # selection matrices (f32 master for the PE transpose)
                sp_f = wp.tile([128, B_pad * 128], f32, tag="spf")
                rl3 = y_t[:, 4 * B_pad:5 * B_pad].unsqueeze(2)
                nc.vector.tensor_tensor(
                    out=sp_f[:, :].rearrange("p (b n) -> p b n", n=128),
                    in0=rl3.to_broadcast([128, B_pad, 128]),
                    in1=iota_s[:, :].unsqueeze(1).to_broadcast(
                        [128, B_pad, 128]),
                    op=AL.is_equal)
                sp_t = wp.tile([128, B_pad * 128], bf16, tag="spl")
                nc.vector.tensor_copy(out=sp_t[:, :], in_=sp_f[:, :])
                # transposed one-hot (node-partition) built on-device
                st_t = gp.tile([128, EPT], bf16, tag="stT")
                for q0 in range(0, B_pad, 4):
                    qn = min(4, B_pad - q0)
                    pq = psC.tile([128, 512], f32, tag="ptw")
                    for j in range(qn):
                        nc.tensor.transpose(
                            out=pq[:, 128 * j:128 * (j + 1)],
                            in_=sp_f[:, 128 * (q0 + j):128 * (q0 + j + 1)],
                            identity=idf_s[:, :])
                    nc.scalar.activation(
                        out=st_t[:, 128 * q0:128 * (q0 + qn)],
                        in_=pq[:, :128 * qn], func=AF.Copy)

                agg = psAgg.tile([128, 1024], f32, tag="agg")

                nb_groups = (B_pad + BB - 1) // BB
                for g in range(nb_groups):
                    b0 = g * BB
                    gsz = min(BB, B_pad - b0)
                    p1 = psB.tile([128, 128 * BB], f32, tag="p1")
                    for bi in range(gsz):
                        b = b0 + bi
                        o = 128 * bi
                        nc.tensor.matmul(out=p1[:, o:o + 128],
                                         lhsT=tl_t[:, 128 * b:128 * (b + 1)],
                                         rhs=w1c10, start=True, stop=False)
                        nc.tensor.matmul(out=p1[:, o:o + 128], lhsT=idb_s[:, :],
                                         rhs=gs_t[:, 640 * b + 512:640 * b + 640],
                                         start=False, stop=False)
                        nc.tensor.matmul(out=p1[:, o:o + 128],
                                         lhsT=st_t[:, 128 * b:128 * (b + 1)],
                                         rhs=prt[:, :],
                                         start=False, stop=True)
                    h1 = wp.tile([128, 128 * BB], f32, tag="h1")
                    if USE_SILU:
                        nc.scalar.activation(out=h1[:, :128 * gsz],
                                             in_=p1[:, :128 * gsz], func=AF.Silu)
                    else:
                        sg1 = wp.tile([128, 128 * BB], f32, tag="sg1")
                        nc.scalar.activation(out=sg1[:, :128 * gsz],
                                             in_=p1[:, :128 * gsz], func=AF.Sigmoid)
                        nc.vector.tensor_tensor(out=h1[:, :128 * gsz],
                                                in0=p1[:, :128 * gsz],
                                                in1=sg1[:, :128 * gsz], op=AL.mult)
                    pt1 = psB.tile([128, 128 * BB], f32, tag="pt1")
                    for bi in range(gsz):
                        o = 128 * bi
                        nc.tensor.transpose(out=pt1[:, o:o + 128],
                                            in_=h1[:, o:o + 128], identity=idf_s[:, :])
                    h1t = wp.tile([128, 128 * BB], bf16, tag="h1t")
                    nc.scalar.activation(out=h1t[:, :128 * gsz],
                                         in_=pt1[:, :128 * gsz], func=AF.Copy)

                    p2 = psB.tile([128, 128 * BB], f32, tag="p2")
                    for bi in range(gsz):
                        o = 128 * bi
                        nc.tensor.matmul(out=p2[:, o:o + 128], lhsT=h1t[:, o:o + 128],
                                         rhs=w2_s, start=True, stop=False)
                        nc.tensor.matmul(out=p2[:, o:o + 128], lhsT=ones_s[:, :],
                                         rhs=b2_s[:, :], start=False, stop=True)
                    h2 = wp.tile([128, 128 * BB], f32, tag="h2")
                    if USE_SILU:
                        nc.scalar.activation(out=h2[:, :128 * gsz],
                                             in_=p2[:, :128 * gsz], func=AF.Silu)
                    else:
                        sg2 = wp.tile([128, 128 * BB], f32, tag="sg2")
                        nc.scalar.activation(out=sg2[:, :128 * gsz],
                                             in_=p2[:, :128 * gsz], func=AF.Sigmoid)
                        nc.vector.tensor_tensor(out=h2[:, :128 * gsz],
                                                in0=p2[:, :128 * gsz],
                                                in1=sg2[:, :128 * gsz], op=AL.mult)
                    pt2 = psB.tile([128, 128 * BB], f32, tag="pt2")
                    for bi in range(gsz):
                        o = 128 * bi
                        nc.tensor.transpose(out=pt2[:, o:o + 128],
                                            in_=h2[:, o:o + 128], identity=idf_s[:, :])
                    h2t = wp.tile([128, 128 * BB], bf16, tag="h2t")
                    nc.scalar.activation(out=h2t[:, :128 * gsz],
                                         in_=pt2[:, :128 * gsz], func=AF.Copy)

                    for bi in range(gsz):
                        b = b0 + bi
                        o = 128 * bi
                        ptw = psC.tile([128, 512], f32, tag="ptw")
                        nc.tensor.matmul(out=ptw[:, :], lhsT=h2t[:, o:o + 128],
                                         rhs=w3_s, start=True, stop=True)
                        tpw = wp.tile([128, 512], bf16, tag="tpw")
                        nc.scalar.activation(out=tpw[:, :], in_=ptw[:, :],
                                             func=AF.Copy)

                        xs0 = gs_t[:, 640 * b:640 * b + 128]
                        xs1 = gs_t[:, 640 * b + 128:640 * b + 512]
                        y0 = y_t[:, b:b + 1]
                        y13b = (y_t[:, B_pad + b:4 * B_pad:B_pad]
                                .unsqueeze(2)
                                .to_broadcast([128, 3, 128]))
                        # msgs layout: [A(128) | D(128) | B(384) | C(384)]
                        msgs = wp.tile([128, 1024], bf16, tag="msgs")
                        # A = xs0*wA*y0
                        nc.vector.tensor_tensor(out=msgs[:, 0:128], in0=xs0,
                                                in1=tpw[:, 0:128], op=AL.mult)
                        nc.scalar.activation(out=msgs[:, 0:128],
                                             in_=msgs[:, 0:128],
                                             func=AF.Copy, scale=y0)
                        # D = (sum_i xs1_i*y1_i) * wD
                        pd = wp.tile([128, 384], bf16, tag="pd")
                        nc.vector.tensor_tensor(
                            out=pd[:, :].rearrange("p (f n) -> p f n", f=3),
                            in0=xs1.rearrange("p (f n) -> p f n", f=3),
                            in1=y13b, op=AL.mult)
                        dd = wp.tile([128, 128], f32, tag="dd")
                        nc.vector.tensor_reduce(
                            out=dd[:, :],
                            in_=pd[:, :].rearrange("p (f n) -> p n f", f=3),
                            axis=mybir.AxisListType.X, op=AL.add)
                        nc.vector.tensor_tensor(out=msgs[:, 128:256],
                                                in0=dd[:, :],
                                                in1=tpw[:, 128:256], op=AL.mult)
                        # B_i = xs0*wB*y1_i
                        pb = wp.tile([128, 128], bf16, tag="pb")
                        nc.vector.tensor_tensor(out=pb[:, :], in0=xs0,
                                                in1=tpw[:, 256:384], op=AL.mult)
                        nc.vector.tensor_tensor(
                            out=msgs[:, 256:640].rearrange(
                                "p (f n) -> p f n", f=3),
                            in0=pb[:, :].unsqueeze(1).to_broadcast(
                                [128, 3, 128]),
                            in1=y13b, op=AL.mult)
                        # C_i = xs1_i*wC*y0
                        wc3 = tpw[:, 384:512].unsqueeze(1).to_broadcast(
                            [128, 3, 128])
                        nc.vector.tensor_tensor(
                            out=msgs[:, 640:1024].rearrange(
                                "p (f n) -> p f n", f=3),
                            in0=xs1.rearrange("p (f n) -> p f n", f=3),
                            in1=wc3, op=AL.mult)
                        nc.scalar.activation(out=msgs[:, 640:1024],
                                             in_=msgs[:, 640:1024],
                                             func=AF.Copy, scale=y0)

                        # node-partitioned scatter: agg[n, X] += sum_e sp[e,n]*msgs[e,X]
                        lastb = (b == B_pad - 1)
                        sp_b = sp_t[:, 128 * b:128 * (b + 1)]
                        nc.tensor.matmul(out=agg[:, 0:512], lhsT=sp_b,
                                         rhs=msgs[:, 0:512],
                                         start=(b == 0), stop=lastb,
                                         skip_group_check=True)
                        nc.tensor.matmul(out=agg[:, 512:1024], lhsT=sp_b,
                                         rhs=msgs[:, 512:1024],
                                         start=(b == 0), stop=lastb,
                                         skip_group_check=True)

                # ---- transpose aggregate back to channel-partition ----
                aggc = wp.tile([128, 1024], f32, tag="aggc")
                nc.scalar.activation(out=aggc[:, :], in_=agg[:, :], func=AF.Copy)
                aggT = wp.tile([128, 1024], bf16, tag="aggT")
                for h in range(2):
                    pq = psC.tile([128, 512], f32, tag="ptw")
                    for j in range(4):
                        nc.tensor.transpose(
                            out=pq[:, 128 * j:128 * (j + 1)],
                            in_=aggc[:, 512 * h + 128 * j:512 * h + 128 * (j + 1)],
                            identity=idf_s[:, :])
                    nc.scalar.activation(out=aggT[:, 512 * h:512 * (h + 1)],
                                         in_=pq[:, :], func=AF.Copy)

                # ---- final linear for this node tile ----
                # aggT chunks: [A | D | B0 B1 B2 | C0 C1 C2], each [m=128, n=128]
                pf = psC.tile([128, 512], f32, tag="ptw")
                nc.tensor.matmul(out=pf[:, 0:128], lhsT=aggT[:, 0:128],
                                 rhs=wo0t_s, start=True, stop=False,
                                 skip_group_check=True)
                nc.tensor.matmul(out=pf[:, 0:128], lhsT=aggT[:, 128:256],
                                 rhs=wo0b_s, start=False, stop=True,
                                 skip_group_check=True)
                for i in range(3):
                    o = 128 * (i + 1)
                    nc.tensor.matmul(out=pf[:, o:o + 128],
                                     lhsT=aggT[:, 256 + 128 * i:384 + 128 * i],
                                     rhs=wo1t_s, start=True, stop=False,
                                     skip_group_check=True)
                    nc.tensor.matmul(out=pf[:, o:o + 128],
                                     lhsT=aggT[:, 640 + 128 * i:768 + 128 * i],
                                     rhs=wo1b_s, start=False, stop=True,
                                     skip_group_check=True)
                outs = wp.tile([128, 512], bf16, tag="outs")
                ov = outs[:, :].rearrange("p (m c) -> p m c", c=4)
                for c4 in range(4):
                    nc.vector.tensor_copy(out=ov[:, :, c4],
                                          in_=pf[:, 128 * c4:128 * (c4 + 1)])
                nc.sync.dma_start(out=out_d[ds(t * 128, 128), :],
                                  in_=outs[:, :])

    nc.compile()
    return nc


# --------------------------------------------------------------------------
# Entry point
# --------------------------------------------------------------------------

def kernel(**inputs):
    B_pad, in_maps = _host_prep(inputs)
    if B_pad not in _compiled:
        _compiled[B_pad] = _build(B_pad)
    nc = _compiled[B_pad]

    if os.environ.get("BASS_KERNEL_SIM"):
        from concourse.bass_interp import MultiCoreSim
        sim = MultiCoreSim(nc, NCORES)
        for c in range(NCORES):
            for k, v in in_maps[c].items():
                sim.cores[c].tensor(k)[:] = v
        sim.simulate(check_with_hw=False)
        outs = [np.array(sim.cores[c].tensor("out_dram")) for c in range(NCORES)]
    else:
        res = run_bass_kernel_spmd(nc, in_maps, list(range(NCORES)))
        outs = [res.results[c]["out_dram"] for c in range(NCORES)]

    full = np.zeros((N, MUL, 4), np.float32)
    for c in range(NCORES):
        full[c * NPC:(c + 1) * NPC] = (
            outs[c][:NPC].astype(np.float32).reshape(NPC, MUL, 4))
    return full


# revision 16
# speedup vs baseline: 5.5241x; 1.1685x over previous
"""Trainium2 Bass kernel for nn_DiffusionInteractionBlock (GNN message passing).

Strategy: shard EDGES by receiver node range across 8 cores (receiver-sharded
edge parallelism).  Each core owns nodes [c*1250, (c+1)*1250) and processes
exactly the edges whose receiver lands in its range, so the segment-sum is
fully local.  Node-level linear tables are computed SHARDED (1/8 of nodes per
core) and exchanged with an on-device AllGather, so each core ships only its
1/8 slice of node features over the host link (in fp8).  Per-edge
sender rows are fetched with indirect DMA, the edge MLP + tensor product run
on-chip, and messages scatter-sum into PSUM via a single one-hot matmul pair
per 128-edge block (node-partitioned aggregate).  The per-node-tile edge
pipeline runs under a hardware For_i loop so the program stays small — the
host-side per-call cost of shipping/loading the program scales with
instruction count.

Host-side prep (inside kernel()): sorting edges by (core, node-tile),
padding each (core, tile) edge list to a uniform block count so all 8 cores
run the same program (SPMD), packing per-edge side arrays in fp8/bf16/int16,
and folding / pre-scaling weight matrices into one packed tensor that is
itself sharded across cores and AllGathered on device.
"""

import os
import sys

import numpy as np

sys.path.insert(0, "/opt/trn_rl_repo")

import ml_dtypes

from concourse import bacc, bass, mybir, tile
from concourse.bass import ds
from concourse.bass_utils import run_bass_kernel_spmd

BF16 = ml_dtypes.bfloat16
FP8 = ml_dtypes.float8_e4m3fn

N = 10000
E = 160000
MUL = 128
NCORES = 8
NPC = N // NCORES  # 1250 nodes per core (edge/receiver sharding)
NT = 10            # node tiles of 128 per core (1280 >= 1250)
NPAD = 80 * 128    # padded node count for the tables (10240)
NSH = NPAD // NCORES  # 1280 nodes per core-shard (node-table sharding)
SQ3 = float(np.sqrt(3.0))
INV = 1.0 / np.sqrt(MUL)
OUT_SCALE = 1.0 / (np.sqrt(2 * MUL) * 16.0)

dt = mybir.dt

USE_SILU = os.environ.get("BASS_NO_SILU", "") != "1"

_compiled = {}


# --------------------------------------------------------------------------
# Host-side preprocessing
# --------------------------------------------------------------------------

def _host_prep(inputs):
    node_feats = np.asarray(inputs["node_feats"], np.float32)
    edge_attrs = np.asarray(inputs["edge_attrs"], np.float32)
    edge_feats = np.asarray(inputs["edge_feats"], np.float32)
    lengths = np.asarray(inputs["lengths"], np.float32)
    edge_index = np.asarray(inputs["edge_index"], np.int64)
    W_scalar = np.asarray(inputs["W_scalar"], np.float32)
    W_up0 = np.asarray(inputs["W_up0"], np.float32)
    W_up1 = np.asarray(inputs["W_up1"], np.float32)
    W1 = np.asarray(inputs["W1"], np.float32)
    b1 = np.asarray(inputs["b1"], np.float32)
    W2 = np.asarray(inputs["W2"], np.float32)
    b2 = np.asarray(inputs["b2"], np.float32)
    W3 = np.asarray(inputs["W3"], np.float32)
    Wout0 = np.asarray(inputs["Wout0"], np.float32)
    Wout1 = np.asarray(inputs["Wout1"], np.float32)

    sender, receiver = edge_index[0], edge_index[1]

    # global tile id: core * NT + local tile
    loc = receiver % NPC
    gtile = (receiver // NPC) * NT + loc // 128
    counts = np.bincount(gtile, minlength=NCORES * NT)
    B_pad = int(np.ceil(counts.max() / 128))
    EPT = 128 * B_pad                 # edges per node-tile (padded)

    # edge ids grouped by gtile; -1 marks padding
    order = np.argsort(gtile, kind="stable")
    epos = np.full((NCORES * NT, EPT), -1, np.int64)
    off = 0
    for g in range(NCORES * NT):
        c = counts[g]
        epos[g, :c] = order[off:off + c]
        off += c

    # per-edge padded values (pad: sender -> node 0, y/tail -> 0)
    valid = epos >= 0
    ep = np.where(valid, epos, 0)
    es = np.where(valid, sender[ep], 0).astype(np.int16)        # [G, EPT]
    rl = np.where(valid, loc[ep] % 128, 0).astype(np.float32)   # local id in tile
    y = np.where(valid[..., None], edge_attrs[ep], 0.0)         # [G, EPT, 4]
    ef = np.where(valid[..., None], edge_feats[ep], 0.0)        # [G, EPT, 8]
    ln = np.where(valid[..., None], lengths[ep], 0.0)           # [G, EPT, 1]
    one = valid.astype(np.float32)[..., None]                   # bias row (0 on pads)

    def wrap_idx(a):  # [G, EPT] -> [NCORES, 128, NT*B_pad] (col = t*B_pad+b)
        a = a.reshape(NCORES, NT, B_pad, 128)
        a = np.transpose(a, (0, 3, 1, 2)).reshape(NCORES, 128, NT * B_pad)
        return np.ascontiguousarray(a)

    idx_s = wrap_idx(es)

    # tail10 [NCORES, NT, 10, EPT]: rows = [ef(8), len, indicator] per edge
    tail = np.concatenate([ef, ln, one], axis=-1)               # [G, EPT, 10]
    tail10 = np.transpose(tail.reshape(NCORES, NT, EPT, 10), (0, 1, 3, 2))
    tail10 = np.ascontiguousarray(tail10.astype(FP8))

    # yrl [NCORES, NT, 128, 5, B_pad]: f=0:y0, f=1..3:y1_i, f=4:recvloc
    yv = np.concatenate([y, rl[..., None]], axis=-1)            # [G, EPT, 5]
    yv = yv.reshape(NCORES, NT, B_pad, 128, 5)                  # j = b*128 + e
    yrl = np.transpose(yv, (0, 1, 3, 4, 2))                     # [NC, NT, 128, 5, B]
    yrl = np.ascontiguousarray(yrl.astype(BF16))

    # --- node feature shards (transposed for matmul lhsT) ---
    x0 = node_feats[:, :MUL]                                    # [N,128]
    x1 = node_feats[:, MUL:].reshape(N, MUL, 3)
    nfT = np.zeros((512, NPAD), np.float32)
    nfT[0:128, :N] = x0.T
    for i in range(3):
        nfT[128 * (i + 1):128 * (i + 2), :N] = x1[:, :, i].T
    # int8 quantization, one scale per node column
    nfsc = np.maximum(np.abs(nfT).max(axis=0), 1e-20) / 127.0   # [NPAD]
    nfq = np.rint(nfT / nfsc[None, :]).astype(np.int8)
    nfsc = nfsc.astype(np.float32)

    Ws_inv = W_scalar * INV
    # packed 128-row weights [128, 1664]
    wpack = np.concatenate([
        W_up0 * INV,                       # 0:128    h0
        W_up1 * INV,                       # 128:256  h1
        Ws_inv @ W1[:MUL],                 # 256:384  P_s
        Ws_inv @ W1[MUL:2 * MUL],          # 384:512  P_r
        W2,                                # 512:640
        np.concatenate(                    # 640:1152 W3 (with /sqrt3 fold)
            [W3[:, :MUL], W3[:, MUL:2 * MUL] / SQ3,
             W3[:, 2 * MUL:3 * MUL], W3[:, 3 * MUL:]], axis=1),
        Wout0[:MUL] * OUT_SCALE,           # 1152:1280
        Wout0[MUL:] * OUT_SCALE,           # 1280:1408
        Wout1[:MUL] * OUT_SCALE,           # 1408:1536
        Wout1[MUL:] * OUT_SCALE,           # 1536:1664
    ], axis=1).astype(BF16)
    # small-row pack [11, 128]: rows 0-9 = [W1 tail rows; b1], row 10 = b2
    w1c11 = np.concatenate(
        [W1[2 * MUL:], b1[None, :], b2[None, :]], axis=0).astype(BF16)

    tid = np.zeros((NCORES, 128, NT), np.int32)
    for c in range(NCORES):
        for t in range(NT):
            tid[c, :, t] = c * NPC + t * 128 + np.arange(128)
    tid = np.minimum(tid, NPAD - 1)

    WSH = 128 // NCORES  # 16 wpack rows per core
    in_maps = []
    for c in range(NCORES):
        m = {
            "wps": np.ascontiguousarray(wpack[c * WSH:(c + 1) * WSH]),
            "w1c": w1c11,
            "nfs": np.ascontiguousarray(nfq[:, c * NSH:(c + 1) * NSH]),
            "nsc": np.ascontiguousarray(
                nfsc[c * NSH:(c + 1) * NSH].reshape(NSH // 128, 128).T),
            "idx_s": idx_s[c],
            "idx_t": tid[c],
            "tail10": tail10[c].reshape(NT * 10, EPT),
            "yrl": yrl[c].reshape(NT * 128, 5 * B_pad),
        }
        in_maps.append(m)
    return B_pad, in_maps


# --------------------------------------------------------------------------
# Device program
# --------------------------------------------------------------------------

def _build(B_pad):
    EPT = 128 * B_pad
    nc = bacc.Bacc("TRN2", target_bir_lowering=False, debug=False,
                   num_devices=NCORES)

    f32, bf16, fp8, i16, i32 = (dt.float32, dt.bfloat16, dt.float8e4,
                                dt.int16, dt.int32)

    # inputs
    def din(name, shape, dtype):
        return nc.dram_tensor(name, list(shape), dtype, kind="ExternalInput")

    WSH = 128 // NCORES
    nfs = din("nfs", [512, NSH], dt.int8)
    nsc = din("nsc", [128, NSH // 128], f32)
    wps = din("wps", [WSH, 1664], bf16)
    w1c = din("w1c", [11, 128], bf16)
    idx_s = din("idx_s", [128, NT * B_pad], i16)
    idx_t = din("idx_t", [128, NT], i32)
    tail10 = din("tail10", [NT * 10, EPT], fp8)
    yrl = din("yrl", [NT * 128, 5 * B_pad], bf16)

    out_d = nc.dram_tensor("out_dram", [NT * 128, 512], dt.int8,
                           kind="ExternalOutput")
    out_sc = nc.dram_tensor("out_sc", [128, NT], f32, kind="ExternalOutput")

    # NEFF-embedded constants (no per-call transfer)
    identf = nc.inline_tensor(np.eye(128, dtype=np.float32), name="identf")
    iota = nc.inline_tensor(
        np.tile(np.arange(128, dtype=np.float32), (128, 1)), name="iota")

    # internal DRAM: weight-pack bounce + node tables (part/full)
    wps_bin = nc.dram_tensor("wps_bin", [WSH, 1664], bf16)
    wpk_full = nc.dram_tensor("wpk_full", [128, 1664], bf16)
    T_s_part = nc.dram_tensor("T_s_part", [NSH, 640], bf16)
    T_r_part = nc.dram_tensor("T_r_part", [NSH, 128], bf16)
    T_s = nc.dram_tensor("T_s", [NPAD, 640], bf16)
    T_r = nc.dram_tensor("T_r", [NPAD, 128], bf16)

    AL = mybir.AluOpType
    AF = mybir.ActivationFunctionType
    RG = [list(range(NCORES))]

    with tile.TileContext(nc) as tc:
        with (
            tc.tile_pool(name="const", bufs=1) as cp,
            tc.tile_pool(name="work", bufs=2) as wp,
            tc.tile_pool(name="gath", bufs=2) as gp,
            tc.tile_pool(name="psB", bufs=1, space="PSUM") as psB,
            tc.tile_pool(name="psC", bufs=2, space="PSUM") as psC,
            tc.tile_pool(name="psAgg", bufs=1, space="PSUM") as psAgg,
        ):
            # ---- gather the packed weights across cores ----
            nc.gpsimd.dma_start(out=wps_bin[:, :], in_=wps[:, :])
            nc.gpsimd.collective_compute(
                "AllGather", AL.bypass, replica_groups=RG,
                ins=[wps_bin[:, :].opt()], outs=[wpk_full[:, :].opt()])

            # ---- load constants to SBUF ----
            def ld(src, p, fdim, dtype, pool=cp):
                t = pool.tile([p, fdim], dtype, tag=f"c_{src.name}")
                nc.sync.dma_start(out=t[:, :], in_=src[:, :])
                return t

            wpk_s = ld(wpk_full, 128, 1664, bf16)
            wup0_s = wpk_s[:, 0:128]
            wup1_s = wpk_s[:, 128:256]
            wps_s = wpk_s[:, 256:384]
            wpr_s = wpk_s[:, 384:512]
            w2_s = wpk_s[:, 512:640]
            w3_s = wpk_s[:, 640:1152]
            wo0t_s = wpk_s[:, 1152:1280]
            wo0b_s = wpk_s[:, 1280:1408]
            wo1t_s = wpk_s[:, 1408:1536]
            wo1b_s = wpk_s[:, 1536:1664]
            w1c_s = ld(w1c, 11, 128, bf16)
            w1c10 = w1c_s[0:10, :]
            b2_s = cp.tile([1, 128], bf16, tag="c_b2")
            nc.sync.dma_start(out=b2_s[:, :], in_=w1c[10:11, :])
            idf_s = ld(identf, 128, 128, f32)
            iota_s = ld(iota, 128, 128, f32)
            idb_s = cp.tile([128, 128], bf16, tag="c_idb")
            nc.vector.tensor_copy(out=idb_s[:, :], in_=idf_s[:, :])
            ones_s = cp.tile([1, 128], bf16, tag="c_ones")
            nc.vector.memset(ones_s[:, :], 1.0)
            ix16_s = ld(idx_s, 128, NT * B_pad, i16)
            ixs_s = cp.tile([128, NT * B_pad], i32, tag="c_ixs")
            nc.vector.tensor_copy(out=ixs_s[:, :], in_=ix16_s[:, :])
            ixt_s = ld(idx_t, 128, NT, i32)
            nsc_s = ld(nsc, 128, NSH // 128, f32)

            # ---- node-table phase (this core's 1/8 shard) ----
            with tc.tile_pool(name="nodes", bufs=1) as npool:
                xts = []
                for k in range(4):
                    x8 = npool.tile([128, NSH], dt.int8, tag=f"n8_{k}")
                    nc.sync.dma_start(
                        out=x8[:, :], in_=nfs[128 * k:128 * (k + 1), :])
                    xb = npool.tile([128, NSH], bf16, tag=f"nb_{k}")
                    nc.vector.tensor_copy(out=xb[:, :], in_=x8[:, :])
                    xts.append(xb)
                x0t, x1t0, x1t1, x1t2 = xts
                for s in range(NSH // 128):
                    sl = slice(128 * s, 128 * (s + 1))
                    pn = psAgg.tile([128, 1024], f32, tag="agg")
                    mms = [
                        (x0t, wup0_s, 0),      # h0
                        (x1t0, wup1_s, 128),   # h1_0
                        (x1t1, wup1_s, 256),
                        (x1t2, wup1_s, 384),
                        (x0t, wps_s, 512),     # P_s
                        (x0t, wpr_s, 640),     # P_r
                    ]
                    for lhs, rhs, o in mms:
                        nc.tensor.matmul(out=pn[:, o:o + 128], lhsT=lhs[:, sl],
                                         rhs=rhs, start=True, stop=True)
                    tsb = wp.tile([128, 640], bf16, tag="tsb")
                    trb = wp.tile([128, 128], bf16, tag="trb")
                    nc.scalar.activation(out=tsb[:, :], in_=pn[:, 0:640],
                                         func=AF.Copy, scale=nsc_s[:, s:s + 1])
                    nc.scalar.activation(out=trb[:, :], in_=pn[:, 640:768],
                                         func=AF.Copy, scale=nsc_s[:, s:s + 1])
                    nc.sync.dma_start(out=T_s_part[sl, :], in_=tsb[:, :])
                    nc.sync.dma_start(out=T_r_part[sl, :], in_=trb[:, :])

            # ---- all-gather the node tables across the 8 cores ----
            nc.gpsimd.collective_compute(
                "AllGather", AL.bypass, replica_groups=RG,
                ins=[T_s_part[:, :].opt()], outs=[T_s[:, :].opt()])
            nc.gpsimd.collective_compute(
                "AllGather", AL.bypass, replica_groups=RG,
                ins=[T_r_part[:, :].opt()], outs=[T_r[:, :].opt()])

            # ---- edge phase (hardware loop over node tiles) ----
            BB = 4  # blocks per batch-group
            with tc.For_i(0, NT, 1) as t:
                # stage this tile's gather offsets into fixed tiles so the
                # indirect DMAs see static access patterns
                ixcur = wp.tile([128, B_pad + 1], i32, tag="ixcur")
                nc.vector.tensor_copy(out=ixcur[:, 0:B_pad],
                                      in_=ixs_s[:, ds(t * B_pad, B_pad)])
                nc.vector.tensor_copy(out=ixcur[:, B_pad:B_pad + 1],
                                      in_=ixt_s[:, ds(t, 1)])
                gs_t = gp.tile([128, B_pad * 640], bf16, tag="gs")
                for b in range(B_pad):
                    nc.gpsimd.indirect_dma_start(
                        out=gs_t[:, 640 * b:640 * (b + 1)], out_offset=None,
                        in_=T_s[:, :],
                        in_offset=bass.IndirectOffsetOnAxis(
                            ap=ixcur[:, b:b + 1], axis=0))
                prt = gp.tile([128, 128], bf16, tag="prt")
                nc.gpsimd.indirect_dma_start(
                    out=prt[:, :], out_offset=None, in_=T_r[:, :],
                    in_offset=bass.IndirectOffsetOnAxis(
                        ap=ixcur[:, B_pad:B_pad + 1], axis=0))
                tl8_t = wp.tile([10, EPT], fp8, tag="tail8")
                nc.sync.dma_start(out=tl8_t[:, :], in_=tail10[ds(t * 10, 10), :])
                tl_t = wp.tile([10, EPT], bf16, tag="tail")
                nc.vector.tensor_copy(out=tl_t[:, :], in_=tl8_t[:, :])
                yb_t = wp.tile([128, 5 * B_pad], bf16, tag="yrlb")
                nc.sync.dma_start(out=yb_t[:, :], in_=yrl[ds(t * 128, 128), :])
                y_t = wp.tile([128, 5 * B_pad], f32, tag="yrl")
                nc.vector.tensor_copy(out=y_t[:, :], in_=yb_t[:, :])

                # selection matrices (f32 master for the PE transpose)
                sp_f = wp.tile([128, B_pad * 128], f32, tag="spf")
                rl3 = y_t[:, 4 * B_pad:5 * B_pad].unsqueeze(2)
                nc.vector.tensor_tensor(
                    out=sp_f[:, :].rearrange("p (b n) -> p b n", n=128),
                    in0=rl3.to_broadcast([128, B_pad, 128]),
                    in1=iota_s[:, :].unsqueeze(1).to_broadcast(
                        [128, B_pad, 128]),
                    op=AL.is_equal)
                sp_t = wp.tile([128, B_pad * 128], bf16, tag="spl")
                nc.vector.tensor_copy(out=sp_t[:, :], in_=sp_f[:, :])
                # transposed one-hot (node-partition) built on-device
                st_t = gp.tile([128, EPT], bf16, tag="stT")
                for q0 in range(0, B_pad, 4):
                    qn = min(4, B_pad - q0)
                    pq = psC.tile([128, 512], f32, tag="ptw")
                    for j in range(qn):
                        nc.tensor.transpose(
                            out=pq[:, 128 * j:128 * (j + 1)],
                            in_=sp_f[:, 128 * (q0 + j):128 * (q0 + j + 1)],
                            identity=idf_s[:, :])
                    nc.scalar.activation(
                        out=st_t[:, 128 * q0:128 * (q0 + qn)],
                        in_=pq[:, :128 * qn], func=AF.Copy)

                agg = psAgg.tile([128, 1024], f32, tag="agg")

                nb_groups = (B_pad + BB - 1) // BB
                for g in range(nb_groups):
                    b0 = g * BB
                    gsz = min(BB, B_pad - b0)
                    p1 = psB.tile([128, 128 * BB], f32, tag="p1")
                    for bi in range(gsz):
                        b = b0 + bi
                        o = 128 * bi
                        nc.tensor.matmul(out=p1[:, o:o + 128],
                                         lhsT=tl_t[:, 128 * b:128 * (b + 1)],
                                         rhs=w1c10, start=True, stop=False)
                        nc.tensor.matmul(out=p1[:, o:o + 128], lhsT=idb_s[:, :],
                                         rhs=gs_t[:, 640 * b + 512:640 * b + 640],
                                         start=False, stop=False)
                        nc.tensor.matmul(out=p1[:, o:o + 128],
                                         lhsT=st_t[:, 128 * b:128 * (b + 1)],
                                         rhs=prt[:, :],
                                         start=False, stop=True)
                    h1 = wp.tile([128, 128 * BB], f32, tag="h1")
                    if USE_SILU:
                        nc.scalar.activation(out=h1[:, :128 * gsz],
                                             in_=p1[:, :128 * gsz], func=AF.Silu)
                    else:
                        sg1 = wp.tile([128, 128 * BB], f32, tag="sg1")
                        nc.scalar.activation(out=sg1[:, :128 * gsz],
                                             in_=p1[:, :128 * gsz], func=AF.Sigmoid)
                        nc.vector.tensor_tensor(out=h1[:, :128 * gsz],
                                                in0=p1[:, :128 * gsz],
                                                in1=sg1[:, :128 * gsz], op=AL.mult)
                    pt1 = psB.tile([128, 128 * BB], f32, tag="pt1")
                    for bi in range(gsz):
                        o = 128 * bi
                        nc.tensor.transpose(out=pt1[:, o:o + 128],
                                            in_=h1[:, o:o + 128], identity=idf_s[:, :])
                    h1t = wp.tile([128, 128 * BB], bf16, tag="h1t")
                    nc.scalar.activation(out=h1t[:, :128 * gsz],
                                         in_=pt1[:, :128 * gsz], func=AF.Copy)

                    p2 = psB.tile([128, 128 * BB], f32, tag="p2")
                    for bi in range(gsz):
                        o = 128 * bi
                        nc.tensor.matmul(out=p2[:, o:o + 128], lhsT=h1t[:, o:o + 128],
                                         rhs=w2_s, start=True, stop=False)
                        nc.tensor.matmul(out=p2[:, o:o + 128], lhsT=ones_s[:, :],
                                         rhs=b2_s[:, :], start=False, stop=True)
                    h2 = wp.tile([128, 128 * BB], f32, tag="h2")
                    if USE_SILU:
                        nc.scalar.activation(out=h2[:, :128 * gsz],
                                             in_=p2[:, :128 * gsz], func=AF.Silu)
                    else:
                        sg2 = wp.tile([128, 128 * BB], f32, tag="sg2")
                        nc.scalar.activation(out=sg2[:, :128 * gsz],
                                             in_=p2[:, :128 * gsz], func=AF.Sigmoid)
                        nc.vector.tensor_tensor(out=h2[:, :128 * gsz],
                                                in0=p2[:, :128 * gsz],
                                                in1=sg2[:, :128 * gsz], op=AL.mult)
                    pt2 = psB.tile([128, 128 * BB], f32, tag="pt2")
                    for bi in range(gsz):
                        o = 128 * bi
                        nc.tensor.transpose(out=pt2[:, o:o + 128],
                                            in_=h2[:, o:o + 128], identity=idf_s[:, :])
                    h2t = wp.tile([128, 128 * BB], bf16, tag="h2t")
                    nc.scalar.activation(out=h2t[:, :128 * gsz],
                                         in_=pt2[:, :128 * gsz], func=AF.Copy)

                    for bi in range(gsz):
                        b = b0 + bi
                        o = 128 * bi
                        ptw = psC.tile([128, 512], f32, tag="ptw")
                        nc.tensor.matmul(out=ptw[:, :], lhsT=h2t[:, o:o + 128],
                                         rhs=w3_s, start=True, stop=True)
                        tpw = wp.tile([128, 512], bf16, tag="tpw")
                        nc.scalar.activation(out=tpw[:, :], in_=ptw[:, :],
                                             func=AF.Copy)

                        xs0 = gs_t[:, 640 * b:640 * b + 128]
                        xs1 = gs_t[:, 640 * b + 128:640 * b + 512]
                        y0 = y_t[:, b:b + 1]
                        y13b = (y_t[:, B_pad + b:4 * B_pad:B_pad]
                                .unsqueeze(2)
                                .to_broadcast([128, 3, 128]))
                        # msgs layout: [A(128) | D(128) | B(384) | C(384)]
                        msgs = wp.tile([128, 1024], bf16, tag="msgs")
                        # A = xs0*wA*y0
                        nc.vector.tensor_tensor(out=msgs[:, 0:128], in0=xs0,
                                                in1=tpw[:, 0:128], op=AL.mult)
                        nc.scalar.activation(out=msgs[:, 0:128],
                                             in_=msgs[:, 0:128],
                                             func=AF.Copy, scale=y0)
                        # D = (sum_i xs1_i*y1_i) * wD
                        pd = wp.tile([128, 384], bf16, tag="pd")
                        nc.vector.tensor_tensor(
                            out=pd[:, :].rearrange("p (f n) -> p f n", f=3),
                            in0=xs1.rearrange("p (f n) -> p f n", f=3),
                            in1=y13b, op=AL.mult)
                        dd = wp.tile([128, 128], f32, tag="dd")
                        nc.vector.tensor_reduce(
                            out=dd[:, :],
                            in_=pd[:, :].rearrange("p (f n) -> p n f", f=3),
                            axis=mybir.AxisListType.X, op=AL.add)
                        nc.vector.tensor_tensor(out=msgs[:, 128:256],
                                                in0=dd[:, :],
                                                in1=tpw[:, 128:256], op=AL.mult)
                        # B_i = xs0*wB*y1_i
                        pb = wp.tile([128, 128], bf16, tag="pb")
                        nc.vector.tensor_tensor(out=pb[:, :], in0=xs0,
                                                in1=tpw[:, 256:384], op=AL.mult)
                        nc.vector.tensor_tensor(
                            out=msgs[:, 256:640].rearrange(
                                "p (f n) -> p f n", f=3),
                            in0=pb[:, :].unsqueeze(1).to_broadcast(
                                [128, 3, 128]),
                            in1=y13b, op=AL.mult)
                        # C_i = xs1_i*wC*y0
                        wc3 = tpw[:, 384:512].unsqueeze(1).to_broadcast(
                            [128, 3, 128])
                        nc.vector.tensor_tensor(
                            out=msgs[:, 640:1024].rearrange(
                                "p (f n) -> p f n", f=3),
                            in0=xs1.rearrange("p (f n) -> p f n", f=3),
                            in1=wc3, op=AL.mult)
                        nc.scalar.activation(out=msgs[:, 640:1024],
                                             in_=msgs[:, 640:1024],
                                             func=AF.Copy, scale=y0)

                        # node-partitioned scatter: agg[n, X] += sum_e sp[e,n]*msgs[e,X]
                        lastb = (b == B_pad - 1)
                        sp_b = sp_t[:, 128 * b:128 * (b + 1)]
                        nc.tensor.matmul(out=agg[:, 0:512], lhsT=sp_b,
                                         rhs=msgs[:, 0:512],
                                         start=(b == 0), stop=lastb,
                                         skip_group_check=True)
                        nc.tensor.matmul(out=agg[:, 512:1024], lhsT=sp_b,
                                         rhs=msgs[:, 512:1024],
                                         start=(b == 0), stop=lastb,
                                         skip_group_check=True)

                # ---- transpose aggregate back to channel-partition ----
                aggc = wp.tile([128, 1024], f32, tag="aggc")
                nc.scalar.activation(out=aggc[:, :], in_=agg[:, :], func=AF.Copy)
                aggT = wp.tile([128, 1024], bf16, tag="aggT")
                for h in range(2):
                    pq = psC.tile([128, 512], f32, tag="ptw")
                    for j in range(4):
                        nc.tensor.transpose(
                            out=pq[:, 128 * j:128 * (j + 1)],
                            in_=aggc[:, 512 * h + 128 * j:512 * h + 128 * (j + 1)],
                            identity=idf_s[:, :])
                    nc.scalar.activation(out=aggT[:, 512 * h:512 * (h + 1)],
                                         in_=pq[:, :], func=AF.Copy)

                # ---- final linear for this node tile ----
                # aggT chunks: [A | D | B0 B1 B2 | C0 C1 C2], each [m=128, n=128]
                pf = psC.tile([128, 512], f32, tag="ptw")
                nc.tensor.matmul(out=pf[:, 0:128], lhsT=aggT[:, 0:128],
                                 rhs=wo0t_s, start=True, stop=False,
                                 skip_group_check=True)
                nc.tensor.matmul(out=pf[:, 0:128], lhsT=aggT[:, 128:256],
                                 rhs=wo0b_s, start=False, stop=True,
                                 skip_group_check=True)
                for i in range(3):
                    o = 128 * (i + 1)
                    nc.tensor.matmul(out=pf[:, o:o + 128],
                                     lhsT=aggT[:, 256 + 128 * i:384 + 128 * i],
                                     rhs=wo1t_s, start=True, stop=False,
                                     skip_group_check=True)
                    nc.tensor.matmul(out=pf[:, o:o + 128],
                                     lhsT=aggT[:, 640 + 128 * i:768 + 128 * i],
                                     rhs=wo1b_s, start=False, stop=True,
                                     skip_group_check=True)
                # int8 output with per-node scale + round-to-nearest
                mx = wp.tile([128, 1], f32, tag="mx")
                nc.vector.tensor_reduce(out=mx[:, :], in_=pf[:, :],
                                        axis=mybir.AxisListType.X, op=AL.max,
                                        apply_absolute_value=True)
                nc.vector.tensor_scalar(out=mx[:, :], in0=mx[:, :],
                                        scalar1=1e-20, scalar2=None,
                                        op0=AL.max)
                inv = wp.tile([128, 1], f32, tag="inv")
                nc.vector.reciprocal(out=inv[:, :], in_=mx[:, :])
                nc.vector.tensor_scalar(out=inv[:, :], in0=inv[:, :],
                                        scalar1=127.0, scalar2=None,
                                        op0=AL.mult)
                qf = wp.tile([128, 512], f32, tag="qf")
                nc.scalar.activation(out=qf[:, :], in_=pf[:, :],
                                     func=AF.Copy, scale=inv[:, :])
                outs = wp.tile([128, 512], dt.int8, tag="outs")
                ov = outs[:, :].rearrange("p (m c) -> p m c", c=4)
                for c4 in range(4):
                    nc.vector.tensor_copy(out=ov[:, :, c4],
                                          in_=qf[:, 128 * c4:128 * (c4 + 1)])
                nc.sync.dma_start(out=out_d[ds(t * 128, 128), :],
                                  in_=outs[:, :])
                nc.sync.dma_start(out=out_sc[:, ds(t, 1)], in_=mx[:, :])

    nc.compile()
    return nc


# --------------------------------------------------------------------------
# Entry point
# --------------------------------------------------------------------------

def kernel(**inputs):
    B_pad, in_maps = _host_prep(inputs)
    if B_pad not in _compiled:
        _compiled[B_pad] = _build(B_pad)
    nc = _compiled[B_pad]

    if os.environ.get("BASS_KERNEL_SIM"):
        from concourse.bass_interp import MultiCoreSim
        sim = MultiCoreSim(nc, NCORES)
        for c in range(NCORES):
            for k, v in in_maps[c].items():
                sim.cores[c].tensor(k)[:] = v
        sim.simulate(check_with_hw=False)
        outs = [np.array(sim.cores[c].tensor("out_dram")) for c in range(NCORES)]
        scs = [np.array(sim.cores[c].tensor("out_sc")) for c in range(NCORES)]
    else:
        res = run_bass_kernel_spmd(nc, in_maps, list(range(NCORES)))
        outs = [res.results[c]["out_dram"] for c in range(NCORES)]
        scs = [res.results[c]["out_sc"] for c in range(NCORES)]

    full = np.zeros((N, MUL, 4), np.float32)
    for c in range(NCORES):
        sc = np.asarray(scs[c], np.float32).T.reshape(NT * 128)[:NPC]
        q = np.asarray(outs[c][:NPC], np.float32) * (sc / 127.0)[:, None]
        full[c * NPC:(c + 1) * NPC] = q.reshape(NPC, MUL, 4)
    return full


# revision 19
# speedup vs baseline: 6.7504x; 1.2220x over previous
"""Trainium2 Bass kernel for nn_DiffusionInteractionBlock (GNN message passing).

Strategy: shard EDGES by receiver node range across 8 cores (receiver-sharded
edge parallelism).  Each core owns nodes [c*1250, (c+1)*1250) and processes
exactly the edges whose receiver lands in its range, so the segment-sum is
fully local.  Node-level linear tables are computed SHARDED (1/8 of nodes per
core) and exchanged with an on-device AllGather, so each core ships only its
1/8 slice of node features over the host link (in fp8).  Per-edge
sender rows are fetched with indirect DMA, the edge MLP + tensor product run
on-chip, and messages scatter-sum into PSUM via a single one-hot matmul pair
per 128-edge block (node-partitioned aggregate).  The per-node-tile edge
pipeline runs under a hardware For_i loop so the program stays small — the
host-side per-call cost of shipping/loading the program scales with
instruction count.

Host-side prep (inside kernel()): sorting edges by (core, node-tile),
padding each (core, tile) edge list to a uniform block count so all 8 cores
run the same program (SPMD), packing per-edge side arrays in fp8/bf16/int16,
and folding / pre-scaling weight matrices into one packed tensor that is
itself sharded across cores and AllGathered on device.
"""

import os
import sys

import numpy as np

sys.path.insert(0, "/opt/trn_rl_repo")

import jax

# Persistent XLA compilation cache: run_bass_kernel_spmd re-jits a fresh
# closure per call, so the in-memory jit cache always misses; the persistent
# cache keys on the HLO fingerprint and skips the identical recompile.
try:
    jax.config.update("jax_compilation_cache_dir", "/tmp/jax_comp_cache")
    jax.config.update("jax_persistent_cache_min_entry_size_bytes", 0)
    jax.config.update("jax_persistent_cache_min_compile_time_secs", 0.0)
except Exception:
    pass

import ml_dtypes

from concourse import bacc, bass, mybir, tile
from concourse.bass import ds
from concourse.bass_utils import run_bass_kernel_spmd

BF16 = ml_dtypes.bfloat16
FP8 = ml_dtypes.float8_e4m3fn

N = 10000
E = 160000
MUL = 128
NCORES = 8
NPC = N // NCORES  # 1250 nodes per core (edge/receiver sharding)
NT = 10            # node tiles of 128 per core (1280 >= 1250)
NPAD = 80 * 128    # padded node count for the tables (10240)
NSH = NPAD // NCORES  # 1280 nodes per core-shard (node-table sharding)
SQ3 = float(np.sqrt(3.0))
INV = 1.0 / np.sqrt(MUL)
OUT_SCALE = 1.0 / (np.sqrt(2 * MUL) * 16.0)

dt = mybir.dt

USE_SILU = os.environ.get("BASS_NO_SILU", "") != "1"

_compiled = {}


# --------------------------------------------------------------------------
# Host-side preprocessing
# --------------------------------------------------------------------------

def _host_prep(inputs):
    node_feats = np.asarray(inputs["node_feats"], np.float32)
    edge_attrs = np.asarray(inputs["edge_attrs"], np.float32)
    edge_feats = np.asarray(inputs["edge_feats"], np.float32)
    lengths = np.asarray(inputs["lengths"], np.float32)
    edge_index = np.asarray(inputs["edge_index"], np.int64)
    W_scalar = np.asarray(inputs["W_scalar"], np.float32)
    W_up0 = np.asarray(inputs["W_up0"], np.float32)
    W_up1 = np.asarray(inputs["W_up1"], np.float32)
    W1 = np.asarray(inputs["W1"], np.float32)
    b1 = np.asarray(inputs["b1"], np.float32)
    W2 = np.asarray(inputs["W2"], np.float32)
    b2 = np.asarray(inputs["b2"], np.float32)
    W3 = np.asarray(inputs["W3"], np.float32)
    Wout0 = np.asarray(inputs["Wout0"], np.float32)
    Wout1 = np.asarray(inputs["Wout1"], np.float32)

    sender, receiver = edge_index[0], edge_index[1]

    # global tile id: core * NT + local tile
    loc = receiver % NPC
    gtile = (receiver // NPC) * NT + loc // 128
    counts = np.bincount(gtile, minlength=NCORES * NT)
    B_pad = int(np.ceil(counts.max() / 128))
    EPT = 128 * B_pad                 # edges per node-tile (padded)

    # edge ids grouped by gtile; -1 marks padding
    order = np.argsort(gtile, kind="stable")
    epos = np.full((NCORES * NT, EPT), -1, np.int64)
    off = 0
    for g in range(NCORES * NT):
        c = counts[g]
        epos[g, :c] = order[off:off + c]
        off += c

    # per-edge padded values (pad: sender -> node 0, y/tail -> 0)
    valid = epos >= 0
    ep = np.where(valid, epos, 0)
    es = np.where(valid, sender[ep], 0).astype(np.int16)        # [G, EPT]
    rl = np.where(valid, loc[ep] % 128, 0).astype(np.float32)   # local id in tile
    y = np.where(valid[..., None], edge_attrs[ep], 0.0)         # [G, EPT, 4]
    ef = np.where(valid[..., None], edge_feats[ep], 0.0)        # [G, EPT, 8]
    ln = np.where(valid[..., None], lengths[ep], 0.0)           # [G, EPT, 1]
    one = valid.astype(np.float32)[..., None]                   # bias row (0 on pads)

    def wrap_idx(a):  # [G, EPT] -> [NCORES, 128, NT*B_pad] (col = t*B_pad+b)
        a = a.reshape(NCORES, NT, B_pad, 128)
        a = np.transpose(a, (0, 3, 1, 2)).reshape(NCORES, 128, NT * B_pad)
        return np.ascontiguousarray(a)

    idx_s = wrap_idx(es)

    # tail10 [NCORES, NT, 10, EPT]: rows = [ef(8), len, indicator] per edge
    tail = np.concatenate([ef, ln, one], axis=-1)               # [G, EPT, 10]
    tail10 = np.transpose(tail.reshape(NCORES, NT, EPT, 10), (0, 1, 3, 2))
    tail10 = np.ascontiguousarray(tail10.astype(FP8))

    # yrl [NCORES, NT, 128, 5, B_pad]: f=0:y0, f=1..3:y1_i, f=4:recvloc
    yv = np.concatenate([y, rl[..., None]], axis=-1)            # [G, EPT, 5]
    yv = yv.reshape(NCORES, NT, B_pad, 128, 5)                  # j = b*128 + e
    yrl = np.transpose(yv, (0, 1, 3, 4, 2))                     # [NC, NT, 128, 5, B]
    yrl = np.ascontiguousarray(yrl.astype(BF16))

    # --- node feature shards (transposed for matmul lhsT) ---
    x0 = node_feats[:, :MUL]                                    # [N,128]
    x1 = node_feats[:, MUL:].reshape(N, MUL, 3)
    nfT = np.zeros((512, NPAD), np.float32)
    nfT[0:128, :N] = x0.T
    for i in range(3):
        nfT[128 * (i + 1):128 * (i + 2), :N] = x1[:, :, i].T
    # int8 quantization, one scale per node column
    nfsc = np.maximum(np.abs(nfT).max(axis=0), 1e-20) / 127.0   # [NPAD]
    nfq = np.rint(nfT / nfsc[None, :]).astype(np.int8)
    nfsc = nfsc.astype(np.float32)

    Ws_inv = W_scalar * INV
    # packed 128-row weights [128, 1664]
    wpack = np.concatenate([
        W_up0 * INV,                       # 0:128    h0
        W_up1 * INV,                       # 128:256  h1
        Ws_inv @ W1[:MUL],                 # 256:384  P_s
        Ws_inv @ W1[MUL:2 * MUL],          # 384:512  P_r
        W2,                                # 512:640
        np.concatenate(                    # 640:1152 W3 (with /sqrt3 fold)
            [W3[:, :MUL], W3[:, MUL:2 * MUL] / SQ3,
             W3[:, 2 * MUL:3 * MUL], W3[:, 3 * MUL:]], axis=1),
        Wout0[:MUL] * OUT_SCALE,           # 1152:1280
        Wout0[MUL:] * OUT_SCALE,           # 1280:1408
        Wout1[:MUL] * OUT_SCALE,           # 1408:1536
        Wout1[MUL:] * OUT_SCALE,           # 1536:1664
    ], axis=1).astype(BF16)
    # small-row pack [11, 128]: rows 0-9 = [W1 tail rows; b1], row 10 = b2
    w1c11 = np.concatenate(
        [W1[2 * MUL:], b1[None, :], b2[None, :]], axis=0).astype(BF16)

    tid = np.zeros((NCORES, 128, NT), np.int32)
    for c in range(NCORES):
        for t in range(NT):
            tid[c, :, t] = c * NPC + t * 128 + np.arange(128)
    tid = np.minimum(tid, NPAD - 1)

    WSH = 128 // NCORES  # 16 wpack rows per core
    in_maps = []
    for c in range(NCORES):
        nsc_c = np.ascontiguousarray(
            nfsc[c * NSH:(c + 1) * NSH].reshape(NSH // 128, 128).T)
        blob = np.concatenate([
            idx_s[c].ravel(),
            tid[c].astype(np.int32).ravel().view(np.int16),
            nsc_c.astype(np.float32).ravel().view(np.int16),
            w1c11.ravel().view(np.int16),
            np.ascontiguousarray(
                wpack[c * WSH:(c + 1) * WSH]).ravel().view(np.int16),
        ])
        m = {
            "blob": np.ascontiguousarray(blob[None, :]),
            "nfs": np.ascontiguousarray(nfq[:, c * NSH:(c + 1) * NSH]),
            "tail10": tail10[c].reshape(NT * 10, EPT),
            "yrl": yrl[c].reshape(NT * 128, 5 * B_pad),
        }
        in_maps.append(m)
    return B_pad, in_maps


# --------------------------------------------------------------------------
# Device program
# --------------------------------------------------------------------------

def _build(B_pad):
    EPT = 128 * B_pad
    nc = bacc.Bacc("TRN2", target_bir_lowering=False, debug=False,
                   num_devices=NCORES)

    f32, bf16, fp8, i16, i32 = (dt.float32, dt.bfloat16, dt.float8e4,
                                dt.int16, dt.int32)

    # inputs
    def din(name, shape, dtype):
        return nc.dram_tensor(name, list(shape), dtype, kind="ExternalInput")

    WSH = 128 // NCORES
    nfs = din("nfs", [512, NSH], dt.int8)
    tail10 = din("tail10", [NT * 10, EPT], fp8)
    yrl = din("yrl", [NT * 128, 5 * B_pad], bf16)
    # packed small constants: [idx_s i16 | idx_t i32 | nsc f32 | w1c bf16 | wps bf16]
    O_IDXT = 128 * NT * B_pad
    O_NSC = O_IDXT + 128 * NT * 2
    O_W1C = O_NSC + 128 * (NSH // 128) * 2
    O_WPS = O_W1C + 11 * 128
    TOTI16 = O_WPS + WSH * 1664
    blob = din("blob", [1, TOTI16], i16)

    out_d = nc.dram_tensor("out_dram", [NT * 128, 512], dt.int8,
                           kind="ExternalOutput")
    out_sc = nc.dram_tensor("out_sc", [128, NT], f32, kind="ExternalOutput")

    # NEFF-embedded constants (no per-call transfer)
    identf = nc.inline_tensor(np.eye(128, dtype=np.float32), name="identf")
    iota = nc.inline_tensor(
        np.tile(np.arange(128, dtype=np.float32), (128, 1)), name="iota")

    # internal DRAM: weight-pack bounce + node tables (part/full)
    wps_bin = nc.dram_tensor("wps_bin", [WSH, 1664], bf16)
    wpk_full = nc.dram_tensor("wpk_full", [128, 1664], bf16)
    T_s_part = nc.dram_tensor("T_s_part", [NSH, 768], bf16)
    T_s = nc.dram_tensor("T_s", [NPAD, 768], bf16)

    AL = mybir.AluOpType
    AF = mybir.ActivationFunctionType
    RG = [list(range(NCORES))]

    with tile.TileContext(nc) as tc:
        with (
            tc.tile_pool(name="const", bufs=1) as cp,
            tc.tile_pool(name="work", bufs=2) as wp,
            tc.tile_pool(name="gath", bufs=2) as gp,
            tc.tile_pool(name="psB", bufs=1, space="PSUM") as psB,
            tc.tile_pool(name="psC", bufs=2, space="PSUM") as psC,
            tc.tile_pool(name="psAgg", bufs=1, space="PSUM") as psAgg,
        ):
            # ---- gather the packed weights across cores ----
            bb = blob[0:1, :]
            nc.gpsimd.dma_start(
                out=wps_bin[:, :],
                in_=bb.bitcast(bf16)[0:1, O_WPS:O_WPS + WSH * 1664]
                    .rearrange("a (p q) -> (a p) q", q=1664))
            nc.gpsimd.collective_compute(
                "AllGather", AL.bypass, replica_groups=RG,
                ins=[wps_bin[:, :].opt()], outs=[wpk_full[:, :].opt()])

            # ---- load constants to SBUF ----
            def ld(src, p, fdim, dtype, pool=cp):
                t = pool.tile([p, fdim], dtype, tag=f"c_{src.name}")
                nc.sync.dma_start(out=t[:, :], in_=src[:, :])
                return t

            wpk_s = ld(wpk_full, 128, 1664, bf16)
            wup0_s = wpk_s[:, 0:128]
            wup1_s = wpk_s[:, 128:256]
            wps_s = wpk_s[:, 256:384]
            wpr_s = wpk_s[:, 384:512]
            w2_s = wpk_s[:, 512:640]
            w3_s = wpk_s[:, 640:1152]
            wo0t_s = wpk_s[:, 1152:1280]
            wo0b_s = wpk_s[:, 1280:1408]
            wo1t_s = wpk_s[:, 1408:1536]
            wo1b_s = wpk_s[:, 1536:1664]
            w1c_s = cp.tile([11, 128], bf16, tag="c_w1c")
            nc.sync.dma_start(
                out=w1c_s[:, :],
                in_=bb.bitcast(bf16)[0:1, O_W1C:O_W1C + 1408]
                    .rearrange("a (p q) -> (a p) q", q=128))
            w1c10 = w1c_s[0:10, :]
            b2_s = cp.tile([1, 128], bf16, tag="c_b2")
            nc.sync.dma_start(
                out=b2_s[:, :],
                in_=bb.bitcast(bf16)[0:1, O_W1C + 1280:O_W1C + 1408])
            idf_s = ld(identf, 128, 128, f32)
            iota_s = ld(iota, 128, 128, f32)
            idb_s = cp.tile([128, 128], bf16, tag="c_idb")
            nc.vector.tensor_copy(out=idb_s[:, :], in_=idf_s[:, :])
            ones_s = cp.tile([1, 128], bf16, tag="c_ones")
            nc.vector.memset(ones_s[:, :], 1.0)
            ix16_s = cp.tile([128, NT * B_pad], i16, tag="c_ix16")
            nc.sync.dma_start(
                out=ix16_s[:, :],
                in_=bb[0:1, 0:O_IDXT].rearrange("a (p q) -> (a p) q",
                                                q=NT * B_pad))
            ixs_s = cp.tile([128, NT * B_pad], i32, tag="c_ixs")
            nc.vector.tensor_copy(out=ixs_s[:, :], in_=ix16_s[:, :])
            ixt_s = cp.tile([128, NT], i32, tag="c_ixt")
            nc.sync.dma_start(
                out=ixt_s[:, :],
                in_=bb.bitcast(i32)[0:1, O_IDXT // 2:O_IDXT // 2 + 128 * NT]
                    .rearrange("a (p q) -> (a p) q", q=NT))
            nsc_s = cp.tile([128, NSH // 128], f32, tag="c_nsc")
            nc.sync.dma_start(
                out=nsc_s[:, :],
                in_=bb.bitcast(f32)[0:1,
                                    O_NSC // 2:O_NSC // 2 + 128 * (NSH // 128)]
                    .rearrange("a (p q) -> (a p) q", q=NSH // 128))

            # ---- node-table phase (this core's 1/8 shard) ----
            with tc.tile_pool(name="nodes", bufs=1) as npool:
                xts = []
                for k in range(4):
                    x8 = npool.tile([128, NSH], dt.int8, tag=f"n8_{k}")
                    nc.sync.dma_start(
                        out=x8[:, :], in_=nfs[128 * k:128 * (k + 1), :])
                    xb = npool.tile([128, NSH], bf16, tag=f"nb_{k}")
                    nc.vector.tensor_copy(out=xb[:, :], in_=x8[:, :])
                    xts.append(xb)
                x0t, x1t0, x1t1, x1t2 = xts
                for s in range(NSH // 128):
                    sl = slice(128 * s, 128 * (s + 1))
                    pn = psAgg.tile([128, 1024], f32, tag="agg")
                    mms = [
                        (x0t, wup0_s, 0),      # h0
                        (x1t0, wup1_s, 128),   # h1_0
                        (x1t1, wup1_s, 256),
                        (x1t2, wup1_s, 384),
                        (x0t, wps_s, 512),     # P_s
                        (x0t, wpr_s, 640),     # P_r
                    ]
                    for lhs, rhs, o in mms:
                        nc.tensor.matmul(out=pn[:, o:o + 128], lhsT=lhs[:, sl],
                                         rhs=rhs, start=True, stop=True)
                    tsb = wp.tile([128, 768], bf16, tag="tsb")
                    nc.scalar.activation(out=tsb[:, :], in_=pn[:, 0:768],
                                         func=AF.Copy, scale=nsc_s[:, s:s + 1])
                    nc.sync.dma_start(out=T_s_part[sl, :], in_=tsb[:, :])

            # ---- all-gather the node tables across the 8 cores ----
            nc.gpsimd.collective_compute(
                "AllGather", AL.bypass, replica_groups=RG,
                ins=[T_s_part[:, :].opt()], outs=[T_s[:, :].opt()])

            # ---- edge phase (hardware loop over node tiles) ----
            BB = 4  # blocks per batch-group
            with tc.For_i(0, NT, 1) as t:
                # stage this tile's gather offsets into fixed tiles so the
                # indirect DMAs see static access patterns
                ixcur = wp.tile([128, B_pad + 1], i32, tag="ixcur")
                nc.vector.tensor_copy(out=ixcur[:, 0:B_pad],
                                      in_=ixs_s[:, ds(t * B_pad, B_pad)])
                nc.vector.tensor_copy(out=ixcur[:, B_pad:B_pad + 1],
                                      in_=ixt_s[:, ds(t, 1)])
                gs_t = gp.tile([128, B_pad * 768], bf16, tag="gs")
                for b in range(B_pad):
                    nc.gpsimd.indirect_dma_start(
                        out=gs_t[:, 768 * b:768 * (b + 1)], out_offset=None,
                        in_=T_s[:, :],
                        in_offset=bass.IndirectOffsetOnAxis(
                            ap=ixcur[:, b:b + 1], axis=0))
                prt = gp.tile([128, 128], bf16, tag="prt")
                nc.gpsimd.indirect_dma_start(
                    out=prt[:, :], out_offset=None, in_=T_s[:, :],
                    in_offset=bass.IndirectOffsetOnAxis(
                        ap=ixcur[:, B_pad:B_pad + 1], axis=0),
                    element_offset=640)
                tl8_t = wp.tile([10, EPT], fp8, tag="tail8")
                nc.sync.dma_start(out=tl8_t[:, :], in_=tail10[ds(t * 10, 10), :])
                tl_t = wp.tile([10, EPT], bf16, tag="tail")
                nc.vector.tensor_copy(out=tl_t[:, :], in_=tl8_t[:, :])
                yb_t = wp.tile([128, 5 * B_pad], bf16, tag="yrlb")
                nc.sync.dma_start(out=yb_t[:, :], in_=yrl[ds(t * 128, 128), :])
                y_t = wp.tile([128, 5 * B_pad], f32, tag="yrl")
                nc.vector.tensor_copy(out=y_t[:, :], in_=yb_t[:, :])

                # selection matrices (f32 master for the PE transpose)
                sp_f = wp.tile([128, B_pad * 128], f32, tag="spf")
                rl3 = y_t[:, 4 * B_pad:5 * B_pad].unsqueeze(2)
                nc.vector.tensor_tensor(
                    out=sp_f[:, :].rearrange("p (b n) -> p b n", n=128),
                    in0=rl3.to_broadcast([128, B_pad, 128]),
                    in1=iota_s[:, :].unsqueeze(1).to_broadcast(
                        [128, B_pad, 128]),
                    op=AL.is_equal)
                sp_t = wp.tile([128, B_pad * 128], bf16, tag="spl")
                nc.vector.tensor_copy(out=sp_t[:, :], in_=sp_f[:, :])
                # transposed one-hot (node-partition) built on-device
                st_t = gp.tile([128, EPT], bf16, tag="stT")
                for q0 in range(0, B_pad, 4):
                    qn = min(4, B_pad - q0)
                    pq = psC.tile([128, 512], f32, tag="ptw")
                    for j in range(qn):
                        nc.tensor.transpose(
                            out=pq[:, 128 * j:128 * (j + 1)],
                            in_=sp_f[:, 128 * (q0 + j):128 * (q0 + j + 1)],
                            identity=idf_s[:, :])
                    nc.scalar.activation(
                        out=st_t[:, 128 * q0:128 * (q0 + qn)],
                        in_=pq[:, :128 * qn], func=AF.Copy)

                agg = psAgg.tile([128, 1024], f32, tag="agg")

                nb_groups = (B_pad + BB - 1) // BB
                for g in range(nb_groups):
                    b0 = g * BB
                    gsz = min(BB, B_pad - b0)
                    p1 = psB.tile([128, 128 * BB], f32, tag="p1")
                    for bi in range(gsz):
                        b = b0 + bi
                        o = 128 * bi
                        nc.tensor.matmul(out=p1[:, o:o + 128],
                                         lhsT=tl_t[:, 128 * b:128 * (b + 1)],
                                         rhs=w1c10, start=True, stop=False)
                        nc.tensor.matmul(out=p1[:, o:o + 128], lhsT=idb_s[:, :],
                                         rhs=gs_t[:, 768 * b + 512:768 * b + 640],
                                         start=False, stop=False)
                        nc.tensor.matmul(out=p1[:, o:o + 128],
                                         lhsT=st_t[:, 128 * b:128 * (b + 1)],
                                         rhs=prt[:, :],
                                         start=False, stop=True)
                    h1 = wp.tile([128, 128 * BB], f32, tag="h1")
                    if USE_SILU:
                        nc.scalar.activation(out=h1[:, :128 * gsz],
                                             in_=p1[:, :128 * gsz], func=AF.Silu)
                    else:
                        sg1 = wp.tile([128, 128 * BB], f32, tag="sg1")
                        nc.scalar.activation(out=sg1[:, :128 * gsz],
                                             in_=p1[:, :128 * gsz], func=AF.Sigmoid)
                        nc.vector.tensor_tensor(out=h1[:, :128 * gsz],
                                                in0=p1[:, :128 * gsz],
                                                in1=sg1[:, :128 * gsz], op=AL.mult)
                    pt1 = psB.tile([128, 128 * BB], f32, tag="pt1")
                    for bi in range(gsz):
                        o = 128 * bi
                        nc.tensor.transpose(out=pt1[:, o:o + 128],
                                            in_=h1[:, o:o + 128], identity=idf_s[:, :])
                    h1t = wp.tile([128, 128 * BB], bf16, tag="h1t")
                    nc.scalar.activation(out=h1t[:, :128 * gsz],
                                         in_=pt1[:, :128 * gsz], func=AF.Copy)

                    p2 = psB.tile([128, 128 * BB], f32, tag="p2")
                    for bi in range(gsz):
                        o = 128 * bi
                        nc.tensor.matmul(out=p2[:, o:o + 128], lhsT=h1t[:, o:o + 128],
                                         rhs=w2_s, start=True, stop=False)
                        nc.tensor.matmul(out=p2[:, o:o + 128], lhsT=ones_s[:, :],
                                         rhs=b2_s[:, :], start=False, stop=True)
                    h2 = wp.tile([128, 128 * BB], f32, tag="h2")
                    if USE_SILU:
                        nc.scalar.activation(out=h2[:, :128 * gsz],
                                             in_=p2[:, :128 * gsz], func=AF.Silu)
                    else:
                        sg2 = wp.tile([128, 128 * BB], f32, tag="sg2")
                        nc.scalar.activation(out=sg2[:, :128 * gsz],
                                             in_=p2[:, :128 * gsz], func=AF.Sigmoid)
                        nc.vector.tensor_tensor(out=h2[:, :128 * gsz],
                                                in0=p2[:, :128 * gsz],
                                                in1=sg2[:, :128 * gsz], op=AL.mult)
                    pt2 = psB.tile([128, 128 * BB], f32, tag="pt2")
                    for bi in range(gsz):
                        o = 128 * bi
                        nc.tensor.transpose(out=pt2[:, o:o + 128],
                                            in_=h2[:, o:o + 128], identity=idf_s[:, :])
                    h2t = wp.tile([128, 128 * BB], bf16, tag="h2t")
                    nc.scalar.activation(out=h2t[:, :128 * gsz],
                                         in_=pt2[:, :128 * gsz], func=AF.Copy)

                    for bi in range(gsz):
                        b = b0 + bi
                        o = 128 * bi
                        ptw = psC.tile([128, 512], f32, tag="ptw")
                        nc.tensor.matmul(out=ptw[:, :], lhsT=h2t[:, o:o + 128],
                                         rhs=w3_s, start=True, stop=True)
                        tpw = wp.tile([128, 512], bf16, tag="tpw")
                        nc.scalar.activation(out=tpw[:, :], in_=ptw[:, :],
                                             func=AF.Copy)

                        xs0 = gs_t[:, 768 * b:768 * b + 128]
                        xs1 = gs_t[:, 768 * b + 128:768 * b + 512]
                        y0 = y_t[:, b:b + 1]
                        y13b = (y_t[:, B_pad + b:4 * B_pad:B_pad]
                                .unsqueeze(2)
                                .to_broadcast([128, 3, 128]))
                        # msgs layout: [A(128) | D(128) | B(384) | C(384)]
                        msgs = wp.tile([128, 1024], bf16, tag="msgs")
                        # A = xs0*wA*y0
                        nc.vector.tensor_tensor(out=msgs[:, 0:128], in0=xs0,
                                                in1=tpw[:, 0:128], op=AL.mult)
                        nc.scalar.activation(out=msgs[:, 0:128],
                                             in_=msgs[:, 0:128],
                                             func=AF.Copy, scale=y0)
                        # D = (sum_i xs1_i*y1_i) * wD
                        pd = wp.tile([128, 384], bf16, tag="pd")
                        nc.vector.tensor_tensor(
                            out=pd[:, :].rearrange("p (f n) -> p f n", f=3),
                            in0=xs1.rearrange("p (f n) -> p f n", f=3),
                            in1=y13b, op=AL.mult)
                        dd = wp.tile([128, 128], f32, tag="dd")
                        nc.vector.tensor_reduce(
                            out=dd[:, :],
                            in_=pd[:, :].rearrange("p (f n) -> p n f", f=3),
                            axis=mybir.AxisListType.X, op=AL.add)
                        nc.vector.tensor_tensor(out=msgs[:, 128:256],
                                                in0=dd[:, :],
                                                in1=tpw[:, 128:256], op=AL.mult)
                        # B_i = xs0*wB*y1_i
                        pb = wp.tile([128, 128], bf16, tag="pb")
                        nc.vector.tensor_tensor(out=pb[:, :], in0=xs0,
                                                in1=tpw[:, 256:384], op=AL.mult)
                        nc.vector.tensor_tensor(
                            out=msgs[:, 256:640].rearrange(
                                "p (f n) -> p f n", f=3),
                            in0=pb[:, :].unsqueeze(1).to_broadcast(
                                [128, 3, 128]),
                            in1=y13b, op=AL.mult)
                        # C_i = xs1_i*wC*y0
                        wc3 = tpw[:, 384:512].unsqueeze(1).to_broadcast(
                            [128, 3, 128])
                        nc.vector.tensor_tensor(
                            out=msgs[:, 640:1024].rearrange(
                                "p (f n) -> p f n", f=3),
                            in0=xs1.rearrange("p (f n) -> p f n", f=3),
                            in1=wc3, op=AL.mult)
                        nc.scalar.activation(out=msgs[:, 640:1024],
                                             in_=msgs[:, 640:1024],
                                             func=AF.Copy, scale=y0)

                        # node-partitioned scatter: agg[n, X] += sum_e sp[e,n]*msgs[e,X]
                        lastb = (b == B_pad - 1)
                        sp_b = sp_t[:, 128 * b:128 * (b + 1)]
                        nc.tensor.matmul(out=agg[:, 0:512], lhsT=sp_b,
                                         rhs=msgs[:, 0:512],
                                         start=(b == 0), stop=lastb,
                                         skip_group_check=True)
                        nc.tensor.matmul(out=agg[:, 512:1024], lhsT=sp_b,
                                         rhs=msgs[:, 512:1024],
                                         start=(b == 0), stop=lastb,
                                         skip_group_check=True)

                # ---- transpose aggregate back to channel-partition ----
                aggc = wp.tile([128, 1024], f32, tag="aggc")
                nc.scalar.activation(out=aggc[:, :], in_=agg[:, :], func=AF.Copy)
                aggT = wp.tile([128, 1024], bf16, tag="aggT")
                for h in range(2):
                    pq = psC.tile([128, 512], f32, tag="ptw")
                    for j in range(4):
                        nc.tensor.transpose(
                            out=pq[:, 128 * j:128 * (j + 1)],
                            in_=aggc[:, 512 * h + 128 * j:512 * h + 128 * (j + 1)],
                            identity=idf_s[:, :])
                    nc.scalar.activation(out=aggT[:, 512 * h:512 * (h + 1)],
                                         in_=pq[:, :], func=AF.Copy)

                # ---- final linear for this node tile ----
                # aggT chunks: [A | D | B0 B1 B2 | C0 C1 C2], each [m=128, n=128]
                pf = psC.tile([128, 512], f32, tag="ptw")
                nc.tensor.matmul(out=pf[:, 0:128], lhsT=aggT[:, 0:128],
                                 rhs=wo0t_s, start=True, stop=False,
                                 skip_group_check=True)
                nc.tensor.matmul(out=pf[:, 0:128], lhsT=aggT[:, 128:256],
                                 rhs=wo0b_s, start=False, stop=True,
                                 skip_group_check=True)
                for i in range(3):
                    o = 128 * (i + 1)
                    nc.tensor.matmul(out=pf[:, o:o + 128],
                                     lhsT=aggT[:, 256 + 128 * i:384 + 128 * i],
                                     rhs=wo1t_s, start=True, stop=False,
                                     skip_group_check=True)
                    nc.tensor.matmul(out=pf[:, o:o + 128],
                                     lhsT=aggT[:, 640 + 128 * i:768 + 128 * i],
                                     rhs=wo1b_s, start=False, stop=True,
                                     skip_group_check=True)
                # int8 output with per-node scale + round-to-nearest
                mx = wp.tile([128, 1], f32, tag="mx")
                nc.vector.tensor_reduce(out=mx[:, :], in_=pf[:, :],
                                        axis=mybir.AxisListType.X, op=AL.max,
                                        apply_absolute_value=True)
                nc.vector.tensor_scalar(out=mx[:, :], in0=mx[:, :],
                                        scalar1=1e-20, scalar2=None,
                                        op0=AL.max)
                inv = wp.tile([128, 1], f32, tag="inv")
                nc.vector.reciprocal(out=inv[:, :], in_=mx[:, :])
                nc.vector.tensor_scalar(out=inv[:, :], in0=inv[:, :],
                                        scalar1=127.0, scalar2=None,
                                        op0=AL.mult)
                qf = wp.tile([128, 512], f32, tag="qf")
                nc.scalar.activation(out=qf[:, :], in_=pf[:, :],
                                     func=AF.Copy, scale=inv[:, :])
                outs = wp.tile([128, 512], dt.int8, tag="outs")
                ov = outs[:, :].rearrange("p (m c) -> p m c", c=4)
                for c4 in range(4):
                    nc.vector.tensor_copy(out=ov[:, :, c4],
                                          in_=qf[:, 128 * c4:128 * (c4 + 1)])
                nc.sync.dma_start(out=out_d[ds(t * 128, 128), :],
                                  in_=outs[:, :])
                nc.sync.dma_start(out=out_sc[:, ds(t, 1)], in_=mx[:, :])

    nc.compile()
    return nc


# --------------------------------------------------------------------------
# Entry point
# --------------------------------------------------------------------------

def kernel(**inputs):
    B_pad, in_maps = _host_prep(inputs)
    if B_pad not in _compiled:
        _compiled[B_pad] = _build(B_pad)
    nc = _compiled[B_pad]

    if os.environ.get("BASS_KERNEL_SIM"):
        from concourse.bass_interp import MultiCoreSim
        sim = MultiCoreSim(nc, NCORES)
        for c in range(NCORES):
            for k, v in in_maps[c].items():
                sim.cores[c].tensor(k)[:] = v
        sim.simulate(check_with_hw=False)
        outs = [np.array(sim.cores[c].tensor("out_dram")) for c in range(NCORES)]
        scs = [np.array(sim.cores[c].tensor("out_sc")) for c in range(NCORES)]
    else:
        res = run_bass_kernel_spmd(nc, in_maps, list(range(NCORES)))
        outs = [res.results[c]["out_dram"] for c in range(NCORES)]
        scs = [res.results[c]["out_sc"] for c in range(NCORES)]

    full = np.zeros((N, MUL, 4), np.float32)
    for c in range(NCORES):
        sc = np.asarray(scs[c], np.float32).T.reshape(NT * 128)[:NPC]
        q = np.asarray(outs[c][:NPC], np.float32) * (sc / 127.0)[:, None]
        full[c * NPC:(c + 1) * NPC] = q.reshape(NPC, MUL, 4)
    return full
